# revision 19
# baseline (speedup 1.0000x reference)
"""Trainium2 Bass kernel for ContrastiveNet loss (v6).

Per core k of 8 (SPMD):
  - host: xn = x/||x||*S (S=32) in fp8e4 -> sim = G/(S^2*TEMP), no on-device
    normalization. Rows rolled so core k's 512 anchor rows are rotated cols
    0..511. y is COLUMN-CHUNK-major ([8][128][16][512]) so each chunk's gram
    (4 row-tiles x 8 kp fp8 DoubleRow matmuls into [128,512] PSUM) starts as
    the chunk lands; chunk 7 is dead last in the DMA stream.
  - DMA is the wall (~37us saturated): all scatter planes / masks are batched
    single transfers interleaved so nothing but chunk 7 is late.
  - gather: per (tile, piece 1024/1024/1024/512/512) gpsimd local_scatter
    with HBM col->slot planes; values accumulate in-place (fp16).
    2-member duplicate (row,col) groups are fixed by scatter passes:
    main pass (rep col in pieces 0-2; full NE, runs mid-stream), piece-3 and
    piece-4 passes (narrow: their pairs are ranked into the lowest slots).
    Pairs touching >=3-member groups (~2%) are masked and done on host.
  - loss: exp(scale*x) per tile on ACT (table preloaded; single Ln at the
    end, its table load hidden behind the last den reduce), per-pair den
    reduce, masked (ln den - scale*l0) accumulation -> [1,1] per core.
  - PE p-state: tiny warm matmuls bridge DMA-paced idle gaps so real grams
    stay at full clock.
"""
import os
import sys
import numpy as np
import ml_dtypes

try:
    import concourse  # noqa: F401
except ImportError:
    sys.path.insert(0, "/opt/trn_rl_repo")

from contextlib import ExitStack

import concourse.bass as bass
import concourse.tile as tile
from concourse import bacc, mybir
from concourse._compat import with_exitstack
from concourse.bass_utils import run_bass_kernel_spmd

F16 = np.float16
FP8 = ml_dtypes.float8_e4m3
F32 = mybir.dt.float32
DF16 = mybir.dt.float16
F8 = mybir.dt.float8e4
I16 = mybir.dt.int16

B, D, J = 4096, 2048, 11
NCORES, RPC, NT, NKP = 8, 512, 4, 8
NCH, CW = 8, 512                    # gram column chunks (per core)
POFF = [0, 1024, 2048, 3072, 3584]  # scatter piece offsets
PW = [1024, 1024, 1024, 512, 512]   # scatter piece widths
NP = 5
TEMP = 0.1
S = 32.0
KSC = 1.0 / (S * S * TEMP)
# warm matmuls ([128,64] each, ~75ns) issued after each chunk's grams to keep
# the PE p-state ramped across DMA-paced idle gaps
WARMS = [0, 0, 20, 28, 28, 18, 20, 14]
AF = mybir.ActivationFunctionType
ALU = mybir.AluOpType
AX = mybir.AxisListType
DR = mybir.MatmulPerfMode.DoubleRow


def _even(n):
    return n + (n % 2)


# ---------------------------------------------------------------- host prep
def build_plan(anchor_idx, pos_idx, neg_idx):
    r = anchor_idx.astype(np.int64)
    cols = np.concatenate([pos_idx[:, None], neg_idx], axis=1).astype(np.int64)
    P = r.shape[0]

    er = np.repeat(r, J)
    ec = cols.ravel()
    pair_of = np.repeat(np.arange(P), J)
    core = er // RPC
    t = (er % RPC) // 128
    pp = er % 128
    ec_rot = (ec - core * RPC) % B
    piece = np.searchsorted(POFF, ec_rot, side="right") - 1

    key = er * B + ec
    o2 = np.argsort(key, kind="stable")
    k_sorted = key[o2]
    first2 = np.r_[True, k_sorted[1:] != k_sorted[:-1]]
    gid_sorted = np.cumsum(first2) - 1
    NG = int(gid_sorted[-1]) + 1
    gid = np.empty(P * J, np.int64)
    gid[o2] = gid_sorted
    gsz_g = np.bincount(gid_sorted, minlength=NG)
    gsz = gsz_g[gid]

    # pairs containing any entry of a >=3-member group -> host, masked out
    bad_pairs = np.unique(pair_of[gsz >= 3])
    badp = np.zeros(P, bool)
    badp[bad_pairs] = True

    # 2-member groups: rep = member in the earliest piece
    two = gsz == 2
    order = np.lexsort((np.arange(P * J), piece, gid))
    go = order[two[order]]
    g_of_go = gid[go]
    firstg = np.r_[True, g_of_go[1:] != g_of_go[:-1]]
    rep = go[firstg]
    oth = go[~firstg]
    rep_bad = badp[pair_of[rep]]
    oth_bad = badp[pair_of[oth]]
    swap = rep_bad & ~oth_bad       # partner becomes the plane entry
    pk = ~rep_bad & ~oth_bad        # device dup pass only if both pairs live
    rep_p, oth_p = rep[pk], oth[pk]
    grp_piece = piece[rep_p]        # min piece of the group
    narrow_g = grp_piece >= 3       # piece-3 / piece-4 passes

    # ---- pair ranking: pairs touching narrow-pass groups come first
    narrow_pairs = np.unique(np.r_[pair_of[rep_p[narrow_g]],
                                   pair_of[oth_p[narrow_g]]])
    sev = np.zeros(P, np.int64)
    sev[narrow_pairs] = 1
    order_p = np.lexsort((np.arange(P), -sev, r))
    r_sp = r[order_p]
    firstp = np.r_[True, r_sp[1:] != r_sp[:-1]]
    gidp = np.cumsum(firstp) - 1
    rank_sorted = np.arange(P) - np.flatnonzero(firstp)[gidp]
    srank = np.empty(P, np.int64)
    srank[order_p] = rank_sorted

    n_per_row = np.bincount(r, minlength=B)
    SP = int(max(n_per_row.max(), 1))
    NE = _even(SP * J)
    assert NE * 32 < 2**16
    eslot = srank[pair_of] * J + np.tile(np.arange(J), P)

    cnt_n = np.bincount(r[narrow_pairs], minlength=B) if len(narrow_pairs) \
        else np.zeros(B, np.int64)
    WN = _even(min(int(cnt_n.max()) * J + 2, NE)) if cnt_n.max() > 0 else 0

    # ---- main scatter plane: col -> slot of occ0 entries (incl. singletons)
    is_rep = np.ones(P * J, bool)
    is_rep[oth] = False
    is_rep[oth[swap]] = True
    m0 = is_rep & ~badp[pair_of]
    plane = np.full((NCORES, NT, 128, B), -1, np.int16)
    plane[core[m0], t[m0], pp[m0], ec_rot[m0]] = eslot[m0].astype(np.int16)
    # per-level layout [NCORES][128][NT][PW] for single-DMA loads
    plane_lv = [np.ascontiguousarray(
        plane[:, :, :, POFF[pc]:POFF[pc] + PW[pc]].transpose(0, 2, 1, 3))
        for pc in range(NP)]

    # ---- dup passes (source occ0 slot -> dup slot)
    e_rep, e_oth = eslot[rep_p], eslot[oth_p]

    def mk_pass(mask, W):
        if not mask.any():
            return None
        pl = np.full((NCORES, 128, NT, W), -1, np.int16)
        pl[core[rep_p[mask]], pp[rep_p[mask]], t[rep_p[mask]],
           e_rep[mask]] = e_oth[mask].astype(np.int16)
        return pl

    main_g = grp_piece <= 2
    if narrow_g.any():
        assert (e_rep[narrow_g] < WN).all() and (e_oth[narrow_g] < WN).all()
    pass_main = mk_pass(main_g, NE)
    pass_p3 = mk_pass(grp_piece == 3, WN) if WN else None
    pass_p4 = mk_pass(grp_piece == 4, WN) if WN else None

    nmat = n_per_row.reshape(NCORES, NT, 128)
    pairmask = (np.arange(SP)[None, None, None, :] < nmat[..., None]).astype(F16)
    bp = bad_pairs
    pairmask[r[bp] // RPC, (r[bp] % RPC) // 128, r[bp] % 128, srank[bp]] = 0
    pairmask = np.ascontiguousarray(pairmask.transpose(0, 2, 1, 3))  # [C,128,NT,SP]

    return dict(plane_lv=plane_lv, pass_main=pass_main, pass_p3=pass_p3,
                pass_p4=pass_p4, pairmask=pairmask, SP=SP, NE=NE, WN=WN,
                bad_pairs=bad_pairs)


# ------------------------------------------------------------- device kernel
@with_exitstack
def _build(ctx: ExitStack, tc: "tile.TileContext", io: dict, SP: int, NE: int,
           WN: int, have_main: bool, have_p3: bool, have_p4: bool):
    nc = tc.nc
    y_d, out_d = io["y8"], io["out"]

    consts = ctx.enter_context(tc.tile_pool(name="consts", bufs=1))
    ones_f32c = consts.tile([128, 1], F32, tag="ones_f32c")
    nc.vector.memset(ones_f32c[:], 1.0)
    wz = consts.tile([128, 2, 128], F8, tag="wz")
    nc.vector.memset(wz[:], 0.0)

    ypool = ctx.enter_context(tc.tile_pool(name="y", bufs=1))
    y = ypool.tile([128, NCH, 2 * NKP, CW], F8, tag="y", name="y")

    gpool = ctx.enter_context(tc.tile_pool(name="gbf", bufs=1))
    gbf = {tt: gpool.tile([128, B], DF16, tag=f"gbf{tt}", name=f"gbf{tt}")
           for tt in range(NT)}
    plpool = ctx.enter_context(tc.tile_pool(name="plane", bufs=1))
    pl = {pc: plpool.tile([128, NT, PW[pc]], I16, tag=f"plv{pc}",
                          name=f"plv{pc}") for pc in range(NP)}
    papool = ctx.enter_context(tc.tile_pool(name="passes", bufs=1))
    pam = papool.tile([128, NT, NE], I16, tag="pam", name="pam") if have_main else None
    pa3 = papool.tile([128, NT, WN], I16, tag="pa3", name="pa3") if have_p3 else None
    pa4 = papool.tile([128, NT, WN], I16, tag="pa4", name="pa4") if have_p4 else None

    lpool = ctx.enter_context(tc.tile_pool(name="loss", bufs=1))
    pmall = lpool.tile([128, NT, SP], DF16, tag="pmall")

    # ---- DMA: chunk 7 dead last, everything else just-in-time
    nc.sync.dma_start(y[:, 0], y_d[0])
    nc.sync.dma_start(y[:, 1], y_d[1])
    nc.sync.dma_start(pl[0][:], io["plane0"][:])
    nc.sync.dma_start(y[:, 2], y_d[2])
    nc.sync.dma_start(pl[1][:], io["plane1"][:])
    nc.sync.dma_start(y[:, 3], y_d[3])
    nc.sync.dma_start(pl[2][:], io["plane2"][:])
    nc.sync.dma_start(y[:, 4], y_d[4])
    if have_main:
        nc.sync.dma_start(pam[:], io["passm"][:])
    nc.sync.dma_start(y[:, 5], y_d[5])
    if have_p3:
        nc.sync.dma_start(pa3[:], io["pass3"][:])
    if have_p4:
        nc.sync.dma_start(pa4[:], io["pass4"][:])
    nc.sync.dma_start(pmall[:].rearrange("p t s -> p (t s)"),
                      io["pm"][:].rearrange("p t s -> p (t s)"))
    nc.sync.dma_start(pl[3][:], io["plane3"][:])
    nc.sync.dma_start(y[:, 6], y_d[6])
    nc.sync.dma_start(pl[4][:], io["plane4"][:])
    nc.sync.dma_start(y[:, 7], y_d[7])

    dpool = ctx.enter_context(tc.tile_pool(name="dq", bufs=2))
    hpool = ctx.enter_context(tc.tile_pool(name="hacc", bufs=1))
    expool = ctx.enter_context(tc.tile_pool(name="extra", bufs=2))
    elpool = ctx.enter_context(tc.tile_pool(name="elb", bufs=2))
    hacc = {tt: hpool.tile([128, NE], DF16, tag=f"hacc{tt}", name=f"hacc{tt}")
            for tt in range(NT)}
    denall = lpool.tile([128, NT, SP], F32, tag="denall")
    l0all = lpool.tile([128, NT, SP], DF16, tag="l0all")

    # preload the Exp activation table during the initial DMA idle
    pre = elpool.tile([128, 1], F32, tag="pre")
    nc.scalar.activation(pre[:], ones_f32c[:], AF.Exp)

    dq = {}

    def scatter_piece(pc, tiles=range(NT)):
        for tt in tiles:
            d = dpool.tile([128, NE], DF16, tag=f"d{tt}", name=f"d{tt}_{pc}")
            dq[(tt, pc)] = d
            nc.gpsimd.local_scatter(
                d[:], gbf[tt][:, POFF[pc]:POFF[pc] + PW[pc]],
                pl[pc][:, tt, :], 128, NE, PW[pc])

    with tc.tile_pool(name="gpsum", bufs=1, space="PSUM") as gpsum:
        for c in range(NCH):
            for tt in range(NT):
                ps = gpsum.tile([128, CW], F32, tag=f"ps{tt}_{c % 2}",
                                name=f"ps{tt}_{c}")
                for kp in range(NKP):
                    nc.tensor.matmul(
                        ps[:],
                        lhsT=y[:, 0, 2 * kp:2 * kp + 2, tt * 128:(tt + 1) * 128],
                        rhs=y[:, c, 2 * kp:2 * kp + 2, :],
                        start=(kp == 0), stop=(kp == NKP - 1),
                        perf_mode=DR,
                    )
                dst = gbf[tt][:, c * CW:(c + 1) * CW]
                if (c * NT + tt) % 2 == 0:
                    nc.vector.tensor_copy(dst, ps[:])
                else:
                    nc.scalar.copy(dst, ps[:])
            # p-state bridge across the DMA-paced idle before the next chunk
            if WARMS[c]:
                wps = gpsum.tile([128, CW], F32, tag=f"ps0_{(c + 1) % 2}",
                                 name=f"warm{c}")
                for i in range(WARMS[c]):
                    nc.tensor.matmul(wps[:, 0:64], lhsT=wz[:],
                                     rhs=wz[:, :, 0:64],
                                     start=True, stop=True, perf_mode=DR)
            if c == 1:
                scatter_piece(0)
            elif c == 3:
                scatter_piece(1)
                for tt in range(NT):
                    nc.vector.tensor_tensor(hacc[tt][:], dq[(tt, 0)][:],
                                            dq[(tt, 1)][:], ALU.add)
            elif c == 5:
                scatter_piece(2)
                for tt in range(NT):
                    nc.vector.tensor_tensor(hacc[tt][:], hacc[tt][:],
                                            dq[(tt, 2)][:], ALU.add)
                if have_main:
                    for tt in range(NT):
                        e = expool.tile([128, NE], DF16, tag=f"eAm{tt % 2}",
                                        name=f"eAm{tt}")
                        nc.gpsimd.local_scatter(e[:], hacc[tt][:],
                                                pam[:, tt, :], 128, NE, NE)
                        nc.vector.tensor_tensor(hacc[tt][:], hacc[tt][:],
                                                e[:], ALU.add)
            elif c == 6:
                scatter_piece(3)
                for tt in range(NT):
                    nc.vector.tensor_tensor(hacc[tt][:], hacc[tt][:],
                                            dq[(tt, 3)][:], ALU.add)
                    if have_p3:
                        e = expool.tile([128, WN], DF16, tag=f"eA3{tt % 2}",
                                        name=f"eA3{tt}")
                        nc.gpsimd.local_scatter(e[:], hacc[tt][:, 0:WN],
                                                pa3[:, tt, :], 128, WN, WN)
                        nc.vector.tensor_tensor(hacc[tt][:, 0:WN],
                                                hacc[tt][:, 0:WN], e[:],
                                                ALU.add)

        # ---- tail: piece-4 scatters, narrow late pass, exp/den per tile
        for tt in range(NT):
            scatter_piece(4, [tt])
            nc.vector.tensor_tensor(hacc[tt][:], hacc[tt][:], dq[(tt, 4)][:],
                                    ALU.add)
            if have_p4:
                e = expool.tile([128, WN], DF16, tag=f"eA4{tt % 2}",
                                name=f"eA4{tt}")
                nc.gpsimd.local_scatter(e[:], hacc[tt][:, 0:WN],
                                        pa4[:, tt, :], 128, WN, WN)
                nc.vector.tensor_tensor(hacc[tt][:, 0:WN], hacc[tt][:, 0:WN],
                                        e[:], ALU.add)
            ebuf = elpool.tile([128, NE], F32, tag="ebuf")
            nc.scalar.activation(ebuf[:], hacc[tt][:], AF.Exp, scale=KSC)
            e3 = ebuf[:, 0:SP * J].rearrange("p (s j) -> p s j", j=J)
            nc.vector.tensor_reduce(denall[:, tt, :], e3, AX.X, ALU.add)
            l0 = hacc[tt][:, 0:SP * J].rearrange("p (s j) -> p s j", j=J)[:, :, 0]
            nc.vector.tensor_copy(l0all[:, tt, :], l0)

    # ---- batched tail: Ln (table load hidden behind last den), diff, mask
    with tc.tile_pool(name="p5psum", bufs=1, space="PSUM") as p5psum:
        pre2 = elpool.tile([128, 1], F32, tag="pre2")
        nc.scalar.activation(pre2[:], ones_f32c[:], AF.Ln)
        lnd = lpool.tile([128, NT * SP], F32, tag="lnd")
        nc.scalar.activation(lnd[:], denall[:].rearrange("p t s -> p (t s)"),
                             AF.Ln)
        diff = lpool.tile([128, NT * SP], F32, tag="diff")
        nc.vector.scalar_tensor_tensor(
            diff[:], l0all[:].rearrange("p t s -> p (t s)"), -KSC, lnd[:],
            ALU.mult, ALU.add)
        scrap = lpool.tile([128, NT * SP], F32, tag="scrap")
        acc1 = lpool.tile([128, 1], F32, tag="acc1")
        nc.vector.scalar_tensor_tensor(
            scrap[:], diff[:], 1.0,
            pmall[:].rearrange("p t s -> p (t s)"), ALU.mult, ALU.mult,
            accum_out=acc1[:])
        ps = p5psum.tile([1, 1], F32, tag="ps_out")
        nc.tensor.matmul(ps[:], lhsT=acc1[:], rhs=ones_f32c[:, 0:1],
                         start=True, stop=True)
        res = lpool.tile([1, 1], F32, tag="res")
        nc.scalar.copy(res[:], ps[:])
        nc.sync.dma_start(out_d[:], res[:])


def build_nc(SP, NE, WN, have_main, have_p3, have_p4, enable_asserts=False):
    nc = bacc.Bacc("TRN2", target_bir_lowering=False, debug=False,
                   enable_asserts=enable_asserts, num_devices=NCORES)
    io = {
        "y8": nc.dram_tensor("y8", [NCH, 128, 2 * NKP, CW], F8,
                             kind="ExternalInput").ap(),
        "pm": nc.dram_tensor("pm", [128, NT, SP], DF16,
                             kind="ExternalInput").ap(),
        "out": nc.dram_tensor("out", [1, 1], F32, kind="ExternalOutput").ap(),
    }
    for pc in range(NP):
        io[f"plane{pc}"] = nc.dram_tensor(
            f"plane{pc}", [128, NT, PW[pc]], I16, kind="ExternalInput").ap()
    if have_main:
        io["passm"] = nc.dram_tensor("passm", [128, NT, NE], I16,
                                     kind="ExternalInput").ap()
    if have_p3:
        io["pass3"] = nc.dram_tensor("pass3", [128, NT, WN], I16,
                                     kind="ExternalInput").ap()
    if have_p4:
        io["pass4"] = nc.dram_tensor("pass4", [128, NT, WN], I16,
                                     kind="ExternalInput").ap()
    with tile.TileContext(nc) as tc:
        _build(tc, io, SP, NE, WN, have_main, have_p3, have_p4)
    nc.compile()
    return nc


def _normalize(x):
    x = np.asarray(x, np.float32)
    w = np.sqrt((x.astype(np.float64) ** 2).sum(axis=1, keepdims=True))
    w = np.maximum(w, 1e-8)
    return (x / w).astype(np.float32)


def make_in_maps(x, plan):
    xn = _normalize(x)
    x8 = np.clip(xn * S, -240.0, 240.0).astype(FP8)
    in_maps = []
    for k in range(NCORES):
        xr = np.roll(x8, -RPC * k, axis=0)                     # [B, D]
        y8 = xr.T.reshape(2 * NKP, 128, B).transpose(1, 0, 2)  # [128, 16, B]
        y8c = np.ascontiguousarray(
            y8.reshape(128, 2 * NKP, NCH, CW).transpose(2, 0, 1, 3))
        m = {"y8": y8c, "pm": plan["pairmask"][k]}
        for pc in range(NP):
            m[f"plane{pc}"] = plan["plane_lv"][pc][k]
        if plan["pass_main"] is not None:
            m["passm"] = plan["pass_main"][k]
        if plan["pass_p3"] is not None:
            m["pass3"] = plan["pass_p3"][k]
        if plan["pass_p4"] is not None:
            m["pass4"] = plan["pass_p4"][k]
        in_maps.append(m)
    return in_maps


def host_fixup(x, anchor_idx, pos_idx, neg_idx, bad_pairs):
    """Exact loss terms for pairs masked out on the device."""
    if len(bad_pairs) == 0:
        return 0.0
    xn = _normalize(x).astype(np.float64)
    a = anchor_idx[bad_pairs]
    cols = np.concatenate([pos_idx[bad_pairs][:, None], neg_idx[bad_pairs]],
                          axis=1)
    logits = np.einsum("pd,pjd->pj", xn[a], xn[cols]) / TEMP
    mx = logits.max(axis=1, keepdims=True)
    lse = np.log(np.exp(logits - mx).sum(axis=1)) + mx[:, 0]
    return float((lse - logits[:, 0]).sum())


def kernel(**inputs):
    x = np.asarray(inputs["x"], np.float32)
    anchor_idx = np.asarray(inputs["anchor_idx"])
    pos_idx = np.asarray(inputs["pos_idx"])
    neg_idx = np.asarray(inputs["neg_idx"])
    P = anchor_idx.shape[0]

    plan = build_plan(anchor_idx, pos_idx, neg_idx)
    nc = build_nc(plan["SP"], plan["NE"], plan["WN"],
                  plan["pass_main"] is not None, plan["pass_p3"] is not None,
                  plan["pass_p4"] is not None)
    in_maps = make_in_maps(x, plan)
    res = run_bass_kernel_spmd(nc, in_maps, list(range(NCORES)))
    total = sum(float(res.results[k]["out"][0, 0]) for k in range(NCORES))
    total += host_fixup(x, anchor_idx, pos_idx, neg_idx, plan["bad_pairs"])
    return np.float32(total / P)


# revision 20
# speedup vs baseline: 1.0410x; 1.0410x over previous
"""Trainium2 Bass kernel for ContrastiveNet loss (v6).

Per core k of 8 (SPMD):
  - host: xn = x/||x||*S (S=32) in fp8e4 -> sim = G/(S^2*TEMP), no on-device
    normalization. Rows rolled so core k's 512 anchor rows are rotated cols
    0..511. y is COLUMN-CHUNK-major ([8][128][16][512]) so each chunk's gram
    (4 row-tiles x 8 kp fp8 DoubleRow matmuls into [128,512] PSUM) starts as
    the chunk lands; chunk 7 is dead last in the DMA stream.
  - DMA is the wall (~37us saturated): all scatter planes / masks are batched
    single transfers interleaved so nothing but chunk 7 is late.
  - gather: per (tile, piece 1024/1024/1024/512/512) gpsimd local_scatter
    with HBM col->slot planes; values accumulate in-place (fp16).
    2-member duplicate (row,col) groups are fixed by scatter passes:
    main pass (rep col in pieces 0-2; full NE, runs mid-stream), piece-3 and
    piece-4 passes (narrow: their pairs are ranked into the lowest slots).
    Pairs touching >=3-member groups (~2%) are masked and done on host.
  - loss: exp(scale*x) per tile on ACT (table preloaded; single Ln at the
    end, its table load hidden behind the last den reduce), per-pair den
    reduce, masked (ln den - scale*l0) accumulation -> [1,1] per core.
  - PE p-state: tiny warm matmuls bridge DMA-paced idle gaps so real grams
    stay at full clock.
"""
import os
import sys
import numpy as np
import ml_dtypes

try:
    import concourse  # noqa: F401
except ImportError:
    sys.path.insert(0, "/opt/trn_rl_repo")

from contextlib import ExitStack

import concourse.bass as bass
import concourse.tile as tile
from concourse import bacc, mybir
from concourse._compat import with_exitstack
from concourse.bass_utils import run_bass_kernel_spmd

F16 = np.float16
FP8 = ml_dtypes.float8_e4m3
F32 = mybir.dt.float32
DF16 = mybir.dt.float16
F8 = mybir.dt.float8e4
I16 = mybir.dt.int16

B, D, J = 4096, 2048, 11
NCORES, RPC, NT, NKP = 8, 512, 4, 8
NCH, CW = 8, 512                    # gram column chunks (per core)
POFF = [0, 1024, 2048, 3072, 3584]  # scatter piece offsets
PW = [1024, 1024, 1024, 512, 512]   # scatter piece widths
NP = 5
TEMP = 0.1
S = 32.0
KSC = 1.0 / (S * S * TEMP)
# warm matmuls ([128,64] each, ~75ns) issued after each chunk's grams to keep
# the PE p-state ramped across DMA-paced idle gaps
WARMS = [0, 22, 0, 0, 14, 26, 8, 60]
WARM0 = 52
AF = mybir.ActivationFunctionType
ALU = mybir.AluOpType
AX = mybir.AxisListType
DR = mybir.MatmulPerfMode.DoubleRow


def _even(n):
    return n + (n % 2)


# ---------------------------------------------------------------- host prep
def build_plan(anchor_idx, pos_idx, neg_idx):
    r = anchor_idx.astype(np.int64)
    cols = np.concatenate([pos_idx[:, None], neg_idx], axis=1).astype(np.int64)
    P = r.shape[0]

    er = np.repeat(r, J)
    ec = cols.ravel()
    pair_of = np.repeat(np.arange(P), J)
    core = er // RPC
    t = (er % RPC) // 128
    pp = er % 128
    ec_rot = (ec - core * RPC) % B
    piece = np.searchsorted(POFF, ec_rot, side="right") - 1

    key = er * B + ec
    o2 = np.argsort(key, kind="stable")
    k_sorted = key[o2]
    first2 = np.r_[True, k_sorted[1:] != k_sorted[:-1]]
    gid_sorted = np.cumsum(first2) - 1
    NG = int(gid_sorted[-1]) + 1
    gid = np.empty(P * J, np.int64)
    gid[o2] = gid_sorted
    gsz_g = np.bincount(gid_sorted, minlength=NG)
    gsz = gsz_g[gid]

    # pairs containing any entry of a >=3-member group -> host, masked out
    bad_pairs = np.unique(pair_of[gsz >= 3])
    badp = np.zeros(P, bool)
    badp[bad_pairs] = True

    # 2-member groups: rep = member in the earliest piece
    two = gsz == 2
    order = np.lexsort((np.arange(P * J), piece, gid))
    go = order[two[order]]
    g_of_go = gid[go]
    firstg = np.r_[True, g_of_go[1:] != g_of_go[:-1]]
    rep = go[firstg]
    oth = go[~firstg]
    rep_bad = badp[pair_of[rep]]
    oth_bad = badp[pair_of[oth]]
    swap = rep_bad & ~oth_bad       # partner becomes the plane entry
    pk = ~rep_bad & ~oth_bad        # device dup pass only if both pairs live
    rep_p, oth_p = rep[pk], oth[pk]
    grp_piece = piece[rep_p]        # min piece of the group
    narrow_g = grp_piece >= 3       # piece-3 / piece-4 passes

    # ---- pair ranking: pairs touching narrow-pass groups come first
    narrow_pairs = np.unique(np.r_[pair_of[rep_p[narrow_g]],
                                   pair_of[oth_p[narrow_g]]])
    sev = np.zeros(P, np.int64)
    sev[narrow_pairs] = 1
    order_p = np.lexsort((np.arange(P), -sev, r))
    r_sp = r[order_p]
    firstp = np.r_[True, r_sp[1:] != r_sp[:-1]]
    gidp = np.cumsum(firstp) - 1
    rank_sorted = np.arange(P) - np.flatnonzero(firstp)[gidp]
    srank = np.empty(P, np.int64)
    srank[order_p] = rank_sorted

    n_per_row = np.bincount(r, minlength=B)
    SP = int(max(n_per_row.max(), 1))
    NE = _even(SP * J)
    assert NE * 32 < 2**16
    eslot = srank[pair_of] * J + np.tile(np.arange(J), P)

    cnt_n = np.bincount(r[narrow_pairs], minlength=B) if len(narrow_pairs) \
        else np.zeros(B, np.int64)
    WN = _even(min(int(cnt_n.max()) * J + 2, NE)) if cnt_n.max() > 0 else 0

    # ---- main scatter plane: col -> slot of occ0 entries (incl. singletons)
    is_rep = np.ones(P * J, bool)
    is_rep[oth] = False
    is_rep[oth[swap]] = True
    m0 = is_rep & ~badp[pair_of]
    plane = np.full((NCORES, NT, 128, B), -1, np.int16)
    plane[core[m0], t[m0], pp[m0], ec_rot[m0]] = eslot[m0].astype(np.int16)
    # per-level layout [NCORES][128][NT][PW] for single-DMA loads
    plane_lv = [np.ascontiguousarray(
        plane[:, :, :, POFF[pc]:POFF[pc] + PW[pc]].transpose(0, 2, 1, 3))
        for pc in range(NP)]

    # ---- dup passes (source occ0 slot -> dup slot)
    e_rep, e_oth = eslot[rep_p], eslot[oth_p]

    def mk_pass(mask, W):
        if not mask.any():
            return None
        pl = np.full((NCORES, 128, NT, W), -1, np.int16)
        pl[core[rep_p[mask]], pp[rep_p[mask]], t[rep_p[mask]],
           e_rep[mask]] = e_oth[mask].astype(np.int16)
        return pl

    main_g = grp_piece <= 2
    if narrow_g.any():
        assert (e_rep[narrow_g] < WN).all() and (e_oth[narrow_g] < WN).all()
    pass_main = mk_pass(main_g, NE)
    pass_p3 = mk_pass(grp_piece == 3, WN) if WN else None
    pass_p4 = mk_pass(grp_piece == 4, WN) if WN else None

    nmat = n_per_row.reshape(NCORES, NT, 128)
    pairmask = (np.arange(SP)[None, None, None, :] < nmat[..., None]).astype(F16)
    bp = bad_pairs
    pairmask[r[bp] // RPC, (r[bp] % RPC) // 128, r[bp] % 128, srank[bp]] = 0
    pairmask = np.ascontiguousarray(pairmask.transpose(0, 2, 1, 3))  # [C,128,NT,SP]

    return dict(plane_lv=plane_lv, pass_main=pass_main, pass_p3=pass_p3,
                pass_p4=pass_p4, pairmask=pairmask, SP=SP, NE=NE, WN=WN,
                bad_pairs=bad_pairs)


# ------------------------------------------------------------- device kernel
@with_exitstack
def _build(ctx: ExitStack, tc: "tile.TileContext", io: dict, SP: int, NE: int,
           WN: int, have_main: bool, have_p3: bool, have_p4: bool):
    nc = tc.nc
    y_d, out_d = io["y8"], io["out"]

    consts = ctx.enter_context(tc.tile_pool(name="consts", bufs=1))
    ones_f32c = consts.tile([128, 1], F32, tag="ones_f32c")
    nc.vector.memset(ones_f32c[:], 1.0)
    wz = consts.tile([128, 2, 128], F8, tag="wz")
    nc.vector.memset(wz[:], 0.0)

    ypool = ctx.enter_context(tc.tile_pool(name="y", bufs=1))
    y = ypool.tile([128, NCH, 2 * NKP, CW], F8, tag="y", name="y")

    gpool = ctx.enter_context(tc.tile_pool(name="gbf", bufs=1))
    gbf = {tt: gpool.tile([128, B], DF16, tag=f"gbf{tt}", name=f"gbf{tt}")
           for tt in range(NT)}
    plpool = ctx.enter_context(tc.tile_pool(name="plane", bufs=1))
    pl = {pc: plpool.tile([128, NT, PW[pc]], I16, tag=f"plv{pc}",
                          name=f"plv{pc}") for pc in range(NP)}
    papool = ctx.enter_context(tc.tile_pool(name="passes", bufs=1))
    pam = papool.tile([128, NT, NE], I16, tag="pam", name="pam") if have_main else None
    pa3 = papool.tile([128, NT, WN], I16, tag="pa3", name="pa3") if have_p3 else None
    pa4 = papool.tile([128, NT, WN], I16, tag="pa4", name="pa4") if have_p4 else None

    lpool = ctx.enter_context(tc.tile_pool(name="loss", bufs=1))
    pmall = lpool.tile([128, NT, SP], DF16, tag="pmall")

    # ---- DMA: chunk 7 dead last, everything else just-in-time
    nc.sync.dma_start(y[:, 0], y_d[0])
    nc.sync.dma_start(y[:, 1], y_d[1])
    nc.sync.dma_start(pl[0][:], io["plane0"][:])
    nc.sync.dma_start(y[:, 2], y_d[2])
    nc.sync.dma_start(y[:, 3], y_d[3])
    nc.sync.dma_start(y[:, 4], y_d[4])
    nc.sync.dma_start(pl[1][:], io["plane1"][:])
    nc.sync.dma_start(y[:, 5], y_d[5])
    nc.sync.dma_start(pl[2][:], io["plane2"][:])
    nc.sync.dma_start(y[:, 6], y_d[6])
    nc.sync.dma_start(pl[3][:], io["plane3"][:])
    nc.sync.dma_start(y[:, 7], y_d[7])
    nc.sync.dma_start(pl[4][:], io["plane4"][:])
    if have_main:
        nc.sync.dma_start(pam[:], io["passm"][:])
    if have_p3:
        nc.sync.dma_start(pa3[:], io["pass3"][:])
    nc.sync.dma_start(pmall[:].rearrange("p t s -> p (t s)"),
                      io["pm"][:].rearrange("p t s -> p (t s)"))
    if have_p4:
        nc.sync.dma_start(pa4[:], io["pass4"][:])

    dpool = ctx.enter_context(tc.tile_pool(name="dq", bufs=2))
    hpool = ctx.enter_context(tc.tile_pool(name="hacc", bufs=1))
    expool = ctx.enter_context(tc.tile_pool(name="extra", bufs=2))
    elpool = ctx.enter_context(tc.tile_pool(name="elb", bufs=2))
    hacc = {tt: hpool.tile([128, NE], DF16, tag=f"hacc{tt}", name=f"hacc{tt}")
            for tt in range(NT)}
    denall = lpool.tile([128, NT, SP], F32, tag="denall")
    l0all = lpool.tile([128, NT, SP], DF16, tag="l0all")

    # preload the Exp activation table during the initial DMA idle
    pre = elpool.tile([128, 1], F32, tag="pre")
    nc.scalar.activation(pre[:], ones_f32c[:], AF.Exp)

    dq = {}

    def scatter_piece(pc, tiles=range(NT)):
        for tt in tiles:
            d = dpool.tile([128, NE], DF16, tag=f"d{tt}", name=f"d{tt}_{pc}")
            dq[(tt, pc)] = d
            nc.gpsimd.local_scatter(
                d[:], gbf[tt][:, POFF[pc]:POFF[pc] + PW[pc]],
                pl[pc][:, tt, :], 128, NE, PW[pc])

    with tc.tile_pool(name="gpsum", bufs=1, space="PSUM") as gpsum:
        wps0 = gpsum.tile([128, CW], F32, tag="ps0_1", name="warm_init")
        for i in range(WARM0):
            nc.tensor.matmul(wps0[:, 0:64], lhsT=wz[:], rhs=wz[:, :, 0:64],
                             start=True, stop=True, perf_mode=DR)
        for c in range(NCH):
            for tt in range(NT):
                ps = gpsum.tile([128, CW], F32, tag=f"ps{tt}_{c % 2}",
                                name=f"ps{tt}_{c}")
                for kp in range(NKP):
                    nc.tensor.matmul(
                        ps[:],
                        lhsT=y[:, 0, 2 * kp:2 * kp + 2, tt * 128:(tt + 1) * 128],
                        rhs=y[:, c, 2 * kp:2 * kp + 2, :],
                        start=(kp == 0), stop=(kp == NKP - 1),
                        perf_mode=DR,
                    )
                dst = gbf[tt][:, c * CW:(c + 1) * CW]
                if (c * NT + tt) % 2 == 0:
                    nc.vector.tensor_copy(dst, ps[:])
                else:
                    nc.scalar.copy(dst, ps[:])
            # p-state bridge across the DMA-paced idle before the next chunk
            if WARMS[c]:
                wps = gpsum.tile([128, CW], F32, tag=f"ps0_{(c + 1) % 2}",
                                 name=f"warm{c}")
                for i in range(WARMS[c]):
                    nc.tensor.matmul(wps[:, 0:64], lhsT=wz[:],
                                     rhs=wz[:, :, 0:64],
                                     start=True, stop=True, perf_mode=DR)
            if c == 1:
                scatter_piece(0)
            elif c == 3:
                scatter_piece(1)
                for tt in range(NT):
                    nc.vector.tensor_tensor(hacc[tt][:], dq[(tt, 0)][:],
                                            dq[(tt, 1)][:], ALU.add)
            elif c == 5:
                scatter_piece(2)
                for tt in range(NT):
                    nc.vector.tensor_tensor(hacc[tt][:], hacc[tt][:],
                                            dq[(tt, 2)][:], ALU.add)
            elif c == 6:
                scatter_piece(3)
                for tt in range(NT):
                    nc.vector.tensor_tensor(hacc[tt][:], hacc[tt][:],
                                            dq[(tt, 3)][:], ALU.add)
                if have_main:
                    for tt in range(NT):
                        e = expool.tile([128, NE], DF16, tag=f"eAm{tt % 2}",
                                        name=f"eAm{tt}")
                        nc.gpsimd.local_scatter(e[:], hacc[tt][:],
                                                pam[:, tt, :], 128, NE, NE)
                        nc.vector.tensor_tensor(hacc[tt][:], hacc[tt][:],
                                                e[:], ALU.add)
                if have_p3:
                    for tt in range(NT):
                        e = expool.tile([128, WN], DF16, tag=f"eA3{tt % 2}",
                                        name=f"eA3{tt}")
                        nc.gpsimd.local_scatter(e[:], hacc[tt][:, 0:WN],
                                                pa3[:, tt, :], 128, WN, WN)
                        nc.vector.tensor_tensor(hacc[tt][:, 0:WN],
                                                hacc[tt][:, 0:WN], e[:],
                                                ALU.add)

        # ---- tail: piece-4 scatters, narrow late pass, exp/den per tile
        for tt in range(NT):
            scatter_piece(4, [tt])
            nc.vector.tensor_tensor(hacc[tt][:], hacc[tt][:], dq[(tt, 4)][:],
                                    ALU.add)
            if have_p4:
                e = expool.tile([128, WN], DF16, tag=f"eA4{tt % 2}",
                                name=f"eA4{tt}")
                nc.gpsimd.local_scatter(e[:], hacc[tt][:, 0:WN],
                                        pa4[:, tt, :], 128, WN, WN)
                nc.vector.tensor_tensor(hacc[tt][:, 0:WN], hacc[tt][:, 0:WN],
                                        e[:], ALU.add)
            ebuf = elpool.tile([128, NE], F32, tag="ebuf")
            nc.scalar.activation(ebuf[:], hacc[tt][:], AF.Exp, scale=KSC)
            e3 = ebuf[:, 0:SP * J].rearrange("p (s j) -> p s j", j=J)
            nc.vector.tensor_reduce(denall[:, tt, :], e3, AX.X, ALU.add)
            l0 = hacc[tt][:, 0:SP * J].rearrange("p (s j) -> p s j", j=J)[:, :, 0]
            nc.vector.tensor_copy(l0all[:, tt, :], l0)

    # ---- batched tail: Ln (table load hidden behind last den), diff, mask
    with tc.tile_pool(name="p5psum", bufs=1, space="PSUM") as p5psum:
        pre2 = elpool.tile([128, 1], F32, tag="pre2")
        nc.scalar.activation(pre2[:], ones_f32c[:], AF.Ln)
        lnd = lpool.tile([128, NT * SP], F32, tag="lnd")
        nc.scalar.activation(lnd[:], denall[:].rearrange("p t s -> p (t s)"),
                             AF.Ln)
        diff = lpool.tile([128, NT * SP], F32, tag="diff")
        nc.vector.scalar_tensor_tensor(
            diff[:], l0all[:].rearrange("p t s -> p (t s)"), -KSC, lnd[:],
            ALU.mult, ALU.add)
        scrap = lpool.tile([128, NT * SP], F32, tag="scrap")
        acc1 = lpool.tile([128, 1], F32, tag="acc1")
        nc.vector.scalar_tensor_tensor(
            scrap[:], diff[:], 1.0,
            pmall[:].rearrange("p t s -> p (t s)"), ALU.mult, ALU.mult,
            accum_out=acc1[:])
        ps = p5psum.tile([1, 1], F32, tag="ps_out")
        nc.tensor.matmul(ps[:], lhsT=acc1[:], rhs=ones_f32c[:, 0:1],
                         start=True, stop=True)
        res = lpool.tile([1, 1], F32, tag="res")
        nc.scalar.copy(res[:], ps[:])
        nc.sync.dma_start(out_d[:], res[:])


def build_nc(SP, NE, WN, have_main, have_p3, have_p4, enable_asserts=False):
    nc = bacc.Bacc("TRN2", target_bir_lowering=False, debug=False,
                   enable_asserts=enable_asserts, num_devices=NCORES)
    io = {
        "y8": nc.dram_tensor("y8", [NCH, 128, 2 * NKP, CW], F8,
                             kind="ExternalInput").ap(),
        "pm": nc.dram_tensor("pm", [128, NT, SP], DF16,
                             kind="ExternalInput").ap(),
        "out": nc.dram_tensor("out", [1, 1], F32, kind="ExternalOutput").ap(),
    }
    for pc in range(NP):
        io[f"plane{pc}"] = nc.dram_tensor(
            f"plane{pc}", [128, NT, PW[pc]], I16, kind="ExternalInput").ap()
    if have_main:
        io["passm"] = nc.dram_tensor("passm", [128, NT, NE], I16,
                                     kind="ExternalInput").ap()
    if have_p3:
        io["pass3"] = nc.dram_tensor("pass3", [128, NT, WN], I16,
                                     kind="ExternalInput").ap()
    if have_p4:
        io["pass4"] = nc.dram_tensor("pass4", [128, NT, WN], I16,
                                     kind="ExternalInput").ap()
    with tile.TileContext(nc) as tc:
        _build(tc, io, SP, NE, WN, have_main, have_p3, have_p4)
    nc.compile()
    return nc


def _normalize(x):
    x = np.asarray(x, np.float32)
    w = np.sqrt((x.astype(np.float64) ** 2).sum(axis=1, keepdims=True))
    w = np.maximum(w, 1e-8)
    return (x / w).astype(np.float32)


def make_in_maps(x, plan):
    xn = _normalize(x)
    x8 = np.clip(xn * S, -240.0, 240.0).astype(FP8)
    in_maps = []
    for k in range(NCORES):
        xr = np.roll(x8, -RPC * k, axis=0)                     # [B, D]
        y8 = xr.T.reshape(2 * NKP, 128, B).transpose(1, 0, 2)  # [128, 16, B]
        y8c = np.ascontiguousarray(
            y8.reshape(128, 2 * NKP, NCH, CW).transpose(2, 0, 1, 3))
        m = {"y8": y8c, "pm": plan["pairmask"][k]}
        for pc in range(NP):
            m[f"plane{pc}"] = plan["plane_lv"][pc][k]
        if plan["pass_main"] is not None:
            m["passm"] = plan["pass_main"][k]
        if plan["pass_p3"] is not None:
            m["pass3"] = plan["pass_p3"][k]
        if plan["pass_p4"] is not None:
            m["pass4"] = plan["pass_p4"][k]
        in_maps.append(m)
    return in_maps


def host_fixup(x, anchor_idx, pos_idx, neg_idx, bad_pairs):
    """Exact loss terms for pairs masked out on the device."""
    if len(bad_pairs) == 0:
        return 0.0
    xn = _normalize(x).astype(np.float64)
    a = anchor_idx[bad_pairs]
    cols = np.concatenate([pos_idx[bad_pairs][:, None], neg_idx[bad_pairs]],
                          axis=1)
    logits = np.einsum("pd,pjd->pj", xn[a], xn[cols]) / TEMP
    mx = logits.max(axis=1, keepdims=True)
    lse = np.log(np.exp(logits - mx).sum(axis=1)) + mx[:, 0]
    return float((lse - logits[:, 0]).sum())


def kernel(**inputs):
    x = np.asarray(inputs["x"], np.float32)
    anchor_idx = np.asarray(inputs["anchor_idx"])
    pos_idx = np.asarray(inputs["pos_idx"])
    neg_idx = np.asarray(inputs["neg_idx"])
    P = anchor_idx.shape[0]

    plan = build_plan(anchor_idx, pos_idx, neg_idx)
    nc = build_nc(plan["SP"], plan["NE"], plan["WN"],
                  plan["pass_main"] is not None, plan["pass_p3"] is not None,
                  plan["pass_p4"] is not None)
    in_maps = make_in_maps(x, plan)
    res = run_bass_kernel_spmd(nc, in_maps, list(range(NCORES)))
    total = sum(float(res.results[k]["out"][0, 0]) for k in range(NCORES))
    total += host_fixup(x, anchor_idx, pos_idx, neg_idx, plan["bad_pairs"])
    return np.float32(total / P)


# revision 21
# speedup vs baseline: 1.0503x; 1.0089x over previous
"""Trainium2 Bass kernel for ContrastiveNet loss (v6).

Per core k of 8 (SPMD):
  - host: xn = x/||x||*S (S=32) in fp8e4 -> sim = G/(S^2*TEMP), no on-device
    normalization. Rows rolled so core k's 512 anchor rows are rotated cols
    0..511. y is COLUMN-CHUNK-major ([8][128][16][512]) so each chunk's gram
    (4 row-tiles x 8 kp fp8 DoubleRow matmuls into [128,512] PSUM) starts as
    the chunk lands; chunk 7 is dead last in the DMA stream.
  - DMA is the wall (~37us saturated): all scatter planes / masks are batched
    single transfers interleaved so nothing but chunk 7 is late.
  - gather: per (tile, piece 1024/1024/1024/512/512) gpsimd local_scatter
    with HBM col->slot planes; values accumulate in-place (fp16).
    2-member duplicate (row,col) groups are fixed by scatter passes:
    main pass (rep col in pieces 0-2; full NE, runs mid-stream), piece-3 and
    piece-4 passes (narrow: their pairs are ranked into the lowest slots).
    Pairs touching >=3-member groups (~2%) are masked and done on host.
  - loss: exp(scale*x) per tile on ACT (table preloaded; single Ln at the
    end, its table load hidden behind the last den reduce), per-pair den
    reduce, masked (ln den - scale*l0) accumulation -> [1,1] per core.
  - PE p-state: tiny warm matmuls bridge DMA-paced idle gaps so real grams
    stay at full clock.
"""
import os
import sys
import numpy as np
import ml_dtypes

try:
    import concourse  # noqa: F401
except ImportError:
    sys.path.insert(0, "/opt/trn_rl_repo")

from contextlib import ExitStack

import concourse.bass as bass
import concourse.tile as tile
from concourse import bacc, mybir
from concourse._compat import with_exitstack
from concourse.bass_utils import run_bass_kernel_spmd

F16 = np.float16
FP8 = ml_dtypes.float8_e4m3
F32 = mybir.dt.float32
DF16 = mybir.dt.float16
F8 = mybir.dt.float8e4
I16 = mybir.dt.int16

B, D, J = 4096, 2048, 11
NCORES, RPC, NT, NKP = 8, 512, 4, 8
NCH, CW = 8, 512                    # gram column chunks (per core)
POFF = [0, 1024, 2048, 3072, 3584]  # scatter piece offsets
PW = [1024, 1024, 1024, 512, 512]   # scatter piece widths
NP = 5
TEMP = 0.1
S = 32.0
KSC = 1.0 / (S * S * TEMP)
# warm matmuls ([128,64] each, ~75ns) issued after each chunk's grams to keep
# the PE p-state ramped across DMA-paced idle gaps
WARMS = [0, 55, 0, 0, 35, 50, 45, 0]
WARM0 = 130
AF = mybir.ActivationFunctionType
ALU = mybir.AluOpType
AX = mybir.AxisListType
DR = mybir.MatmulPerfMode.DoubleRow


def _even(n):
    return n + (n % 2)


# ---------------------------------------------------------------- host prep
def build_plan(anchor_idx, pos_idx, neg_idx):
    r = anchor_idx.astype(np.int64)
    cols = np.concatenate([pos_idx[:, None], neg_idx], axis=1).astype(np.int64)
    P = r.shape[0]

    er = np.repeat(r, J)
    ec = cols.ravel()
    pair_of = np.repeat(np.arange(P), J)
    core = er // RPC
    t = (er % RPC) // 128
    pp = er % 128
    ec_rot = (ec - core * RPC) % B
    piece = np.searchsorted(POFF, ec_rot, side="right") - 1

    key = er * B + ec
    o2 = np.argsort(key, kind="stable")
    k_sorted = key[o2]
    first2 = np.r_[True, k_sorted[1:] != k_sorted[:-1]]
    gid_sorted = np.cumsum(first2) - 1
    NG = int(gid_sorted[-1]) + 1
    gid = np.empty(P * J, np.int64)
    gid[o2] = gid_sorted
    gsz_g = np.bincount(gid_sorted, minlength=NG)
    gsz = gsz_g[gid]

    # pairs containing any entry of a >=3-member group -> host, masked out
    bad_pairs = np.unique(pair_of[gsz >= 3])
    badp = np.zeros(P, bool)
    badp[bad_pairs] = True

    # 2-member groups: rep = member in the earliest piece
    two = gsz == 2
    order = np.lexsort((np.arange(P * J), piece, gid))
    go = order[two[order]]
    g_of_go = gid[go]
    firstg = np.r_[True, g_of_go[1:] != g_of_go[:-1]]
    rep = go[firstg]
    oth = go[~firstg]
    rep_bad = badp[pair_of[rep]]
    oth_bad = badp[pair_of[oth]]
    swap = rep_bad & ~oth_bad       # partner becomes the plane entry
    pk = ~rep_bad & ~oth_bad        # device dup pass only if both pairs live
    rep_p, oth_p = rep[pk], oth[pk]
    grp_piece = piece[rep_p]        # min piece of the group
    narrow_g = grp_piece >= 3       # piece-3 / piece-4 passes

    # ---- pair ranking: pairs touching narrow-pass groups come first
    narrow_pairs = np.unique(np.r_[pair_of[rep_p[narrow_g]],
                                   pair_of[oth_p[narrow_g]]])
    sev = np.zeros(P, np.int64)
    sev[narrow_pairs] = 1
    order_p = np.lexsort((np.arange(P), -sev, r))
    r_sp = r[order_p]
    firstp = np.r_[True, r_sp[1:] != r_sp[:-1]]
    gidp = np.cumsum(firstp) - 1
    rank_sorted = np.arange(P) - np.flatnonzero(firstp)[gidp]
    srank = np.empty(P, np.int64)
    srank[order_p] = rank_sorted

    n_per_row = np.bincount(r, minlength=B)
    SP = int(max(n_per_row.max(), 1))
    NE = _even(SP * J)
    assert NE * 32 < 2**16
    eslot = srank[pair_of] * J + np.tile(np.arange(J), P)

    cnt_n = np.bincount(r[narrow_pairs], minlength=B) if len(narrow_pairs) \
        else np.zeros(B, np.int64)
    WN = _even(min(int(cnt_n.max()) * J + 2, NE)) if cnt_n.max() > 0 else 0

    # ---- main scatter plane: col -> slot of occ0 entries (incl. singletons)
    is_rep = np.ones(P * J, bool)
    is_rep[oth] = False
    is_rep[oth[swap]] = True
    m0 = is_rep & ~badp[pair_of]
    plane = np.full((NCORES, NT, 128, B), -1, np.int16)
    plane[core[m0], t[m0], pp[m0], ec_rot[m0]] = eslot[m0].astype(np.int16)
    # per-level layout [NCORES][128][NT][PW] for single-DMA loads
    plane_lv = [np.ascontiguousarray(
        plane[:, :, :, POFF[pc]:POFF[pc] + PW[pc]].transpose(0, 2, 1, 3))
        for pc in range(NP)]

    # ---- dup passes (source occ0 slot -> dup slot)
    e_rep, e_oth = eslot[rep_p], eslot[oth_p]

    def mk_pass(mask, W):
        if not mask.any():
            return None
        pl = np.full((NCORES, 128, NT, W), -1, np.int16)
        pl[core[rep_p[mask]], pp[rep_p[mask]], t[rep_p[mask]],
           e_rep[mask]] = e_oth[mask].astype(np.int16)
        return pl

    main_g = grp_piece <= 2
    if narrow_g.any():
        assert (e_rep[narrow_g] < WN).all() and (e_oth[narrow_g] < WN).all()
    pass_main = mk_pass(main_g, NE)
    pass_p3 = mk_pass(grp_piece == 3, WN) if WN else None
    pass_p4 = mk_pass(grp_piece == 4, WN) if WN else None

    nmat = n_per_row.reshape(NCORES, NT, 128)
    pairmask = (np.arange(SP)[None, None, None, :] < nmat[..., None]).astype(F16)
    bp = bad_pairs
    pairmask[r[bp] // RPC, (r[bp] % RPC) // 128, r[bp] % 128, srank[bp]] = 0
    pairmask = np.ascontiguousarray(pairmask.transpose(0, 2, 1, 3))  # [C,128,NT,SP]

    return dict(plane_lv=plane_lv, pass_main=pass_main, pass_p3=pass_p3,
                pass_p4=pass_p4, pairmask=pairmask, SP=SP, NE=NE, WN=WN,
                bad_pairs=bad_pairs)


# ------------------------------------------------------------- device kernel
@with_exitstack
def _build(ctx: ExitStack, tc: "tile.TileContext", io: dict, SP: int, NE: int,
           WN: int, have_main: bool, have_p3: bool, have_p4: bool):
    nc = tc.nc
    y_d, out_d = io["y8"], io["out"]

    consts = ctx.enter_context(tc.tile_pool(name="consts", bufs=1))
    ones_f32c = consts.tile([128, 1], F32, tag="ones_f32c")
    nc.vector.memset(ones_f32c[:], 1.0)
    wz = consts.tile([128, 2, 128], F8, tag="wz")
    nc.vector.memset(wz[:], 0.0)

    ypool = ctx.enter_context(tc.tile_pool(name="y", bufs=1))
    y = ypool.tile([128, NCH, 2 * NKP, CW], F8, tag="y", name="y")

    gpool = ctx.enter_context(tc.tile_pool(name="gbf", bufs=1))
    gbf = {tt: gpool.tile([128, B], DF16, tag=f"gbf{tt}", name=f"gbf{tt}")
           for tt in range(NT)}
    plpool = ctx.enter_context(tc.tile_pool(name="plane", bufs=1))
    pl = {pc: plpool.tile([128, NT, PW[pc]], I16, tag=f"plv{pc}",
                          name=f"plv{pc}") for pc in range(NP)}
    papool = ctx.enter_context(tc.tile_pool(name="passes", bufs=1))
    pam = papool.tile([128, NT, NE], I16, tag="pam", name="pam") if have_main else None
    pa3 = papool.tile([128, NT, WN], I16, tag="pa3", name="pa3") if have_p3 else None
    pa4 = papool.tile([128, NT, WN], I16, tag="pa4", name="pa4") if have_p4 else None

    lpool = ctx.enter_context(tc.tile_pool(name="loss", bufs=1))
    pmall = lpool.tile([128, NT, SP], DF16, tag="pmall")

    # ---- DMA: chunk 7 dead last, everything else just-in-time
    nc.sync.dma_start(y[:, 0], y_d[0])
    nc.sync.dma_start(y[:, 1], y_d[1])
    nc.sync.dma_start(pl[0][:], io["plane0"][:])
    nc.sync.dma_start(y[:, 2], y_d[2])
    nc.sync.dma_start(y[:, 3], y_d[3])
    nc.sync.dma_start(y[:, 4], y_d[4])
    nc.sync.dma_start(pl[1][:], io["plane1"][:])
    nc.sync.dma_start(y[:, 5], y_d[5])
    nc.sync.dma_start(pl[2][:], io["plane2"][:])
    nc.sync.dma_start(y[:, 6], y_d[6])
    nc.sync.dma_start(pl[3][:], io["plane3"][:])
    nc.sync.dma_start(y[:, 7], y_d[7])
    nc.sync.dma_start(pl[4][:], io["plane4"][:])
    if have_main:
        nc.sync.dma_start(pam[:], io["passm"][:])
    if have_p3:
        nc.sync.dma_start(pa3[:], io["pass3"][:])
    nc.sync.dma_start(pmall[:].rearrange("p t s -> p (t s)"),
                      io["pm"][:].rearrange("p t s -> p (t s)"))
    if have_p4:
        nc.sync.dma_start(pa4[:], io["pass4"][:])

    dpool = ctx.enter_context(tc.tile_pool(name="dq", bufs=2))
    hpool = ctx.enter_context(tc.tile_pool(name="hacc", bufs=1))
    expool = ctx.enter_context(tc.tile_pool(name="extra", bufs=2))
    elpool = ctx.enter_context(tc.tile_pool(name="elb", bufs=2))
    hacc = {tt: hpool.tile([128, NE], DF16, tag=f"hacc{tt}", name=f"hacc{tt}")
            for tt in range(NT)}
    denall = lpool.tile([128, NT, SP], F32, tag="denall")
    l0all = lpool.tile([128, NT, SP], DF16, tag="l0all")

    # preload the Exp activation table during the initial DMA idle
    pre = elpool.tile([128, 1], F32, tag="pre")
    nc.scalar.activation(pre[:], ones_f32c[:], AF.Exp)

    dq = {}

    def scatter_piece(pc, tiles=range(NT)):
        for tt in tiles:
            d = dpool.tile([128, NE], DF16, tag=f"d{tt}", name=f"d{tt}_{pc}")
            dq[(tt, pc)] = d
            nc.gpsimd.local_scatter(
                d[:], gbf[tt][:, POFF[pc]:POFF[pc] + PW[pc]],
                pl[pc][:, tt, :], 128, NE, PW[pc])

    with tc.tile_pool(name="gpsum", bufs=1, space="PSUM") as gpsum:
        wps0 = gpsum.tile([128, CW], F32, tag="ps0_1", name="warm_init")
        for i in range(WARM0):
            nc.tensor.matmul(wps0[:, 0:64], lhsT=wz[:], rhs=wz[:, :, 0:64],
                             start=True, stop=True, perf_mode=DR)
        for c in range(NCH):
            for tt in range(NT):
                ps = gpsum.tile([128, CW], F32, tag=f"ps{tt}_{c % 2}",
                                name=f"ps{tt}_{c}")
                for kp in range(NKP):
                    nc.tensor.matmul(
                        ps[:],
                        lhsT=y[:, 0, 2 * kp:2 * kp + 2, tt * 128:(tt + 1) * 128],
                        rhs=y[:, c, 2 * kp:2 * kp + 2, :],
                        start=(kp == 0), stop=(kp == NKP - 1),
                        perf_mode=DR,
                    )
                dst = gbf[tt][:, c * CW:(c + 1) * CW]
                if (c * NT + tt) % 2 == 0:
                    nc.vector.tensor_copy(dst, ps[:])
                else:
                    nc.scalar.copy(dst, ps[:])
            # p-state bridge across the DMA-paced idle before the next chunk
            if WARMS[c]:
                wps = gpsum.tile([128, CW], F32, tag=f"ps0_{(c + 1) % 2}",
                                 name=f"warm{c}")
                for i in range(WARMS[c]):
                    nc.tensor.matmul(wps[:, 0:64], lhsT=wz[:],
                                     rhs=wz[:, :, 0:64],
                                     start=True, stop=True, perf_mode=DR)
            if c == 1:
                scatter_piece(0)
            elif c == 3:
                scatter_piece(1)
            elif c == 5:
                scatter_piece(2)
                # piece 0+1 sums issue after c5 drains: ready (sc1 done)
                # without blocking any drain behind them
                for tt in range(NT):
                    nc.vector.tensor_tensor(hacc[tt][:], dq[(tt, 0)][:],
                                            dq[(tt, 1)][:], ALU.add)
            elif c == 6:
                scatter_piece(3)
                for tt in range(NT):
                    nc.vector.tensor_tensor(hacc[tt][:], hacc[tt][:],
                                            dq[(tt, 2)][:], ALU.add)
                if have_main:
                    # sources in pieces 0-2: reads hacc(d0+d1+d2)
                    for tt in range(NT):
                        e = expool.tile([128, NE], DF16, tag=f"eAm{tt % 2}",
                                        name=f"eAm{tt}")
                        nc.gpsimd.local_scatter(e[:], hacc[tt][:],
                                                pam[:, tt, :], 128, NE, NE)
                        nc.vector.tensor_tensor(hacc[tt][:], hacc[tt][:],
                                                e[:], ALU.add)
                if have_p3:
                    # sources in piece 3: reads d3 directly
                    for tt in range(NT):
                        e = expool.tile([128, WN], DF16, tag=f"eA3{tt % 2}",
                                        name=f"eA3{tt}")
                        nc.gpsimd.local_scatter(e[:], dq[(tt, 3)][:, 0:WN],
                                                pa3[:, tt, :], 128, WN, WN)
                        nc.vector.tensor_tensor(hacc[tt][:, 0:WN],
                                                hacc[tt][:, 0:WN], e[:],
                                                ALU.add)

        # ---- tail: piece-3/4 sums, piece-4 scatters, narrow late pass
        for tt in range(NT):
            scatter_piece(4, [tt])
            nc.vector.tensor_tensor(hacc[tt][:], hacc[tt][:], dq[(tt, 3)][:],
                                    ALU.add)
            if have_p4:
                e = expool.tile([128, WN], DF16, tag=f"eA4{tt % 2}",
                                name=f"eA4{tt}")
                nc.gpsimd.local_scatter(e[:], dq[(tt, 4)][:, 0:WN],
                                        pa4[:, tt, :], 128, WN, WN)
                nc.vector.tensor_tensor(hacc[tt][:, 0:WN], hacc[tt][:, 0:WN],
                                        e[:], ALU.add)
            nc.vector.tensor_tensor(hacc[tt][:], hacc[tt][:], dq[(tt, 4)][:],
                                    ALU.add)
            ebuf = elpool.tile([128, NE], F32, tag="ebuf")
            nc.scalar.activation(ebuf[:], hacc[tt][:], AF.Exp, scale=KSC)
            e3 = ebuf[:, 0:SP * J].rearrange("p (s j) -> p s j", j=J)
            nc.vector.tensor_reduce(denall[:, tt, :], e3, AX.X, ALU.add)
            l0 = hacc[tt][:, 0:SP * J].rearrange("p (s j) -> p s j", j=J)[:, :, 0]
            nc.vector.tensor_copy(l0all[:, tt, :], l0)

    # ---- batched tail: Ln (table load hidden behind last den), diff, mask
    with tc.tile_pool(name="p5psum", bufs=1, space="PSUM") as p5psum:
        pre2 = elpool.tile([128, 1], F32, tag="pre2")
        nc.scalar.activation(pre2[:], denall[:, NT - 1, 0:1], AF.Ln)
        lnd = lpool.tile([128, NT * SP], F32, tag="lnd")
        nc.scalar.activation(lnd[:], denall[:].rearrange("p t s -> p (t s)"),
                             AF.Ln)
        diff = lpool.tile([128, NT * SP], F32, tag="diff")
        nc.vector.scalar_tensor_tensor(
            diff[:], l0all[:].rearrange("p t s -> p (t s)"), -KSC, lnd[:],
            ALU.mult, ALU.add)
        scrap = lpool.tile([128, NT * SP], F32, tag="scrap")
        acc1 = lpool.tile([128, 1], F32, tag="acc1")
        nc.vector.scalar_tensor_tensor(
            scrap[:], diff[:], 1.0,
            pmall[:].rearrange("p t s -> p (t s)"), ALU.mult, ALU.mult,
            accum_out=acc1[:])
        ps = p5psum.tile([1, 1], F32, tag="ps_out")
        nc.tensor.matmul(ps[:], lhsT=acc1[:], rhs=ones_f32c[:, 0:1],
                         start=True, stop=True)
        res = lpool.tile([1, 1], F32, tag="res")
        nc.scalar.copy(res[:], ps[:])
        nc.sync.dma_start(out_d[:], res[:])


def build_nc(SP, NE, WN, have_main, have_p3, have_p4, enable_asserts=False):
    nc = bacc.Bacc("TRN2", target_bir_lowering=False, debug=False,
                   enable_asserts=enable_asserts, num_devices=NCORES)
    io = {
        "y8": nc.dram_tensor("y8", [NCH, 128, 2 * NKP, CW], F8,
                             kind="ExternalInput").ap(),
        "pm": nc.dram_tensor("pm", [128, NT, SP], DF16,
                             kind="ExternalInput").ap(),
        "out": nc.dram_tensor("out", [1, 1], F32, kind="ExternalOutput").ap(),
    }
    for pc in range(NP):
        io[f"plane{pc}"] = nc.dram_tensor(
            f"plane{pc}", [128, NT, PW[pc]], I16, kind="ExternalInput").ap()
    if have_main:
        io["passm"] = nc.dram_tensor("passm", [128, NT, NE], I16,
                                     kind="ExternalInput").ap()
    if have_p3:
        io["pass3"] = nc.dram_tensor("pass3", [128, NT, WN], I16,
                                     kind="ExternalInput").ap()
    if have_p4:
        io["pass4"] = nc.dram_tensor("pass4", [128, NT, WN], I16,
                                     kind="ExternalInput").ap()
    with tile.TileContext(nc) as tc:
        _build(tc, io, SP, NE, WN, have_main, have_p3, have_p4)
    nc.compile()
    return nc


def _normalize(x):
    x = np.asarray(x, np.float32)
    w = np.sqrt((x.astype(np.float64) ** 2).sum(axis=1, keepdims=True))
    w = np.maximum(w, 1e-8)
    return (x / w).astype(np.float32)


def make_in_maps(x, plan):
    xn = _normalize(x)
    x8 = np.clip(xn * S, -240.0, 240.0).astype(FP8)
    in_maps = []
    for k in range(NCORES):
        xr = np.roll(x8, -RPC * k, axis=0)                     # [B, D]
        y8 = xr.T.reshape(2 * NKP, 128, B).transpose(1, 0, 2)  # [128, 16, B]
        y8c = np.ascontiguousarray(
            y8.reshape(128, 2 * NKP, NCH, CW).transpose(2, 0, 1, 3))
        m = {"y8": y8c, "pm": plan["pairmask"][k]}
        for pc in range(NP):
            m[f"plane{pc}"] = plan["plane_lv"][pc][k]
        if plan["pass_main"] is not None:
            m["passm"] = plan["pass_main"][k]
        if plan["pass_p3"] is not None:
            m["pass3"] = plan["pass_p3"][k]
        if plan["pass_p4"] is not None:
            m["pass4"] = plan["pass_p4"][k]
        in_maps.append(m)
    return in_maps


def host_fixup(x, anchor_idx, pos_idx, neg_idx, bad_pairs):
    """Exact loss terms for pairs masked out on the device."""
    if len(bad_pairs) == 0:
        return 0.0
    xn = _normalize(x).astype(np.float64)
    a = anchor_idx[bad_pairs]
    cols = np.concatenate([pos_idx[bad_pairs][:, None], neg_idx[bad_pairs]],
                          axis=1)
    logits = np.einsum("pd,pjd->pj", xn[a], xn[cols]) / TEMP
    mx = logits.max(axis=1, keepdims=True)
    lse = np.log(np.exp(logits - mx).sum(axis=1)) + mx[:, 0]
    return float((lse - logits[:, 0]).sum())


def kernel(**inputs):
    x = np.asarray(inputs["x"], np.float32)
    anchor_idx = np.asarray(inputs["anchor_idx"])
    pos_idx = np.asarray(inputs["pos_idx"])
    neg_idx = np.asarray(inputs["neg_idx"])
    P = anchor_idx.shape[0]

    plan = build_plan(anchor_idx, pos_idx, neg_idx)
    nc = build_nc(plan["SP"], plan["NE"], plan["WN"],
                  plan["pass_main"] is not None, plan["pass_p3"] is not None,
                  plan["pass_p4"] is not None)
    in_maps = make_in_maps(x, plan)
    res = run_bass_kernel_spmd(nc, in_maps, list(range(NCORES)))
    total = sum(float(res.results[k]["out"][0, 0]) for k in range(NCORES))
    total += host_fixup(x, anchor_idx, pos_idx, neg_idx, plan["bad_pairs"])
    return np.float32(total / P)


# revision 22
# speedup vs baseline: 1.0804x; 1.0287x over previous
"""Trainium2 Bass kernel for ContrastiveNet loss (v6).

Per core k of 8 (SPMD):
  - host: xn = x/||x||*S (S=32) in fp8e4 -> sim = G/(S^2*TEMP), no on-device
    normalization. Rows rolled so core k's 512 anchor rows are rotated cols
    0..511. y is COLUMN-CHUNK-major ([8][128][16][512]) so each chunk's gram
    (4 row-tiles x 8 kp fp8 DoubleRow matmuls into [128,512] PSUM) starts as
    the chunk lands; chunk 7 is dead last in the DMA stream.
  - DMA is the wall (~37us saturated): all scatter planes / masks are batched
    single transfers interleaved so nothing but chunk 7 is late.
  - gather: per (tile, piece 1024/1024/1024/512/512) gpsimd local_scatter
    with HBM col->slot planes; values accumulate in-place (fp16).
    2-member duplicate (row,col) groups are fixed by scatter passes:
    main pass (rep col in pieces 0-2; full NE, runs mid-stream), piece-3 and
    piece-4 passes (narrow: their pairs are ranked into the lowest slots).
    Pairs touching >=3-member groups (~2%) are masked and done on host.
  - loss: exp(scale*x) per tile on ACT (table preloaded; single Ln at the
    end, its table load hidden behind the last den reduce), per-pair den
    reduce, masked (ln den - scale*l0) accumulation -> [1,1] per core.
  - PE p-state: tiny warm matmuls bridge DMA-paced idle gaps so real grams
    stay at full clock.
"""
import os
import sys
import numpy as np
import ml_dtypes

try:
    import concourse  # noqa: F401
except ImportError:
    sys.path.insert(0, "/opt/trn_rl_repo")

from contextlib import ExitStack

import concourse.bass as bass
import concourse.tile as tile
from concourse import bacc, mybir
from concourse._compat import with_exitstack
from concourse.bass_utils import run_bass_kernel_spmd

F16 = np.float16
FP8 = ml_dtypes.float8_e4m3
F32 = mybir.dt.float32
DF16 = mybir.dt.float16
F8 = mybir.dt.float8e4
I16 = mybir.dt.int16

B, D, J = 4096, 2048, 11
NCORES, RPC, NT, NKP = 8, 512, 4, 8
NCH, CW = 8, 512                    # gram column chunks (per core)
POFF = [0, 1024, 2048, 3072, 3584]  # scatter piece offsets
PW = [1024, 1024, 1024, 512, 512]   # scatter piece widths
NP = 5
TEMP = 0.1
S = 32.0
KSC = 1.0 / (S * S * TEMP)
# warm matmuls ([128,64] each, ~75ns) issued after each chunk's grams to keep
# the PE p-state ramped across DMA-paced idle gaps
WARMS = [0, 55, 0, 0, 35, 50, 45, 0]
WARM0 = 130
AF = mybir.ActivationFunctionType
ALU = mybir.AluOpType
AX = mybir.AxisListType
DR = mybir.MatmulPerfMode.DoubleRow


def _even(n):
    return n + (n % 2)


# ---------------------------------------------------------------- host prep
def build_plan(anchor_idx, pos_idx, neg_idx):
    r = anchor_idx.astype(np.int64)
    cols = np.concatenate([pos_idx[:, None], neg_idx], axis=1).astype(np.int64)
    P = r.shape[0]

    er = np.repeat(r, J)
    ec = cols.ravel()
    pair_of = np.repeat(np.arange(P), J)
    core = er // RPC
    t = (er % RPC) // 128
    pp = er % 128
    ec_rot = (ec - core * RPC) % B
    piece = np.searchsorted(POFF, ec_rot, side="right") - 1

    key = er * B + ec
    o2 = np.argsort(key, kind="stable")
    k_sorted = key[o2]
    first2 = np.r_[True, k_sorted[1:] != k_sorted[:-1]]
    gid_sorted = np.cumsum(first2) - 1
    NG = int(gid_sorted[-1]) + 1
    gid = np.empty(P * J, np.int64)
    gid[o2] = gid_sorted
    gsz_g = np.bincount(gid_sorted, minlength=NG)
    gsz = gsz_g[gid]

    # pairs containing any entry of a >=3-member group -> host, masked out
    bad_pairs = np.unique(pair_of[gsz >= 3])
    badp = np.zeros(P, bool)
    badp[bad_pairs] = True

    # 2-member groups: rep = member in the earliest piece
    two = gsz == 2
    order = np.lexsort((np.arange(P * J), piece, gid))
    go = order[two[order]]
    g_of_go = gid[go]
    firstg = np.r_[True, g_of_go[1:] != g_of_go[:-1]]
    rep = go[firstg]
    oth = go[~firstg]
    rep_bad = badp[pair_of[rep]]
    oth_bad = badp[pair_of[oth]]
    swap = rep_bad & ~oth_bad       # partner becomes the plane entry
    pk = ~rep_bad & ~oth_bad        # device dup pass only if both pairs live
    rep_p, oth_p = rep[pk], oth[pk]
    grp_piece = piece[rep_p]        # min piece of the group
    narrow_g = grp_piece >= 3       # piece-3 / piece-4 passes

    # ---- pair ranking: pairs touching narrow-pass groups come first
    narrow_pairs = np.unique(np.r_[pair_of[rep_p[narrow_g]],
                                   pair_of[oth_p[narrow_g]]])
    sev = np.zeros(P, np.int64)
    sev[narrow_pairs] = 1
    order_p = np.lexsort((np.arange(P), -sev, r))
    r_sp = r[order_p]
    firstp = np.r_[True, r_sp[1:] != r_sp[:-1]]
    gidp = np.cumsum(firstp) - 1
    rank_sorted = np.arange(P) - np.flatnonzero(firstp)[gidp]
    srank = np.empty(P, np.int64)
    srank[order_p] = rank_sorted

    n_per_row = np.bincount(r, minlength=B)
    SP = int(max(n_per_row.max(), 1))
    NE = _even(SP * J)
    assert NE * 32 < 2**16
    eslot = srank[pair_of] * J + np.tile(np.arange(J), P)

    cnt_n = np.bincount(r[narrow_pairs], minlength=B) if len(narrow_pairs) \
        else np.zeros(B, np.int64)
    WN = _even(min(int(cnt_n.max()) * J + 2, NE)) if cnt_n.max() > 0 else 0

    # ---- main scatter plane: col -> slot of occ0 entries (incl. singletons)
    is_rep = np.ones(P * J, bool)
    is_rep[oth] = False
    is_rep[oth[swap]] = True
    m0 = is_rep & ~badp[pair_of]
    plane = np.full((NCORES, NT, 128, B), -1, np.int16)
    plane[core[m0], t[m0], pp[m0], ec_rot[m0]] = eslot[m0].astype(np.int16)
    # per-level layout [NCORES][128][NT][PW] for single-DMA loads
    plane_lv = [np.ascontiguousarray(
        plane[:, :, :, POFF[pc]:POFF[pc] + PW[pc]].transpose(0, 2, 1, 3))
        for pc in range(NP)]

    # ---- dup passes (source occ0 slot -> dup slot)
    e_rep, e_oth = eslot[rep_p], eslot[oth_p]

    def mk_pass(mask, W):
        if not mask.any():
            return None
        pl = np.full((NCORES, 128, NT, W), -1, np.int16)
        pl[core[rep_p[mask]], pp[rep_p[mask]], t[rep_p[mask]],
           e_rep[mask]] = e_oth[mask].astype(np.int16)
        return pl

    main_g = grp_piece <= 2
    if narrow_g.any():
        assert (e_rep[narrow_g] < WN).all() and (e_oth[narrow_g] < WN).all()
    pass_main = mk_pass(main_g, NE)
    pass_p3 = mk_pass(grp_piece == 3, WN) if WN else None
    pass_p4 = mk_pass(grp_piece == 4, WN) if WN else None

    nmat = n_per_row.reshape(NCORES, NT, 128)
    pairmask = (np.arange(SP)[None, None, None, :] < nmat[..., None]).astype(F16)
    bp = bad_pairs
    pairmask[r[bp] // RPC, (r[bp] % RPC) // 128, r[bp] % 128, srank[bp]] = 0
    pairmask = np.ascontiguousarray(pairmask.transpose(0, 2, 1, 3))  # [C,128,NT,SP]

    return dict(plane_lv=plane_lv, pass_main=pass_main, pass_p3=pass_p3,
                pass_p4=pass_p4, pairmask=pairmask, SP=SP, NE=NE, WN=WN,
                bad_pairs=bad_pairs)


# ------------------------------------------------------------- device kernel
@with_exitstack
def _build(ctx: ExitStack, tc: "tile.TileContext", io: dict, SP: int, NE: int,
           WN: int, have_main: bool, have_p3: bool, have_p4: bool):
    nc = tc.nc
    y_d, out_d = io["y8"], io["out"]

    consts = ctx.enter_context(tc.tile_pool(name="consts", bufs=1))
    ones_f32c = consts.tile([128, 1], F32, tag="ones_f32c")
    nc.vector.memset(ones_f32c[:], 1.0)
    wz = consts.tile([128, 2, 128], F8, tag="wz")
    nc.vector.memset(wz[:], 0.0)

    ypool = ctx.enter_context(tc.tile_pool(name="y", bufs=1))
    y = ypool.tile([128, NCH, 2 * NKP, CW], F8, tag="y", name="y")

    gpool = ctx.enter_context(tc.tile_pool(name="gbf", bufs=1))
    gbf = {tt: gpool.tile([128, B], DF16, tag=f"gbf{tt}", name=f"gbf{tt}")
           for tt in range(NT)}
    plpool = ctx.enter_context(tc.tile_pool(name="plane", bufs=1))
    pl = {pc: plpool.tile([128, NT, PW[pc]], I16, tag=f"plv{pc}",
                          name=f"plv{pc}") for pc in range(NP)}
    papool = ctx.enter_context(tc.tile_pool(name="passes", bufs=1))
    pam = papool.tile([128, NT, NE], I16, tag="pam", name="pam") if have_main else None
    pa3 = papool.tile([128, NT, WN], I16, tag="pa3", name="pa3") if have_p3 else None
    pa4 = papool.tile([128, NT, WN], I16, tag="pa4", name="pa4") if have_p4 else None

    lpool = ctx.enter_context(tc.tile_pool(name="loss", bufs=1))
    pmall = lpool.tile([128, NT, SP], DF16, tag="pmall")

    # ---- DMA: chunk 7 dead last, everything else just-in-time
    nc.sync.dma_start(y[:, 0], y_d[0])
    nc.sync.dma_start(y[:, 1], y_d[1])
    nc.sync.dma_start(pl[0][:], io["plane0"][:])
    nc.sync.dma_start(y[:, 2], y_d[2])
    nc.sync.dma_start(y[:, 3], y_d[3])
    nc.sync.dma_start(pl[1][:], io["plane1"][:])
    nc.sync.dma_start(y[:, 4], y_d[4])
    nc.sync.dma_start(y[:, 5], y_d[5])
    nc.sync.dma_start(pl[2][:], io["plane2"][:])
    nc.sync.dma_start(y[:, 6], y_d[6])
    nc.sync.dma_start(pl[3][:], io["plane3"][:])
    nc.sync.dma_start(y[:, 7], y_d[7])
    nc.sync.dma_start(pl[4][:], io["plane4"][:])
    if have_main:
        nc.sync.dma_start(pam[:], io["passm"][:])
    if have_p3:
        nc.sync.dma_start(pa3[:], io["pass3"][:])
    nc.sync.dma_start(pmall[:].rearrange("p t s -> p (t s)"),
                      io["pm"][:].rearrange("p t s -> p (t s)"))
    if have_p4:
        nc.sync.dma_start(pa4[:], io["pass4"][:])

    dpool = ctx.enter_context(tc.tile_pool(name="dq", bufs=2))
    hpool = ctx.enter_context(tc.tile_pool(name="hacc", bufs=1))
    expool = ctx.enter_context(tc.tile_pool(name="extra", bufs=2))
    elpool = ctx.enter_context(tc.tile_pool(name="elb", bufs=2))
    hacc = {tt: hpool.tile([128, NE], DF16, tag=f"hacc{tt}", name=f"hacc{tt}")
            for tt in range(NT)}
    denall = lpool.tile([128, NT, SP], F32, tag="denall")
    l0all = lpool.tile([128, NT, SP], DF16, tag="l0all")

    # preload the Exp activation table during the initial DMA idle
    pre = elpool.tile([128, 1], F32, tag="pre")
    nc.scalar.activation(pre[:], ones_f32c[:], AF.Exp)

    dq = {}

    def scatter_piece(pc, tiles=range(NT)):
        for tt in tiles:
            d = dpool.tile([128, NE], DF16, tag=f"d{tt}", name=f"d{tt}_{pc}")
            dq[(tt, pc)] = d
            nc.gpsimd.local_scatter(
                d[:], gbf[tt][:, POFF[pc]:POFF[pc] + PW[pc]],
                pl[pc][:, tt, :], 128, NE, PW[pc])

    with tc.tile_pool(name="gpsum", bufs=1, space="PSUM") as gpsum:
        wps0 = gpsum.tile([128, CW], F32, tag="ps0_1", name="warm_init")
        for i in range(WARM0):
            nc.tensor.matmul(wps0[:, 0:64], lhsT=wz[:], rhs=wz[:, :, 0:64],
                             start=True, stop=True, perf_mode=DR)
        for c in range(NCH):
            for tt in range(NT):
                ps = gpsum.tile([128, CW], F32, tag=f"ps{tt}_{c % 2}",
                                name=f"ps{tt}_{c}")
                for kp in range(NKP):
                    nc.tensor.matmul(
                        ps[:],
                        lhsT=y[:, 0, 2 * kp:2 * kp + 2, tt * 128:(tt + 1) * 128],
                        rhs=y[:, c, 2 * kp:2 * kp + 2, :],
                        start=(kp == 0), stop=(kp == NKP - 1),
                        perf_mode=DR,
                    )
                dst = gbf[tt][:, c * CW:(c + 1) * CW]
                if (c * NT + tt) % 2 == 0:
                    nc.vector.tensor_copy(dst, ps[:])
                else:
                    nc.scalar.copy(dst, ps[:])
            # p-state bridge across the DMA-paced idle before the next chunk
            if WARMS[c]:
                wps = gpsum.tile([128, CW], F32, tag=f"ps0_{(c + 1) % 2}",
                                 name=f"warm{c}")
                for i in range(WARMS[c]):
                    nc.tensor.matmul(wps[:, 0:64], lhsT=wz[:],
                                     rhs=wz[:, :, 0:64],
                                     start=True, stop=True, perf_mode=DR)
            if c == 1:
                scatter_piece(0)
            elif c == 3:
                scatter_piece(1)
            elif c == 5:
                scatter_piece(2)
                # piece 0+1 sums issue after c5 drains: ready (sc1 done)
                # without blocking any drain behind them
                for tt in range(NT):
                    nc.vector.tensor_tensor(hacc[tt][:], dq[(tt, 0)][:],
                                            dq[(tt, 1)][:], ALU.add)
            elif c == 6:
                scatter_piece(3)

        # ---- tail: piece-4 scatters, then per-tile passes + exp/den
        scatter_piece(4)
        for tt in range(NT):
            nc.vector.tensor_tensor(hacc[tt][:], hacc[tt][:],
                                    dq[(tt, 2)][:], ALU.add)
        for tt in range(NT):
            if have_main:
                e = expool.tile([128, NE], DF16, tag=f"eAm{tt % 2}",
                                name=f"eAm{tt}")
                nc.gpsimd.local_scatter(e[:], hacc[tt][:],
                                        pam[:, tt, :], 128, NE, NE)
                nc.vector.tensor_tensor(hacc[tt][:], hacc[tt][:],
                                        e[:], ALU.add)
            if have_p3:
                e = expool.tile([128, WN], DF16, tag=f"eA3{tt % 2}",
                                name=f"eA3{tt}")
                nc.gpsimd.local_scatter(e[:], dq[(tt, 3)][:, 0:WN],
                                        pa3[:, tt, :], 128, WN, WN)
                nc.vector.tensor_tensor(hacc[tt][:, 0:WN],
                                        hacc[tt][:, 0:WN], e[:], ALU.add)
            if have_p4:
                e = expool.tile([128, WN], DF16, tag=f"eA4{tt % 2}",
                                name=f"eA4{tt}")
                nc.gpsimd.local_scatter(e[:], dq[(tt, 4)][:, 0:WN],
                                        pa4[:, tt, :], 128, WN, WN)
                nc.vector.tensor_tensor(hacc[tt][:, 0:WN], hacc[tt][:, 0:WN],
                                        e[:], ALU.add)
            nc.vector.tensor_tensor(hacc[tt][:], hacc[tt][:], dq[(tt, 3)][:],
                                    ALU.add)
            nc.vector.tensor_tensor(hacc[tt][:], hacc[tt][:], dq[(tt, 4)][:],
                                    ALU.add)
            ebuf = elpool.tile([128, NE], F32, tag="ebuf")
            nc.scalar.activation(ebuf[:], hacc[tt][:], AF.Exp, scale=KSC)
            e3 = ebuf[:, 0:SP * J].rearrange("p (s j) -> p s j", j=J)
            nc.vector.tensor_reduce(denall[:, tt, :], e3, AX.X, ALU.add)
            l0 = hacc[tt][:, 0:SP * J].rearrange("p (s j) -> p s j", j=J)[:, :, 0]
            nc.vector.tensor_copy(l0all[:, tt, :], l0)

    # ---- batched tail: Ln (table load hidden behind last den), diff, mask
    with tc.tile_pool(name="p5psum", bufs=1, space="PSUM") as p5psum:
        pre2 = elpool.tile([128, 1], F32, tag="pre2")
        nc.scalar.activation(pre2[:], denall[:, NT - 1, 0:1], AF.Ln)
        lnd = lpool.tile([128, NT * SP], F32, tag="lnd")
        nc.scalar.activation(lnd[:], denall[:].rearrange("p t s -> p (t s)"),
                             AF.Ln)
        diff = lpool.tile([128, NT * SP], F32, tag="diff")
        nc.vector.scalar_tensor_tensor(
            diff[:], l0all[:].rearrange("p t s -> p (t s)"), -KSC, lnd[:],
            ALU.mult, ALU.add)
        scrap = lpool.tile([128, NT * SP], F32, tag="scrap")
        acc1 = lpool.tile([128, 1], F32, tag="acc1")
        nc.vector.scalar_tensor_tensor(
            scrap[:], diff[:], 1.0,
            pmall[:].rearrange("p t s -> p (t s)"), ALU.mult, ALU.mult,
            accum_out=acc1[:])
        ps = p5psum.tile([1, 1], F32, tag="ps_out")
        nc.tensor.matmul(ps[:], lhsT=acc1[:], rhs=ones_f32c[:, 0:1],
                         start=True, stop=True)
        res = lpool.tile([1, 1], F32, tag="res")
        nc.scalar.copy(res[:], ps[:])
        nc.sync.dma_start(out_d[:], res[:])


def build_nc(SP, NE, WN, have_main, have_p3, have_p4, enable_asserts=False):
    nc = bacc.Bacc("TRN2", target_bir_lowering=False, debug=False,
                   enable_asserts=enable_asserts, num_devices=NCORES)
    io = {
        "y8": nc.dram_tensor("y8", [NCH, 128, 2 * NKP, CW], F8,
                             kind="ExternalInput").ap(),
        "pm": nc.dram_tensor("pm", [128, NT, SP], DF16,
                             kind="ExternalInput").ap(),
        "out": nc.dram_tensor("out", [1, 1], F32, kind="ExternalOutput").ap(),
    }
    for pc in range(NP):
        io[f"plane{pc}"] = nc.dram_tensor(
            f"plane{pc}", [128, NT, PW[pc]], I16, kind="ExternalInput").ap()
    if have_main:
        io["passm"] = nc.dram_tensor("passm", [128, NT, NE], I16,
                                     kind="ExternalInput").ap()
    if have_p3:
        io["pass3"] = nc.dram_tensor("pass3", [128, NT, WN], I16,
                                     kind="ExternalInput").ap()
    if have_p4:
        io["pass4"] = nc.dram_tensor("pass4", [128, NT, WN], I16,
                                     kind="ExternalInput").ap()
    with tile.TileContext(nc) as tc:
        _build(tc, io, SP, NE, WN, have_main, have_p3, have_p4)
    nc.compile()
    return nc


def _normalize(x):
    x = np.asarray(x, np.float32)
    w = np.sqrt((x.astype(np.float64) ** 2).sum(axis=1, keepdims=True))
    w = np.maximum(w, 1e-8)
    return (x / w).astype(np.float32)


def make_in_maps(x, plan):
    xn = _normalize(x)
    x8 = np.clip(xn * S, -240.0, 240.0).astype(FP8)
    in_maps = []
    for k in range(NCORES):
        xr = np.roll(x8, -RPC * k, axis=0)                     # [B, D]
        y8 = xr.T.reshape(2 * NKP, 128, B).transpose(1, 0, 2)  # [128, 16, B]
        y8c = np.ascontiguousarray(
            y8.reshape(128, 2 * NKP, NCH, CW).transpose(2, 0, 1, 3))
        m = {"y8": y8c, "pm": plan["pairmask"][k]}
        for pc in range(NP):
            m[f"plane{pc}"] = plan["plane_lv"][pc][k]
        if plan["pass_main"] is not None:
            m["passm"] = plan["pass_main"][k]
        if plan["pass_p3"] is not None:
            m["pass3"] = plan["pass_p3"][k]
        if plan["pass_p4"] is not None:
            m["pass4"] = plan["pass_p4"][k]
        in_maps.append(m)
    return in_maps


def host_fixup(x, anchor_idx, pos_idx, neg_idx, bad_pairs):
    """Exact loss terms for pairs masked out on the device."""
    if len(bad_pairs) == 0:
        return 0.0
    xn = _normalize(x).astype(np.float64)
    a = anchor_idx[bad_pairs]
    cols = np.concatenate([pos_idx[bad_pairs][:, None], neg_idx[bad_pairs]],
                          axis=1)
    logits = np.einsum("pd,pjd->pj", xn[a], xn[cols]) / TEMP
    mx = logits.max(axis=1, keepdims=True)
    lse = np.log(np.exp(logits - mx).sum(axis=1)) + mx[:, 0]
    return float((lse - logits[:, 0]).sum())


def kernel(**inputs):
    x = np.asarray(inputs["x"], np.float32)
    anchor_idx = np.asarray(inputs["anchor_idx"])
    pos_idx = np.asarray(inputs["pos_idx"])
    neg_idx = np.asarray(inputs["neg_idx"])
    P = anchor_idx.shape[0]

    plan = build_plan(anchor_idx, pos_idx, neg_idx)
    nc = build_nc(plan["SP"], plan["NE"], plan["WN"],
                  plan["pass_main"] is not None, plan["pass_p3"] is not None,
                  plan["pass_p4"] is not None)
    in_maps = make_in_maps(x, plan)
    res = run_bass_kernel_spmd(nc, in_maps, list(range(NCORES)))
    total = sum(float(res.results[k]["out"][0, 0]) for k in range(NCORES))
    total += host_fixup(x, anchor_idx, pos_idx, neg_idx, plan["bad_pairs"])
    return np.float32(total / P)


# revision 24
# speedup vs baseline: 1.2253x; 1.1340x over previous
"""Trainium2 Bass kernel for ContrastiveNet loss (v11).

Per core k of 8 (SPMD):
  - host: xn = x/||x||*S (S=32) in fp8e4 -> sim = G/(S^2*TEMP); no on-device
    normalization. Rows are PERMUTED: sorted by pair count and banded into
    tiles (tile 0 heaviest), dealt round-robin across cores, so the per-tile
    slot space NE_t shrinks (~694/520/350/180 instead of 4x694) and the last
    tile's tail is the lightest. Core k's 512 rows sit at rotated columns
    0..511 of its gram; y is COLUMN-CHUNK-major ([8][128][16][512]) so each
    chunk's gram (4 row-tiles x 8 kp fp8 DoubleRow matmuls into [128,512]
    PSUM) starts as the chunk lands; chunk 7 is dead last in the DMA stream.
  - gather: per (tile, piece 1024/1024/1024/512/512) gpsimd local_scatter
    with HBM col->slot planes (int16, -1 unused), accumulated per tile.
    2-member duplicate (row,col) groups are fixed by scatter passes:
    main pass (rep col in pieces 0-2, width NE_t) and narrow piece-3/4
    passes (their pairs ranked into the lowest slots). Pairs touching
    >=3-member groups (~2%) are masked out and computed exactly on host.
  - device ships den (softmax denominators) + l0 per pair; the final
    ln/mask/sum runs on host along with the masked pairs.
  - PE p-state: tiny warm matmuls bridge DMA-paced idle gaps.
"""
import os
import sys
import numpy as np
import ml_dtypes

try:
    import concourse  # noqa: F401
except ImportError:
    sys.path.insert(0, "/opt/trn_rl_repo")

from contextlib import ExitStack

import concourse.bass as bass
import concourse.tile as tile
from concourse import bacc, mybir
from concourse._compat import with_exitstack
from concourse.bass_utils import run_bass_kernel_spmd

F16 = np.float16
FP8 = ml_dtypes.float8_e4m3
F32 = mybir.dt.float32
DF16 = mybir.dt.float16
F8 = mybir.dt.float8e4
I16 = mybir.dt.int16

B, D, J = 4096, 2048, 11
NCORES, RPC, NT, NKP = 8, 512, 4, 8
NCH, CW = 8, 512                    # gram column chunks (per core)
POFF = [0, 1024, 2048, 3072, 3584]  # scatter piece offsets
PW = [1024, 1024, 1024, 512, 512]   # scatter piece widths
NP = 5
TEMP = 0.1
S = 32.0
KSC = 1.0 / (S * S * TEMP)
WARMS = [0, 55, 0, 0, 35, 50, 45, 0]
WARM0 = 130
AF = mybir.ActivationFunctionType
ALU = mybir.AluOpType
AX = mybir.AxisListType
DR = mybir.MatmulPerfMode.DoubleRow


def _even(n):
    return int(n) + (int(n) % 2)


# ---------------------------------------------------------------- host prep
def build_plan(anchor_idx, pos_idx, neg_idx):
    r0 = anchor_idx.astype(np.int64)
    cols0 = np.concatenate([pos_idx[:, None], neg_idx], axis=1).astype(np.int64)
    P = r0.shape[0]

    # ---- row permutation: band rows by pair count (heaviest -> tile 0),
    # deal each band round-robin across cores
    n0 = np.bincount(r0, minlength=B)
    order_rows = np.argsort(-n0, kind="stable")
    virt = np.empty(B, np.int64)      # original row -> virtual row
    ranks = np.arange(B)
    t_of = ranks // (128 * NCORES)
    i_in = ranks % (128 * NCORES)
    k_of = i_in % NCORES
    p_of = i_in // NCORES
    virt[order_rows] = k_of * RPC + t_of * 128 + p_of

    r = virt[r0]
    cols = virt[cols0]

    er = np.repeat(r, J)
    ec = cols.ravel()
    pair_of = np.repeat(np.arange(P), J)
    core = er // RPC
    t = (er % RPC) // 128
    pp = er % 128
    ec_rot = (ec - core * RPC) % B
    piece = np.searchsorted(POFF, ec_rot, side="right") - 1

    key = er * B + ec
    o2 = np.argsort(key, kind="stable")
    k_sorted = key[o2]
    first2 = np.r_[True, k_sorted[1:] != k_sorted[:-1]]
    gid_sorted = np.cumsum(first2) - 1
    NG = int(gid_sorted[-1]) + 1
    gid = np.empty(P * J, np.int64)
    gid[o2] = gid_sorted
    occ_sorted = np.arange(P * J) - np.flatnonzero(first2)[gid_sorted]
    gsz_g = np.bincount(gid_sorted, minlength=NG)
    gsz = gsz_g[gid]

    bad_pairs = np.unique(pair_of[gsz >= 3])
    badp = np.zeros(P, bool)
    badp[bad_pairs] = True

    # 2-member groups: rep = member in the earliest piece
    two = gsz == 2
    order = np.lexsort((np.arange(P * J), piece, gid))
    go = order[two[order]]
    g_of_go = gid[go]
    firstg = np.r_[True, g_of_go[1:] != g_of_go[:-1]]
    rep = go[firstg]
    oth = go[~firstg]
    rep_bad = badp[pair_of[rep]]
    oth_bad = badp[pair_of[oth]]
    swap = rep_bad & ~oth_bad
    pk = ~rep_bad & ~oth_bad
    rep_p, oth_p = rep[pk], oth[pk]
    grp_piece = piece[rep_p]
    narrow_g = grp_piece >= 3

    # pairs touching narrow-pass groups ranked first
    narrow_pairs = np.unique(np.r_[pair_of[rep_p[narrow_g]],
                                   pair_of[oth_p[narrow_g]]])
    sev = np.zeros(P, np.int64)
    sev[narrow_pairs] = 1
    order_p = np.lexsort((np.arange(P), -sev, r))
    r_sp = r[order_p]
    firstp = np.r_[True, r_sp[1:] != r_sp[:-1]]
    gidp = np.cumsum(firstp) - 1
    rank_sorted = np.arange(P) - np.flatnonzero(firstp)[gidp]
    srank = np.empty(P, np.int64)
    srank[order_p] = rank_sorted

    n_per_row = np.bincount(r, minlength=B)
    # per-band (tile) widths
    nmat = n_per_row.reshape(NCORES, NT, 128)
    SPt = [max(int(nmat[:, tt, :].max()), 1) for tt in range(NT)]
    NEt = [_even(SPt[tt] * J) for tt in range(NT)]
    assert max(NEt) * 32 < 2**16
    SPOFF = np.r_[0, np.cumsum(SPt)].astype(int)
    NEOFF = np.r_[0, np.cumsum(NEt)].astype(int)

    eslot = srank[pair_of] * J + np.tile(np.arange(J), P)

    if len(narrow_pairs):
        cnt_n = np.bincount(r[narrow_pairs], minlength=B)
    else:
        cnt_n = np.zeros(B, np.int64)
    cn = cnt_n.reshape(NCORES, NT, 128)
    WNt = [_even(min(int(cn[:, tt, :].max()) * J + 2, NEt[tt]))
           if cn[:, tt, :].max() > 0 else 0 for tt in range(NT)]
    WNOFF = np.r_[0, np.cumsum(WNt)].astype(int)

    # ---- main scatter plane: col -> slot of occ0 entries
    is_rep = np.ones(P * J, bool)
    is_rep[oth] = False
    is_rep[oth[swap]] = True
    m0 = is_rep & ~badp[pair_of]
    plane = np.full((NCORES, NT, 128, B), -1, np.int16)
    plane[core[m0], t[m0], pp[m0], ec_rot[m0]] = eslot[m0].astype(np.int16)
    plane_lv = [np.ascontiguousarray(
        plane[:, :, :, POFF[pc]:POFF[pc] + PW[pc]].transpose(0, 2, 1, 3))
        for pc in range(NP)]

    # ---- dup passes, packed per-tile widths
    e_rep, e_oth = eslot[rep_p], eslot[oth_p]

    def mk_pass(mask, widths, woff, check=True):
        tot = int(woff[-1])
        if tot == 0 or not mask.any():
            return None
        pl = np.full((NCORES, 128, tot), -1, np.int16)
        src = e_rep[mask]
        tgt = e_oth[mask]
        tts = t[rep_p[mask]]
        offs = np.asarray(woff)[tts]
        ww = np.asarray(widths)[tts]
        ok = (src < ww) & (tgt < ww)
        if check:
            assert ok.all(), "pass slot overflow"
        pl[core[rep_p[mask]][ok], pp[rep_p[mask]][ok],
           (offs + src)[ok]] = tgt[ok].astype(np.int16)
        return pl

    main_g = grp_piece <= 2
    pass_main = mk_pass(main_g, NEt, NEOFF)
    pass_p3 = mk_pass(grp_piece == 3, WNt, WNOFF)
    pass_p4 = mk_pass(grp_piece == 4, WNt, WNOFF)

    pairmask = np.zeros((NCORES, 128, int(SPOFF[-1])), F16)
    for tt in range(NT):
        pm_t = (np.arange(SPt[tt])[None, None, :] <
                nmat[:, tt, :][:, :, None]).astype(F16)
        pairmask[:, :, SPOFF[tt]:SPOFF[tt + 1]] = pm_t
    bp = bad_pairs
    tb = (r[bp] % RPC) // 128
    pairmask[r[bp] // RPC, r[bp] % 128,
             np.asarray(SPOFF)[tb] + srank[bp]] = 0

    return dict(plane_lv=plane_lv, pass_main=pass_main, pass_p3=pass_p3,
                pass_p4=pass_p4, pairmask=pairmask, order_rows=order_rows,
                SPt=SPt, NEt=NEt, WNt=WNt, SPOFF=SPOFF, NEOFF=NEOFF,
                WNOFF=WNOFF, bad_pairs=bad_pairs)


# ------------------------------------------------------------- device kernel
@with_exitstack
def _build(ctx: ExitStack, tc: "tile.TileContext", io: dict, SPt, NEt, WNt,
           SPOFF, NEOFF, WNOFF, have_main, have_p3, have_p4):
    nc = tc.nc
    y_d = io["y8"]
    SPSUM, NESUM, WNSUM = int(SPOFF[-1]), int(NEOFF[-1]), int(WNOFF[-1])

    consts = ctx.enter_context(tc.tile_pool(name="consts", bufs=1))
    wz = consts.tile([128, 2, 128], F8, tag="wz")
    nc.vector.memset(wz[:], 0.0)

    ypool = ctx.enter_context(tc.tile_pool(name="y", bufs=1))
    y = ypool.tile([128, NCH, 2 * NKP, CW], F8, tag="y", name="y")

    gpool = ctx.enter_context(tc.tile_pool(name="gbf", bufs=1))
    gbf = {tt: gpool.tile([128, B], DF16, tag=f"gbf{tt}", name=f"gbf{tt}")
           for tt in range(NT)}
    plpool = ctx.enter_context(tc.tile_pool(name="plane", bufs=1))
    pl = {pc: plpool.tile([128, NT, PW[pc]], I16, tag=f"plv{pc}",
                          name=f"plv{pc}") for pc in range(NP)}
    papool = ctx.enter_context(tc.tile_pool(name="passes", bufs=1))
    pam = papool.tile([128, NESUM], I16, tag="pam", name="pam") \
        if have_main else None
    pa3 = papool.tile([128, WNSUM], I16, tag="pa3", name="pa3") \
        if have_p3 else None
    pa4 = papool.tile([128, WNSUM], I16, tag="pa4", name="pa4") \
        if have_p4 else None

    lpool = ctx.enter_context(tc.tile_pool(name="loss", bufs=1))
    denall = lpool.tile([128, SPSUM], F32, tag="denall")
    l0all = lpool.tile([128, SPSUM], DF16, tag="l0all")

    # ---- DMA: chunk 7 dead last, planes just-in-time
    nc.sync.dma_start(y[:, 0], y_d[0])
    nc.sync.dma_start(y[:, 1], y_d[1])
    nc.sync.dma_start(pl[0][:], io["plane0"][:])
    nc.sync.dma_start(y[:, 2], y_d[2])
    nc.sync.dma_start(y[:, 3], y_d[3])
    nc.sync.dma_start(pl[1][:], io["plane1"][:])
    nc.sync.dma_start(y[:, 4], y_d[4])
    nc.sync.dma_start(y[:, 5], y_d[5])
    nc.sync.dma_start(pl[2][:], io["plane2"][:])
    nc.sync.dma_start(y[:, 6], y_d[6])
    nc.sync.dma_start(pl[3][:], io["plane3"][:])
    nc.sync.dma_start(y[:, 7], y_d[7])
    nc.sync.dma_start(pl[4][:], io["plane4"][:])
    if have_main:
        nc.sync.dma_start(pam[:], io["passm"][:])
    if have_p3:
        nc.sync.dma_start(pa3[:], io["pass3"][:])
    if have_p4:
        nc.sync.dma_start(pa4[:], io["pass4"][:])

    dpool = ctx.enter_context(tc.tile_pool(name="dq", bufs=2))
    hpool = ctx.enter_context(tc.tile_pool(name="hacc", bufs=1))
    expool = ctx.enter_context(tc.tile_pool(name="extra", bufs=2))
    elpool = ctx.enter_context(tc.tile_pool(name="elb", bufs=2))
    hacc = {tt: hpool.tile([128, NEt[tt]], DF16, tag=f"hacc{tt}",
                           name=f"hacc{tt}") for tt in range(NT)}

    # preload the Exp activation table during the initial DMA idle
    pre = elpool.tile([128, 1], F32, tag="pre")
    nc.scalar.activation(pre[:], wz[:, 0, 0:1], AF.Exp)

    dq = {}

    def scatter_piece(pc, tiles=range(NT)):
        for tt in tiles:
            d = dpool.tile([128, NEt[tt]], DF16, tag=f"d{tt}",
                           name=f"d{tt}_{pc}")
            dq[(tt, pc)] = d
            nc.gpsimd.local_scatter(
                d[:], gbf[tt][:, POFF[pc]:POFF[pc] + PW[pc]],
                pl[pc][:, tt, :], 128, NEt[tt], PW[pc])

    with tc.tile_pool(name="gpsum", bufs=1, space="PSUM") as gpsum:
        wps0 = gpsum.tile([128, CW], F32, tag="ps0_1", name="warm_init")
        for i in range(WARM0):
            nc.tensor.matmul(wps0[:, 0:64], lhsT=wz[:], rhs=wz[:, :, 0:64],
                             start=True, stop=True, perf_mode=DR)
        for c in range(NCH):
            for tt in range(NT):
                ps = gpsum.tile([128, CW], F32, tag=f"ps{tt}_{c % 2}",
                                name=f"ps{tt}_{c}")
                for kp in range(NKP):
                    nc.tensor.matmul(
                        ps[:],
                        lhsT=y[:, 0, 2 * kp:2 * kp + 2, tt * 128:(tt + 1) * 128],
                        rhs=y[:, c, 2 * kp:2 * kp + 2, :],
                        start=(kp == 0), stop=(kp == NKP - 1),
                        perf_mode=DR,
                    )
                dst = gbf[tt][:, c * CW:(c + 1) * CW]
                if (c * NT + tt) % 2 == 0:
                    nc.vector.tensor_copy(dst, ps[:])
                else:
                    nc.scalar.copy(dst, ps[:])
            if WARMS[c]:
                wps = gpsum.tile([128, CW], F32, tag=f"ps0_{(c + 1) % 2}",
                                 name=f"warm{c}")
                for i in range(WARMS[c]):
                    nc.tensor.matmul(wps[:, 0:64], lhsT=wz[:],
                                     rhs=wz[:, :, 0:64],
                                     start=True, stop=True, perf_mode=DR)
            if c == 1:
                scatter_piece(0)
            elif c == 3:
                scatter_piece(1)
            elif c == 5:
                scatter_piece(2)
                for tt in range(NT):
                    nc.vector.tensor_tensor(hacc[tt][:], dq[(tt, 0)][:],
                                            dq[(tt, 1)][:], ALU.add)
            elif c == 6:
                scatter_piece(3)

        # ---- tail: piece-4 scatters, then per-tile passes + exp/den
        scatter_piece(4)
        for tt in range(NT):
            nc.vector.tensor_tensor(hacc[tt][:], hacc[tt][:],
                                    dq[(tt, 2)][:], ALU.add)
        for tt in range(NT):
            NEc, WNc = NEt[tt], WNt[tt]
            if have_main:
                e = expool.tile([128, NEc], DF16, tag=f"eAm{tt % 2}",
                                name=f"eAm{tt}")
                nc.gpsimd.local_scatter(
                    e[:], hacc[tt][:], pam[:, NEOFF[tt]:NEOFF[tt + 1]],
                    128, NEc, NEc)
                nc.vector.tensor_tensor(hacc[tt][:], hacc[tt][:],
                                        e[:], ALU.add)
            if have_p3 and WNc:
                e = expool.tile([128, WNc], DF16, tag=f"eA3{tt % 2}",
                                name=f"eA3{tt}")
                nc.gpsimd.local_scatter(
                    e[:], dq[(tt, 3)][:, 0:WNc],
                    pa3[:, WNOFF[tt]:WNOFF[tt + 1]], 128, WNc, WNc)
                nc.vector.tensor_tensor(hacc[tt][:, 0:WNc],
                                        hacc[tt][:, 0:WNc], e[:], ALU.add)
            if have_p4 and WNc:
                e = expool.tile([128, WNc], DF16, tag=f"eA4{tt % 2}",
                                name=f"eA4{tt}")
                nc.gpsimd.local_scatter(
                    e[:], dq[(tt, 4)][:, 0:WNc],
                    pa4[:, WNOFF[tt]:WNOFF[tt + 1]], 128, WNc, WNc)
                nc.vector.tensor_tensor(hacc[tt][:, 0:WNc],
                                        hacc[tt][:, 0:WNc], e[:], ALU.add)
            u = expool.tile([128, NEc], DF16, tag=f"u{tt % 2}", name=f"u{tt}")
            nc.vector.tensor_tensor(u[:], dq[(tt, 3)][:], dq[(tt, 4)][:],
                                    ALU.add)
            nc.vector.tensor_tensor(hacc[tt][:], hacc[tt][:], u[:], ALU.add)
            ebuf = elpool.tile([128, NEt[0]], F32, tag="ebuf")
            nc.scalar.activation(ebuf[:, 0:NEc], hacc[tt][:], AF.Exp,
                                 scale=KSC)
            e3 = ebuf[:, 0:SPt[tt] * J].rearrange("p (s j) -> p s j", j=J)
            nc.vector.tensor_reduce(denall[:, SPOFF[tt]:SPOFF[tt + 1]], e3,
                                    AX.X, ALU.add)
            l0 = hacc[tt][:, 0:SPt[tt] * J].rearrange(
                "p (s j) -> p s j", j=J)[:, :, 0]
            nc.vector.tensor_copy(l0all[:, SPOFF[tt]:SPOFF[tt + 1]], l0)

    # ---- ship den + l0; ln/mask/sum finish on host
    nc.sync.dma_start(io["den"][:], denall[:])
    nc.sync.dma_start(io["l0"][:], l0all[:])


def build_nc(SPt, NEt, WNt, SPOFF, NEOFF, WNOFF, have_main, have_p3, have_p4,
             enable_asserts=False):
    nc = bacc.Bacc("TRN2", target_bir_lowering=False, debug=False,
                   enable_asserts=enable_asserts, num_devices=NCORES)
    SPSUM, NESUM, WNSUM = int(SPOFF[-1]), int(NEOFF[-1]), int(WNOFF[-1])
    io = {
        "y8": nc.dram_tensor("y8", [NCH, 128, 2 * NKP, CW], F8,
                             kind="ExternalInput").ap(),
        "den": nc.dram_tensor("den", [128, SPSUM], F32,
                              kind="ExternalOutput").ap(),
        "l0": nc.dram_tensor("l0", [128, SPSUM], DF16,
                             kind="ExternalOutput").ap(),
    }
    for pc in range(NP):
        io[f"plane{pc}"] = nc.dram_tensor(
            f"plane{pc}", [128, NT, PW[pc]], I16, kind="ExternalInput").ap()
    if have_main:
        io["passm"] = nc.dram_tensor("passm", [128, NESUM], I16,
                                     kind="ExternalInput").ap()
    if have_p3:
        io["pass3"] = nc.dram_tensor("pass3", [128, WNSUM], I16,
                                     kind="ExternalInput").ap()
    if have_p4:
        io["pass4"] = nc.dram_tensor("pass4", [128, WNSUM], I16,
                                     kind="ExternalInput").ap()
    with tile.TileContext(nc) as tc:
        _build(tc, io, SPt, NEt, WNt, SPOFF, NEOFF, WNOFF,
               have_main, have_p3, have_p4)
    nc.compile()
    return nc


def _normalize(x):
    x = np.asarray(x, np.float32)
    w = np.sqrt((x.astype(np.float64) ** 2).sum(axis=1, keepdims=True))
    w = np.maximum(w, 1e-8)
    return (x / w).astype(np.float32)


def make_in_maps(x, plan):
    xn = _normalize(x)
    x8 = np.clip(xn * S, -240.0, 240.0).astype(FP8)
    # virtual-row layout: band-rank i -> virtual row v (see build_plan)
    ranks = np.arange(B)
    t_of = ranks // (128 * NCORES)
    i_in = ranks % (128 * NCORES)
    k_of = i_in % NCORES
    p_of = i_in // NCORES
    v_of = k_of * RPC + t_of * 128 + p_of
    xvirt = np.empty_like(x8)
    xvirt[v_of] = x8[plan["order_rows"]]
    in_maps = []
    for k in range(NCORES):
        xr = np.roll(xvirt, -RPC * k, axis=0)                  # [B, D]
        y8 = xr.T.reshape(2 * NKP, 128, B).transpose(1, 0, 2)  # [128, 16, B]
        y8c = np.ascontiguousarray(
            y8.reshape(128, 2 * NKP, NCH, CW).transpose(2, 0, 1, 3))
        m = {"y8": y8c}
        for pc in range(NP):
            m[f"plane{pc}"] = plan["plane_lv"][pc][k]
        if plan["pass_main"] is not None:
            m["passm"] = plan["pass_main"][k]
        if plan["pass_p3"] is not None:
            m["pass3"] = plan["pass_p3"][k]
        if plan["pass_p4"] is not None:
            m["pass4"] = plan["pass_p4"][k]
        in_maps.append(m)
    return in_maps


def host_fixup(x, anchor_idx, pos_idx, neg_idx, bad_pairs):
    if len(bad_pairs) == 0:
        return 0.0
    xn = _normalize(x).astype(np.float64)
    a = anchor_idx[bad_pairs]
    cols = np.concatenate([pos_idx[bad_pairs][:, None], neg_idx[bad_pairs]],
                          axis=1)
    logits = np.einsum("pd,pjd->pj", xn[a], xn[cols]) / TEMP
    mx = logits.max(axis=1, keepdims=True)
    lse = np.log(np.exp(logits - mx).sum(axis=1)) + mx[:, 0]
    return float((lse - logits[:, 0]).sum())


def kernel(**inputs):
    x = np.asarray(inputs["x"], np.float32)
    anchor_idx = np.asarray(inputs["anchor_idx"])
    pos_idx = np.asarray(inputs["pos_idx"])
    neg_idx = np.asarray(inputs["neg_idx"])
    P = anchor_idx.shape[0]

    plan = build_plan(anchor_idx, pos_idx, neg_idx)
    nc = build_nc(plan["SPt"], plan["NEt"], plan["WNt"], plan["SPOFF"],
                  plan["NEOFF"], plan["WNOFF"],
                  plan["pass_main"] is not None,
                  plan["pass_p3"] is not None,
                  plan["pass_p4"] is not None)
    in_maps = make_in_maps(x, plan)
    res = run_bass_kernel_spmd(nc, in_maps, list(range(NCORES)))
    total = 0.0
    for k in range(NCORES):
        den = np.asarray(res.results[k]["den"], np.float64)   # [128, SPSUM]
        l0 = np.asarray(res.results[k]["l0"], np.float64)
        pm = np.asarray(plan["pairmask"][k], np.float64)
        total += float((pm * (np.log(den) - KSC * l0)).sum())
    total += host_fixup(x, anchor_idx, pos_idx, neg_idx, plan["bad_pairs"])
    return np.float32(total / P)


# revision 26
# speedup vs baseline: 1.2275x; 1.0018x over previous
"""Trainium2 Bass kernel for ContrastiveNet loss (v11).

Per core k of 8 (SPMD):
  - host: xn = x/||x||*S (S=32) in fp8e4 -> sim = G/(S^2*TEMP); no on-device
    normalization. Rows are PERMUTED: sorted by pair count and banded into
    tiles (tile 0 heaviest), dealt round-robin across cores, so the per-tile
    slot space NE_t shrinks (~694/520/350/180 instead of 4x694) and the last
    tile's tail is the lightest. Core k's 512 rows sit at rotated columns
    0..511 of its gram; y is COLUMN-CHUNK-major ([8][128][16][512]) so each
    chunk's gram (4 row-tiles x 8 kp fp8 DoubleRow matmuls into [128,512]
    PSUM) starts as the chunk lands; chunk 7 is dead last in the DMA stream.
  - gather: per (tile, piece 1024/1024/1024/512/512) gpsimd local_scatter
    with HBM col->slot planes (int16, -1 unused), accumulated per tile.
    2-member duplicate (row,col) groups are fixed by scatter passes:
    main pass (rep col in pieces 0-2, width NE_t) and narrow piece-3/4
    passes (their pairs ranked into the lowest slots). Pairs touching
    >=3-member groups (~2%) are masked out and computed exactly on host.
  - device ships den (softmax denominators) + l0 per pair; the final
    ln/mask/sum runs on host along with the masked pairs.
  - PE p-state: tiny warm matmuls bridge DMA-paced idle gaps.
"""
import os
import sys
import numpy as np
import ml_dtypes

try:
    import concourse  # noqa: F401
except ImportError:
    sys.path.insert(0, "/opt/trn_rl_repo")

from contextlib import ExitStack

import concourse.bass as bass
import concourse.tile as tile
from concourse import bacc, mybir
from concourse._compat import with_exitstack
from concourse.bass_utils import run_bass_kernel_spmd

F16 = np.float16
FP8 = ml_dtypes.float8_e4m3
F32 = mybir.dt.float32
DF16 = mybir.dt.float16
F8 = mybir.dt.float8e4
I16 = mybir.dt.int16

B, D, J = 4096, 2048, 11
NCORES, RPC, NT, NKP = 8, 512, 4, 8
NCH, CW = 8, 512                    # gram column chunks (per core)
POFF = [0, 1024, 2048, 3072, 3584]  # scatter piece offsets
PW = [1024, 1024, 1024, 512, 512]   # scatter piece widths
NP = 5
TEMP = 0.1
S = 32.0
KSC = 1.0 / (S * S * TEMP)
WARMS = [0, 55, 0, 0, 35, 50, 45, 0]
WARM0 = 130
AF = mybir.ActivationFunctionType
ALU = mybir.AluOpType
AX = mybir.AxisListType
DR = mybir.MatmulPerfMode.DoubleRow


def _even(n):
    return int(n) + (int(n) % 2)


# ---------------------------------------------------------------- host prep
def build_plan(anchor_idx, pos_idx, neg_idx):
    r0 = anchor_idx.astype(np.int64)
    cols0 = np.concatenate([pos_idx[:, None], neg_idx], axis=1).astype(np.int64)
    P = r0.shape[0]

    # ---- row permutation: band rows by pair count (heaviest -> tile 0),
    # deal each band round-robin across cores
    n0 = np.bincount(r0, minlength=B)
    order_rows = np.argsort(-n0, kind="stable")
    virt = np.empty(B, np.int64)      # original row -> virtual row
    ranks = np.arange(B)
    t_of = ranks // (128 * NCORES)
    i_in = ranks % (128 * NCORES)
    k_of = i_in % NCORES
    p_of = i_in // NCORES
    virt[order_rows] = k_of * RPC + t_of * 128 + p_of

    r = virt[r0]
    cols = virt[cols0]

    er = np.repeat(r, J)
    ec = cols.ravel()
    pair_of = np.repeat(np.arange(P), J)
    core = er // RPC
    t = (er % RPC) // 128
    pp = er % 128
    ec_rot = (ec - core * RPC) % B
    piece = np.searchsorted(POFF, ec_rot, side="right") - 1

    key = er * B + ec
    o2 = np.argsort(key, kind="stable")
    k_sorted = key[o2]
    first2 = np.r_[True, k_sorted[1:] != k_sorted[:-1]]
    gid_sorted = np.cumsum(first2) - 1
    NG = int(gid_sorted[-1]) + 1
    gid = np.empty(P * J, np.int64)
    gid[o2] = gid_sorted
    occ_sorted = np.arange(P * J) - np.flatnonzero(first2)[gid_sorted]
    gsz_g = np.bincount(gid_sorted, minlength=NG)
    gsz = gsz_g[gid]

    bad_pairs = np.unique(pair_of[gsz >= 3])
    badp = np.zeros(P, bool)
    badp[bad_pairs] = True

    # 2-member groups: rep = member in the earliest piece
    two = gsz == 2
    order = np.lexsort((np.arange(P * J), piece, gid))
    go = order[two[order]]
    g_of_go = gid[go]
    firstg = np.r_[True, g_of_go[1:] != g_of_go[:-1]]
    rep = go[firstg]
    oth = go[~firstg]
    rep_bad = badp[pair_of[rep]]
    oth_bad = badp[pair_of[oth]]
    swap = rep_bad & ~oth_bad
    pk = ~rep_bad & ~oth_bad
    rep_p, oth_p = rep[pk], oth[pk]
    grp_piece = piece[rep_p]
    narrow_g = grp_piece >= 3

    # pairs touching narrow-pass groups ranked first
    narrow_pairs = np.unique(np.r_[pair_of[rep_p[narrow_g]],
                                   pair_of[oth_p[narrow_g]]])
    sev = np.zeros(P, np.int64)
    sev[narrow_pairs] = 1
    order_p = np.lexsort((np.arange(P), -sev, r))
    r_sp = r[order_p]
    firstp = np.r_[True, r_sp[1:] != r_sp[:-1]]
    gidp = np.cumsum(firstp) - 1
    rank_sorted = np.arange(P) - np.flatnonzero(firstp)[gidp]
    srank = np.empty(P, np.int64)
    srank[order_p] = rank_sorted

    n_per_row = np.bincount(r, minlength=B)
    # per-band (tile) widths
    nmat = n_per_row.reshape(NCORES, NT, 128)
    SPt = [max(int(nmat[:, tt, :].max()), 1) for tt in range(NT)]
    NEt = [_even(SPt[tt] * J) for tt in range(NT)]
    assert max(NEt) * 32 < 2**16
    SPOFF = np.r_[0, np.cumsum(SPt)].astype(int)
    NEOFF = np.r_[0, np.cumsum(NEt)].astype(int)

    eslot = srank[pair_of] * J + np.tile(np.arange(J), P)

    if len(narrow_pairs):
        cnt_n = np.bincount(r[narrow_pairs], minlength=B)
    else:
        cnt_n = np.zeros(B, np.int64)
    cn = cnt_n.reshape(NCORES, NT, 128)
    WNt = [_even(min(int(cn[:, tt, :].max()) * J + 2, NEt[tt]))
           if cn[:, tt, :].max() > 0 else 0 for tt in range(NT)]
    WNOFF = np.r_[0, np.cumsum(WNt)].astype(int)

    # ---- main scatter plane: col -> slot of occ0 entries
    is_rep = np.ones(P * J, bool)
    is_rep[oth] = False
    is_rep[oth[swap]] = True
    m0 = is_rep & ~badp[pair_of]
    plane = np.full((NCORES, NT, 128, B), -1, np.int16)
    plane[core[m0], t[m0], pp[m0], ec_rot[m0]] = eslot[m0].astype(np.int16)
    plane_lv = [np.ascontiguousarray(
        plane[:, :, :, POFF[pc]:POFF[pc] + PW[pc]].transpose(0, 2, 1, 3))
        for pc in range(NP)]

    # ---- dup passes, packed per-tile widths
    e_rep, e_oth = eslot[rep_p], eslot[oth_p]

    def mk_pass(mask, widths, woff, check=True):
        tot = int(woff[-1])
        if tot == 0 or not mask.any():
            return None
        pl = np.full((NCORES, 128, tot), -1, np.int16)
        src = e_rep[mask]
        tgt = e_oth[mask]
        tts = t[rep_p[mask]]
        offs = np.asarray(woff)[tts]
        ww = np.asarray(widths)[tts]
        ok = (src < ww) & (tgt < ww)
        if check:
            assert ok.all(), "pass slot overflow"
        pl[core[rep_p[mask]][ok], pp[rep_p[mask]][ok],
           (offs + src)[ok]] = tgt[ok].astype(np.int16)
        return pl

    main_g = grp_piece <= 2
    pass_main = mk_pass(main_g, NEt, NEOFF)
    pass_p3 = mk_pass(grp_piece == 3, WNt, WNOFF)
    pass_p4 = mk_pass(grp_piece == 4, WNt, WNOFF)

    pairmask = np.zeros((NCORES, 128, int(SPOFF[-1])), F16)
    for tt in range(NT):
        pm_t = (np.arange(SPt[tt])[None, None, :] <
                nmat[:, tt, :][:, :, None]).astype(F16)
        pairmask[:, :, SPOFF[tt]:SPOFF[tt + 1]] = pm_t
    bp = bad_pairs
    tb = (r[bp] % RPC) // 128
    pairmask[r[bp] // RPC, r[bp] % 128,
             np.asarray(SPOFF)[tb] + srank[bp]] = 0

    return dict(plane_lv=plane_lv, pass_main=pass_main, pass_p3=pass_p3,
                pass_p4=pass_p4, pairmask=pairmask, order_rows=order_rows,
                SPt=SPt, NEt=NEt, WNt=WNt, SPOFF=SPOFF, NEOFF=NEOFF,
                WNOFF=WNOFF, bad_pairs=bad_pairs)


# ------------------------------------------------------------- device kernel
@with_exitstack
def _build(ctx: ExitStack, tc: "tile.TileContext", io: dict, SPt, NEt, WNt,
           SPOFF, NEOFF, WNOFF, have_main, have_p3, have_p4):
    nc = tc.nc
    y_d = io["y8"]
    SPSUM, NESUM, WNSUM = int(SPOFF[-1]), int(NEOFF[-1]), int(WNOFF[-1])

    consts = ctx.enter_context(tc.tile_pool(name="consts", bufs=1))
    wz = consts.tile([128, 2, 128], F8, tag="wz")
    nc.vector.memset(wz[:], 0.0)

    ypool = ctx.enter_context(tc.tile_pool(name="y", bufs=1))
    y = ypool.tile([128, NCH, 2 * NKP, CW], F8, tag="y", name="y")

    gpool = ctx.enter_context(tc.tile_pool(name="gbf", bufs=1))
    gbf = {tt: gpool.tile([128, B], DF16, tag=f"gbf{tt}", name=f"gbf{tt}")
           for tt in range(NT)}
    plpool = ctx.enter_context(tc.tile_pool(name="plane", bufs=1))
    pl = {pc: plpool.tile([128, NT, PW[pc]], I16, tag=f"plv{pc}",
                          name=f"plv{pc}") for pc in range(NP)}
    papool = ctx.enter_context(tc.tile_pool(name="passes", bufs=1))
    pam = papool.tile([128, NESUM], I16, tag="pam", name="pam") \
        if have_main else None
    pa3 = papool.tile([128, WNSUM], I16, tag="pa3", name="pa3") \
        if have_p3 else None
    pa4 = papool.tile([128, WNSUM], I16, tag="pa4", name="pa4") \
        if have_p4 else None

    lpool = ctx.enter_context(tc.tile_pool(name="loss", bufs=1))
    denall = lpool.tile([128, SPSUM], F32, tag="denall")
    l0all = lpool.tile([128, SPSUM], DF16, tag="l0all")

    # ---- DMA: chunk 7 dead last, planes just-in-time
    nc.sync.dma_start(y[:, 0], y_d[0])
    nc.sync.dma_start(y[:, 1], y_d[1])
    nc.sync.dma_start(pl[0][:], io["plane0"][:])
    nc.sync.dma_start(y[:, 2], y_d[2])
    nc.sync.dma_start(y[:, 3], y_d[3])
    nc.sync.dma_start(pl[1][:], io["plane1"][:])
    nc.sync.dma_start(y[:, 4], y_d[4])
    nc.sync.dma_start(y[:, 5], y_d[5])
    nc.sync.dma_start(pl[2][:], io["plane2"][:])
    nc.sync.dma_start(y[:, 6], y_d[6])
    nc.sync.dma_start(pl[3][:], io["plane3"][:])
    nc.sync.dma_start(y[:, 7], y_d[7])
    nc.sync.dma_start(pl[4][:], io["plane4"][:])
    if have_main:
        nc.sync.dma_start(pam[:], io["passm"][:])
    if have_p3:
        nc.sync.dma_start(pa3[:], io["pass3"][:])
    if have_p4:
        nc.sync.dma_start(pa4[:], io["pass4"][:])

    dpool = ctx.enter_context(tc.tile_pool(name="dq", bufs=2))
    hpool = ctx.enter_context(tc.tile_pool(name="hacc", bufs=1))
    expool = ctx.enter_context(tc.tile_pool(name="extra", bufs=2))
    elpool = ctx.enter_context(tc.tile_pool(name="elb", bufs=2))
    hacc = {tt: hpool.tile([128, NEt[tt]], DF16, tag=f"hacc{tt}",
                           name=f"hacc{tt}") for tt in range(NT)}

    # preload the Exp activation table during the initial DMA idle
    pre = elpool.tile([128, 1], F32, tag="pre")
    nc.scalar.activation(pre[:], wz[:, 0, 0:1], AF.Exp)

    dq = {}

    def scatter_piece(pc, tiles=range(NT)):
        for tt in tiles:
            d = dpool.tile([128, NEt[tt]], DF16, tag=f"d{tt}",
                           name=f"d{tt}_{pc}")
            dq[(tt, pc)] = d
            nc.gpsimd.local_scatter(
                d[:], gbf[tt][:, POFF[pc]:POFF[pc] + PW[pc]],
                pl[pc][:, tt, :], 128, NEt[tt], PW[pc])

    with tc.tile_pool(name="gpsum", bufs=1, space="PSUM") as gpsum:
        wps0 = gpsum.tile([128, CW], F32, tag="ps0_1", name="warm_init")
        for i in range(WARM0):
            nc.tensor.matmul(wps0[:, 0:64], lhsT=wz[:], rhs=wz[:, :, 0:64],
                             start=True, stop=True, perf_mode=DR)
        for c in range(NCH):
            for tt in range(NT):
                ps = gpsum.tile([128, CW], F32, tag=f"ps{tt}_{c % 2}",
                                name=f"ps{tt}_{c}")
                for kp in range(NKP):
                    nc.tensor.matmul(
                        ps[:],
                        lhsT=y[:, 0, 2 * kp:2 * kp + 2, tt * 128:(tt + 1) * 128],
                        rhs=y[:, c, 2 * kp:2 * kp + 2, :],
                        start=(kp == 0), stop=(kp == NKP - 1),
                        perf_mode=DR,
                    )
                dst = gbf[tt][:, c * CW:(c + 1) * CW]
                if (c * NT + tt) % 2 == 0:
                    nc.vector.tensor_copy(dst, ps[:])
                else:
                    nc.scalar.copy(dst, ps[:])
            if WARMS[c]:
                wps = gpsum.tile([128, CW], F32, tag=f"ps0_{(c + 1) % 2}",
                                 name=f"warm{c}")
                for i in range(WARMS[c]):
                    nc.tensor.matmul(wps[:, 0:64], lhsT=wz[:],
                                     rhs=wz[:, :, 0:64],
                                     start=True, stop=True, perf_mode=DR)
            if c == 1:
                scatter_piece(0)
            elif c == 3:
                scatter_piece(1)
            elif c == 5:
                scatter_piece(2)
                for tt in range(NT):
                    nc.vector.tensor_tensor(hacc[tt][:], dq[(tt, 0)][:],
                                            dq[(tt, 1)][:], ALU.add)
            elif c == 6:
                scatter_piece(3)

        # ---- tail: piece-4 scatters, then per-tile passes + exp/den
        scatter_piece(4)
        for tt in range(NT):
            nc.vector.tensor_tensor(hacc[tt][:], hacc[tt][:],
                                    dq[(tt, 2)][:], ALU.add)
        for tt in range(NT):
            NEc, WNc = NEt[tt], WNt[tt]
            if have_main:
                e = expool.tile([128, NEc], DF16, tag=f"eAm{tt % 2}",
                                name=f"eAm{tt}")
                nc.gpsimd.local_scatter(
                    e[:], hacc[tt][:], pam[:, NEOFF[tt]:NEOFF[tt + 1]],
                    128, NEc, NEc)
                nc.vector.tensor_tensor(hacc[tt][:], hacc[tt][:],
                                        e[:], ALU.add)
            if have_p3 and WNc:
                e = expool.tile([128, WNc], DF16, tag=f"eA3{tt % 2}",
                                name=f"eA3{tt}")
                nc.gpsimd.local_scatter(
                    e[:], dq[(tt, 3)][:, 0:WNc],
                    pa3[:, WNOFF[tt]:WNOFF[tt + 1]], 128, WNc, WNc)
                nc.vector.tensor_tensor(hacc[tt][:, 0:WNc],
                                        hacc[tt][:, 0:WNc], e[:], ALU.add)
            if have_p4 and WNc:
                e = expool.tile([128, WNc], DF16, tag=f"eA4{tt % 2}",
                                name=f"eA4{tt}")
                nc.gpsimd.local_scatter(
                    e[:], dq[(tt, 4)][:, 0:WNc],
                    pa4[:, WNOFF[tt]:WNOFF[tt + 1]], 128, WNc, WNc)
                nc.vector.tensor_tensor(hacc[tt][:, 0:WNc],
                                        hacc[tt][:, 0:WNc], e[:], ALU.add)
            u = expool.tile([128, NEc], DF16, tag=f"u{tt % 2}", name=f"u{tt}")
            nc.vector.tensor_tensor(u[:], dq[(tt, 3)][:], dq[(tt, 4)][:],
                                    ALU.add)
            nc.vector.tensor_tensor(hacc[tt][:], hacc[tt][:], u[:], ALU.add)
            ebuf = elpool.tile([128, NEt[0]], F32, tag="ebuf")
            nc.scalar.activation(ebuf[:, 0:NEc], hacc[tt][:], AF.Exp,
                                 scale=KSC)
            e3 = ebuf[:, 0:SPt[tt] * J].rearrange("p (s j) -> p s j", j=J)
            nc.vector.tensor_reduce(denall[:, SPOFF[tt]:SPOFF[tt + 1]], e3,
                                    AX.X, ALU.add)
            l0 = hacc[tt][:, 0:SPt[tt] * J].rearrange(
                "p (s j) -> p s j", j=J)[:, :, 0]
            nc.scalar.copy(l0all[:, SPOFF[tt]:SPOFF[tt + 1]], l0)

    # ---- ship den + l0; ln/mask/sum finish on host
    nc.sync.dma_start(io["den"][:], denall[:])
    nc.sync.dma_start(io["l0"][:], l0all[:])


def build_nc(SPt, NEt, WNt, SPOFF, NEOFF, WNOFF, have_main, have_p3, have_p4,
             enable_asserts=False):
    nc = bacc.Bacc("TRN2", target_bir_lowering=False, debug=False,
                   enable_asserts=enable_asserts, num_devices=NCORES)
    SPSUM, NESUM, WNSUM = int(SPOFF[-1]), int(NEOFF[-1]), int(WNOFF[-1])
    io = {
        "y8": nc.dram_tensor("y8", [NCH, 128, 2 * NKP, CW], F8,
                             kind="ExternalInput").ap(),
        "den": nc.dram_tensor("den", [128, SPSUM], F32,
                              kind="ExternalOutput").ap(),
        "l0": nc.dram_tensor("l0", [128, SPSUM], DF16,
                             kind="ExternalOutput").ap(),
    }
    for pc in range(NP):
        io[f"plane{pc}"] = nc.dram_tensor(
            f"plane{pc}", [128, NT, PW[pc]], I16, kind="ExternalInput").ap()
    if have_main:
        io["passm"] = nc.dram_tensor("passm", [128, NESUM], I16,
                                     kind="ExternalInput").ap()
    if have_p3:
        io["pass3"] = nc.dram_tensor("pass3", [128, WNSUM], I16,
                                     kind="ExternalInput").ap()
    if have_p4:
        io["pass4"] = nc.dram_tensor("pass4", [128, WNSUM], I16,
                                     kind="ExternalInput").ap()
    with tile.TileContext(nc) as tc:
        _build(tc, io, SPt, NEt, WNt, SPOFF, NEOFF, WNOFF,
               have_main, have_p3, have_p4)
    nc.compile()
    return nc


def _normalize(x):
    x = np.asarray(x, np.float32)
    w = np.sqrt((x.astype(np.float64) ** 2).sum(axis=1, keepdims=True))
    w = np.maximum(w, 1e-8)
    return (x / w).astype(np.float32)


def make_in_maps(x, plan):
    xn = _normalize(x)
    x8 = np.clip(xn * S, -240.0, 240.0).astype(FP8)
    # virtual-row layout: band-rank i -> virtual row v (see build_plan)
    ranks = np.arange(B)
    t_of = ranks // (128 * NCORES)
    i_in = ranks % (128 * NCORES)
    k_of = i_in % NCORES
    p_of = i_in // NCORES
    v_of = k_of * RPC + t_of * 128 + p_of
    xvirt = np.empty_like(x8)
    xvirt[v_of] = x8[plan["order_rows"]]
    in_maps = []
    for k in range(NCORES):
        xr = np.roll(xvirt, -RPC * k, axis=0)                  # [B, D]
        y8 = xr.T.reshape(2 * NKP, 128, B).transpose(1, 0, 2)  # [128, 16, B]
        y8c = np.ascontiguousarray(
            y8.reshape(128, 2 * NKP, NCH, CW).transpose(2, 0, 1, 3))
        m = {"y8": y8c}
        for pc in range(NP):
            m[f"plane{pc}"] = plan["plane_lv"][pc][k]
        if plan["pass_main"] is not None:
            m["passm"] = plan["pass_main"][k]
        if plan["pass_p3"] is not None:
            m["pass3"] = plan["pass_p3"][k]
        if plan["pass_p4"] is not None:
            m["pass4"] = plan["pass_p4"][k]
        in_maps.append(m)
    return in_maps


def host_fixup(x, anchor_idx, pos_idx, neg_idx, bad_pairs):
    if len(bad_pairs) == 0:
        return 0.0
    xn = _normalize(x).astype(np.float64)
    a = anchor_idx[bad_pairs]
    cols = np.concatenate([pos_idx[bad_pairs][:, None], neg_idx[bad_pairs]],
                          axis=1)
    logits = np.einsum("pd,pjd->pj", xn[a], xn[cols]) / TEMP
    mx = logits.max(axis=1, keepdims=True)
    lse = np.log(np.exp(logits - mx).sum(axis=1)) + mx[:, 0]
    return float((lse - logits[:, 0]).sum())


def kernel(**inputs):
    x = np.asarray(inputs["x"], np.float32)
    anchor_idx = np.asarray(inputs["anchor_idx"])
    pos_idx = np.asarray(inputs["pos_idx"])
    neg_idx = np.asarray(inputs["neg_idx"])
    P = anchor_idx.shape[0]

    plan = build_plan(anchor_idx, pos_idx, neg_idx)
    nc = build_nc(plan["SPt"], plan["NEt"], plan["WNt"], plan["SPOFF"],
                  plan["NEOFF"], plan["WNOFF"],
                  plan["pass_main"] is not None,
                  plan["pass_p3"] is not None,
                  plan["pass_p4"] is not None)
    in_maps = make_in_maps(x, plan)
    res = run_bass_kernel_spmd(nc, in_maps, list(range(NCORES)))
    total = 0.0
    for k in range(NCORES):
        den = np.asarray(res.results[k]["den"], np.float64)   # [128, SPSUM]
        l0 = np.asarray(res.results[k]["l0"], np.float64)
        pm = np.asarray(plan["pairmask"][k], np.float64)
        total += float((pm * (np.log(den) - KSC * l0)).sum())
    total += host_fixup(x, anchor_idx, pos_idx, neg_idx, plan["bad_pairs"])
    return np.float32(total / P)


# revision 27
# speedup vs baseline: 1.2464x; 1.0154x over previous
"""Trainium2 Bass kernel for ContrastiveNet loss (v11).

Per core k of 8 (SPMD):
  - host: xn = x/||x||*S (S=32) in fp8e4 -> sim = G/(S^2*TEMP); no on-device
    normalization. Rows are PERMUTED: sorted by pair count and banded into
    tiles (tile 0 heaviest), dealt round-robin across cores, so the per-tile
    slot space NE_t shrinks (~694/520/350/180 instead of 4x694) and the last
    tile's tail is the lightest. Core k's 512 rows sit at rotated columns
    0..511 of its gram; y is COLUMN-CHUNK-major ([8][128][16][512]) so each
    chunk's gram (4 row-tiles x 8 kp fp8 DoubleRow matmuls into [128,512]
    PSUM) starts as the chunk lands; chunk 7 is dead last in the DMA stream.
  - gather: per (tile, piece 1024/1024/1024/512/512) gpsimd local_scatter
    with HBM col->slot planes (int16, -1 unused), accumulated per tile.
    2-member duplicate (row,col) groups are fixed by scatter passes:
    main pass (rep col in pieces 0-2, width NE_t) and narrow piece-3/4
    passes (their pairs ranked into the lowest slots). Pairs touching
    >=3-member groups (~2%) are masked out and computed exactly on host.
  - device ships den (softmax denominators) + l0 per pair; the final
    ln/mask/sum runs on host along with the masked pairs.
  - PE p-state: tiny warm matmuls bridge DMA-paced idle gaps.
"""
import os
import sys
import numpy as np
import ml_dtypes

try:
    import concourse  # noqa: F401
except ImportError:
    sys.path.insert(0, "/opt/trn_rl_repo")

from contextlib import ExitStack

import concourse.bass as bass
import concourse.tile as tile
from concourse import bacc, mybir
from concourse._compat import with_exitstack
from concourse.bass_utils import run_bass_kernel_spmd

F16 = np.float16
FP8 = ml_dtypes.float8_e4m3
F32 = mybir.dt.float32
DF16 = mybir.dt.float16
F8 = mybir.dt.float8e4
I16 = mybir.dt.int16

B, D, J = 4096, 2048, 11
NCORES, RPC, NT, NKP = 8, 512, 4, 8
NCH, CW = 8, 512                    # gram column chunks (per core)
POFF = [0, 1024, 2048, 3072, 3584]  # scatter piece offsets
PW = [1024, 1024, 1024, 512, 512]   # scatter piece widths
NP = 5
TEMP = 0.1
S = 32.0
KSC = 1.0 / (S * S * TEMP)
WARMS = [0, 51, 0, 85, 0, 0, 35, 0]
WARM0 = 130
AF = mybir.ActivationFunctionType
ALU = mybir.AluOpType
AX = mybir.AxisListType
DR = mybir.MatmulPerfMode.DoubleRow


def _even(n):
    return int(n) + (int(n) % 2)


# ---------------------------------------------------------------- host prep
def build_plan(anchor_idx, pos_idx, neg_idx):
    r0 = anchor_idx.astype(np.int64)
    cols0 = np.concatenate([pos_idx[:, None], neg_idx], axis=1).astype(np.int64)
    P = r0.shape[0]

    # ---- row permutation: band rows by pair count (heaviest -> tile 0),
    # deal each band round-robin across cores
    n0 = np.bincount(r0, minlength=B)
    order_rows = np.argsort(-n0, kind="stable")
    virt = np.empty(B, np.int64)      # original row -> virtual row
    ranks = np.arange(B)
    t_of = ranks // (128 * NCORES)
    i_in = ranks % (128 * NCORES)
    k_of = i_in % NCORES
    p_of = i_in // NCORES
    virt[order_rows] = k_of * RPC + t_of * 128 + p_of

    r = virt[r0]
    cols = virt[cols0]

    er = np.repeat(r, J)
    ec = cols.ravel()
    pair_of = np.repeat(np.arange(P), J)
    core = er // RPC
    t = (er % RPC) // 128
    pp = er % 128
    ec_rot = (ec - core * RPC) % B
    piece = np.searchsorted(POFF, ec_rot, side="right") - 1

    key = er * B + ec
    o2 = np.argsort(key, kind="stable")
    k_sorted = key[o2]
    first2 = np.r_[True, k_sorted[1:] != k_sorted[:-1]]
    gid_sorted = np.cumsum(first2) - 1
    NG = int(gid_sorted[-1]) + 1
    gid = np.empty(P * J, np.int64)
    gid[o2] = gid_sorted
    occ_sorted = np.arange(P * J) - np.flatnonzero(first2)[gid_sorted]
    gsz_g = np.bincount(gid_sorted, minlength=NG)
    gsz = gsz_g[gid]

    bad_pairs = np.unique(pair_of[gsz >= 3])
    badp = np.zeros(P, bool)
    badp[bad_pairs] = True

    # 2-member groups: rep = member in the earliest piece
    two = gsz == 2
    order = np.lexsort((np.arange(P * J), piece, gid))
    go = order[two[order]]
    g_of_go = gid[go]
    firstg = np.r_[True, g_of_go[1:] != g_of_go[:-1]]
    rep = go[firstg]
    oth = go[~firstg]
    rep_bad = badp[pair_of[rep]]
    oth_bad = badp[pair_of[oth]]
    swap = rep_bad & ~oth_bad
    pk = ~rep_bad & ~oth_bad
    rep_p, oth_p = rep[pk], oth[pk]
    grp_piece = piece[rep_p]
    narrow_g = grp_piece >= 3

    # pairs touching narrow-pass groups ranked first
    narrow_pairs = np.unique(np.r_[pair_of[rep_p[narrow_g]],
                                   pair_of[oth_p[narrow_g]]])
    sev = np.zeros(P, np.int64)
    sev[narrow_pairs] = 1
    order_p = np.lexsort((np.arange(P), -sev, r))
    r_sp = r[order_p]
    firstp = np.r_[True, r_sp[1:] != r_sp[:-1]]
    gidp = np.cumsum(firstp) - 1
    rank_sorted = np.arange(P) - np.flatnonzero(firstp)[gidp]
    srank = np.empty(P, np.int64)
    srank[order_p] = rank_sorted

    n_per_row = np.bincount(r, minlength=B)
    # per-band (tile) widths
    nmat = n_per_row.reshape(NCORES, NT, 128)
    SPt = [max(int(nmat[:, tt, :].max()), 1) for tt in range(NT)]
    NEt = [_even(SPt[tt] * J) for tt in range(NT)]
    assert max(NEt) * 32 < 2**16
    SPOFF = np.r_[0, np.cumsum(SPt)].astype(int)
    NEOFF = np.r_[0, np.cumsum(NEt)].astype(int)

    eslot = srank[pair_of] * J + np.tile(np.arange(J), P)

    if len(narrow_pairs):
        cnt_n = np.bincount(r[narrow_pairs], minlength=B)
    else:
        cnt_n = np.zeros(B, np.int64)
    cn = cnt_n.reshape(NCORES, NT, 128)
    WNt = [_even(min(int(cn[:, tt, :].max()) * J + 2, NEt[tt]))
           if cn[:, tt, :].max() > 0 else 0 for tt in range(NT)]
    WNOFF = np.r_[0, np.cumsum(WNt)].astype(int)

    # ---- main scatter plane: col -> slot of occ0 entries
    is_rep = np.ones(P * J, bool)
    is_rep[oth] = False
    is_rep[oth[swap]] = True
    m0 = is_rep & ~badp[pair_of]
    plane = np.full((NCORES, NT, 128, B), -1, np.int16)
    plane[core[m0], t[m0], pp[m0], ec_rot[m0]] = eslot[m0].astype(np.int16)
    plane_lv = [np.ascontiguousarray(
        plane[:, :, :, POFF[pc]:POFF[pc] + PW[pc]].transpose(0, 2, 1, 3))
        for pc in range(NP)]

    # ---- dup passes, packed per-tile widths
    e_rep, e_oth = eslot[rep_p], eslot[oth_p]

    def mk_pass(mask, widths, woff, check=True):
        tot = int(woff[-1])
        if tot == 0 or not mask.any():
            return None
        pl = np.full((NCORES, 128, tot), -1, np.int16)
        src = e_rep[mask]
        tgt = e_oth[mask]
        tts = t[rep_p[mask]]
        offs = np.asarray(woff)[tts]
        ww = np.asarray(widths)[tts]
        ok = (src < ww) & (tgt < ww)
        if check:
            assert ok.all(), "pass slot overflow"
        pl[core[rep_p[mask]][ok], pp[rep_p[mask]][ok],
           (offs + src)[ok]] = tgt[ok].astype(np.int16)
        return pl

    main_g = grp_piece <= 2
    pass_main = mk_pass(main_g, NEt, NEOFF)
    pass_p3 = mk_pass(grp_piece == 3, WNt, WNOFF)
    pass_p4 = mk_pass(grp_piece == 4, WNt, WNOFF)

    pairmask = np.zeros((NCORES, 128, int(SPOFF[-1])), F16)
    for tt in range(NT):
        pm_t = (np.arange(SPt[tt])[None, None, :] <
                nmat[:, tt, :][:, :, None]).astype(F16)
        pairmask[:, :, SPOFF[tt]:SPOFF[tt + 1]] = pm_t
    bp = bad_pairs
    tb = (r[bp] % RPC) // 128
    pairmask[r[bp] // RPC, r[bp] % 128,
             np.asarray(SPOFF)[tb] + srank[bp]] = 0

    return dict(plane_lv=plane_lv, pass_main=pass_main, pass_p3=pass_p3,
                pass_p4=pass_p4, pairmask=pairmask, order_rows=order_rows,
                SPt=SPt, NEt=NEt, WNt=WNt, SPOFF=SPOFF, NEOFF=NEOFF,
                WNOFF=WNOFF, bad_pairs=bad_pairs)


# ------------------------------------------------------------- device kernel
@with_exitstack
def _build(ctx: ExitStack, tc: "tile.TileContext", io: dict, SPt, NEt, WNt,
           SPOFF, NEOFF, WNOFF, have_main, have_p3, have_p4):
    nc = tc.nc
    y_d = io["y8"]
    SPSUM, NESUM, WNSUM = int(SPOFF[-1]), int(NEOFF[-1]), int(WNOFF[-1])

    consts = ctx.enter_context(tc.tile_pool(name="consts", bufs=1))
    wz = consts.tile([128, 2, 128], F8, tag="wz")
    nc.vector.memset(wz[:], 0.0)

    ypool = ctx.enter_context(tc.tile_pool(name="y", bufs=1))
    y = ypool.tile([128, NCH, 2 * NKP, CW], F8, tag="y", name="y")

    gpool = ctx.enter_context(tc.tile_pool(name="gbf", bufs=1))
    gbf = {tt: gpool.tile([128, B], DF16, tag=f"gbf{tt}", name=f"gbf{tt}")
           for tt in range(NT)}
    plpool = ctx.enter_context(tc.tile_pool(name="plane", bufs=1))
    pl = {pc: plpool.tile([128, NT, PW[pc]], I16, tag=f"plv{pc}",
                          name=f"plv{pc}") for pc in range(NP)}
    papool = ctx.enter_context(tc.tile_pool(name="passes", bufs=1))
    pam = papool.tile([128, NESUM], I16, tag="pam", name="pam") \
        if have_main else None
    pa3 = papool.tile([128, WNSUM], I16, tag="pa3", name="pa3") \
        if have_p3 else None
    pa4 = papool.tile([128, WNSUM], I16, tag="pa4", name="pa4") \
        if have_p4 else None

    lpool = ctx.enter_context(tc.tile_pool(name="loss", bufs=1))
    denall = lpool.tile([128, SPSUM], F32, tag="denall")
    l0all = lpool.tile([128, SPSUM], DF16, tag="l0all")

    # ---- DMA: chunk 7 dead last, planes just-in-time
    nc.sync.dma_start(y[:, 0], y_d[0])
    nc.sync.dma_start(y[:, 1], y_d[1])
    nc.sync.dma_start(pl[0][:], io["plane0"][:])
    nc.sync.dma_start(y[:, 2], y_d[2])
    nc.sync.dma_start(y[:, 3], y_d[3])
    nc.sync.dma_start(pl[1][:], io["plane1"][:])
    nc.sync.dma_start(pl[2][:, :, 0:CW], io["plane2"][:, :, 0:CW])
    nc.sync.dma_start(y[:, 4], y_d[4])
    nc.sync.dma_start(y[:, 5], y_d[5])
    nc.sync.dma_start(y[:, 6], y_d[6])
    nc.sync.dma_start(pl[2][:, :, CW:2 * CW], io["plane2"][:, :, CW:2 * CW])
    nc.sync.dma_start(pl[3][:], io["plane3"][:])
    nc.sync.dma_start(y[:, 7], y_d[7])
    nc.sync.dma_start(pl[4][:], io["plane4"][:])
    if have_main:
        nc.sync.dma_start(pam[:], io["passm"][:])
    if have_p3:
        nc.sync.dma_start(pa3[:], io["pass3"][:])
    if have_p4:
        nc.sync.dma_start(pa4[:], io["pass4"][:])

    dpool = ctx.enter_context(tc.tile_pool(name="dq", bufs=2))
    hpool = ctx.enter_context(tc.tile_pool(name="hacc", bufs=1))
    expool = ctx.enter_context(tc.tile_pool(name="extra", bufs=2))
    elpool = ctx.enter_context(tc.tile_pool(name="elb", bufs=2))
    hacc = {tt: hpool.tile([128, NEt[tt]], DF16, tag=f"hacc{tt}",
                           name=f"hacc{tt}") for tt in range(NT)}

    # preload the Exp activation table during the initial DMA idle
    pre = elpool.tile([128, 1], F32, tag="pre")
    nc.scalar.activation(pre[:], wz[:, 0, 0:1], AF.Exp)

    dq = {}

    def scatter_piece(pc, tiles=range(NT), off=0, w=None, key=None):
        w = PW[pc] if w is None else w
        key = pc if key is None else key
        for tt in tiles:
            d = dpool.tile([128, NEt[tt]], DF16, tag=f"d{tt}",
                           name=f"d{tt}_{key}")
            dq[(tt, key)] = d
            nc.gpsimd.local_scatter(
                d[:], gbf[tt][:, POFF[pc] + off:POFF[pc] + off + w],
                pl[pc][:, tt, off:off + w], 128, NEt[tt], w)

    with tc.tile_pool(name="gpsum", bufs=1, space="PSUM") as gpsum:
        wps0 = gpsum.tile([128, CW], F32, tag="ps0_1", name="warm_init")
        for i in range(WARM0):
            nc.tensor.matmul(wps0[:, 0:64], lhsT=wz[:], rhs=wz[:, :, 0:64],
                             start=True, stop=True, perf_mode=DR)
        for c in range(NCH):
            for tt in range(NT):
                ps = gpsum.tile([128, CW], F32, tag=f"ps{tt}_{c % 2}",
                                name=f"ps{tt}_{c}")
                for kp in range(NKP):
                    nc.tensor.matmul(
                        ps[:],
                        lhsT=y[:, 0, 2 * kp:2 * kp + 2, tt * 128:(tt + 1) * 128],
                        rhs=y[:, c, 2 * kp:2 * kp + 2, :],
                        start=(kp == 0), stop=(kp == NKP - 1),
                        perf_mode=DR,
                    )
                dst = gbf[tt][:, c * CW:(c + 1) * CW]
                if (c * NT + tt) % 2 == 0:
                    nc.vector.tensor_copy(dst, ps[:])
                else:
                    nc.scalar.copy(dst, ps[:])
            if WARMS[c]:
                wps = gpsum.tile([128, CW], F32, tag=f"ps0_{(c + 1) % 2}",
                                 name=f"warm{c}")
                for i in range(WARMS[c]):
                    nc.tensor.matmul(wps[:, 0:64], lhsT=wz[:],
                                     rhs=wz[:, :, 0:64],
                                     start=True, stop=True, perf_mode=DR)
            if c == 1:
                scatter_piece(0)
            elif c == 3:
                scatter_piece(1)
            elif c == 4:
                scatter_piece(2, off=0, w=CW, key="2a")
            elif c == 5:
                scatter_piece(2, off=CW, w=CW, key="2b")
                for tt in range(NT):
                    nc.vector.tensor_tensor(hacc[tt][:], dq[(tt, 0)][:],
                                            dq[(tt, 1)][:], ALU.add)
                for tt in range(NT):
                    nc.vector.tensor_tensor(hacc[tt][:], hacc[tt][:],
                                            dq[(tt, "2a")][:], ALU.add)
            elif c == 6:
                scatter_piece(3)
                for tt in range(NT):
                    nc.vector.tensor_tensor(hacc[tt][:], hacc[tt][:],
                                            dq[(tt, "2b")][:], ALU.add)

        # ---- tail: main dup pass (pre-chunk-7), piece-4 scatters, narrows
        if have_main:
            for tt in range(NT):
                NEc = NEt[tt]
                e = expool.tile([128, NEc], DF16, tag=f"eAm{tt % 2}",
                                name=f"eAm{tt}")
                nc.gpsimd.local_scatter(
                    e[:], hacc[tt][:], pam[:, NEOFF[tt]:NEOFF[tt + 1]],
                    128, NEc, NEc)
                nc.vector.tensor_tensor(hacc[tt][:], hacc[tt][:],
                                        e[:], ALU.add)
        scatter_piece(4)
        for tt in range(NT):
            NEc, WNc = NEt[tt], WNt[tt]
            if have_p3 and WNc:
                e = expool.tile([128, WNc], DF16, tag=f"eA3{tt % 2}",
                                name=f"eA3{tt}")
                nc.gpsimd.local_scatter(
                    e[:], dq[(tt, 3)][:, 0:WNc],
                    pa3[:, WNOFF[tt]:WNOFF[tt + 1]], 128, WNc, WNc)
                nc.vector.tensor_tensor(hacc[tt][:, 0:WNc],
                                        hacc[tt][:, 0:WNc], e[:], ALU.add)
            if have_p4 and WNc:
                e = expool.tile([128, WNc], DF16, tag=f"eA4{tt % 2}",
                                name=f"eA4{tt}")
                nc.gpsimd.local_scatter(
                    e[:], dq[(tt, 4)][:, 0:WNc],
                    pa4[:, WNOFF[tt]:WNOFF[tt + 1]], 128, WNc, WNc)
                nc.vector.tensor_tensor(hacc[tt][:, 0:WNc],
                                        hacc[tt][:, 0:WNc], e[:], ALU.add)
            u = expool.tile([128, NEc], DF16, tag=f"u{tt % 2}", name=f"u{tt}")
            nc.vector.tensor_tensor(u[:], dq[(tt, 3)][:], dq[(tt, 4)][:],
                                    ALU.add)
            nc.vector.tensor_tensor(hacc[tt][:], hacc[tt][:], u[:], ALU.add)
            ebuf = elpool.tile([128, NEt[0]], F32, tag="ebuf")
            nc.scalar.activation(ebuf[:, 0:NEc], hacc[tt][:], AF.Exp,
                                 scale=KSC)
            e3 = ebuf[:, 0:SPt[tt] * J].rearrange("p (s j) -> p s j", j=J)
            nc.vector.tensor_reduce(denall[:, SPOFF[tt]:SPOFF[tt + 1]], e3,
                                    AX.X, ALU.add)
            l0 = hacc[tt][:, 0:SPt[tt] * J].rearrange(
                "p (s j) -> p s j", j=J)[:, :, 0]
            nc.scalar.copy(l0all[:, SPOFF[tt]:SPOFF[tt + 1]], l0)

    # ---- ship den + l0; ln/mask/sum finish on host
    nc.sync.dma_start(io["den"][:], denall[:])
    nc.sync.dma_start(io["l0"][:], l0all[:])


def build_nc(SPt, NEt, WNt, SPOFF, NEOFF, WNOFF, have_main, have_p3, have_p4,
             enable_asserts=False):
    nc = bacc.Bacc("TRN2", target_bir_lowering=False, debug=False,
                   enable_asserts=enable_asserts, num_devices=NCORES)
    SPSUM, NESUM, WNSUM = int(SPOFF[-1]), int(NEOFF[-1]), int(WNOFF[-1])
    io = {
        "y8": nc.dram_tensor("y8", [NCH, 128, 2 * NKP, CW], F8,
                             kind="ExternalInput").ap(),
        "den": nc.dram_tensor("den", [128, SPSUM], F32,
                              kind="ExternalOutput").ap(),
        "l0": nc.dram_tensor("l0", [128, SPSUM], DF16,
                             kind="ExternalOutput").ap(),
    }
    for pc in range(NP):
        io[f"plane{pc}"] = nc.dram_tensor(
            f"plane{pc}", [128, NT, PW[pc]], I16, kind="ExternalInput").ap()
    if have_main:
        io["passm"] = nc.dram_tensor("passm", [128, NESUM], I16,
                                     kind="ExternalInput").ap()
    if have_p3:
        io["pass3"] = nc.dram_tensor("pass3", [128, WNSUM], I16,
                                     kind="ExternalInput").ap()
    if have_p4:
        io["pass4"] = nc.dram_tensor("pass4", [128, WNSUM], I16,
                                     kind="ExternalInput").ap()
    with tile.TileContext(nc) as tc:
        _build(tc, io, SPt, NEt, WNt, SPOFF, NEOFF, WNOFF,
               have_main, have_p3, have_p4)
    nc.compile()
    return nc


def _normalize(x):
    x = np.asarray(x, np.float32)
    w = np.sqrt((x.astype(np.float64) ** 2).sum(axis=1, keepdims=True))
    w = np.maximum(w, 1e-8)
    return (x / w).astype(np.float32)


def make_in_maps(x, plan):
    xn = _normalize(x)
    x8 = np.clip(xn * S, -240.0, 240.0).astype(FP8)
    # virtual-row layout: band-rank i -> virtual row v (see build_plan)
    ranks = np.arange(B)
    t_of = ranks // (128 * NCORES)
    i_in = ranks % (128 * NCORES)
    k_of = i_in % NCORES
    p_of = i_in // NCORES
    v_of = k_of * RPC + t_of * 128 + p_of
    xvirt = np.empty_like(x8)
    xvirt[v_of] = x8[plan["order_rows"]]
    in_maps = []
    for k in range(NCORES):
        xr = np.roll(xvirt, -RPC * k, axis=0)                  # [B, D]
        y8 = xr.T.reshape(2 * NKP, 128, B).transpose(1, 0, 2)  # [128, 16, B]
        y8c = np.ascontiguousarray(
            y8.reshape(128, 2 * NKP, NCH, CW).transpose(2, 0, 1, 3))
        m = {"y8": y8c}
        for pc in range(NP):
            m[f"plane{pc}"] = plan["plane_lv"][pc][k]
        if plan["pass_main"] is not None:
            m["passm"] = plan["pass_main"][k]
        if plan["pass_p3"] is not None:
            m["pass3"] = plan["pass_p3"][k]
        if plan["pass_p4"] is not None:
            m["pass4"] = plan["pass_p4"][k]
        in_maps.append(m)
    return in_maps


def host_fixup(x, anchor_idx, pos_idx, neg_idx, bad_pairs):
    if len(bad_pairs) == 0:
        return 0.0
    xn = _normalize(x).astype(np.float64)
    a = anchor_idx[bad_pairs]
    cols = np.concatenate([pos_idx[bad_pairs][:, None], neg_idx[bad_pairs]],
                          axis=1)
    logits = np.einsum("pd,pjd->pj", xn[a], xn[cols]) / TEMP
    mx = logits.max(axis=1, keepdims=True)
    lse = np.log(np.exp(logits - mx).sum(axis=1)) + mx[:, 0]
    return float((lse - logits[:, 0]).sum())


def kernel(**inputs):
    x = np.asarray(inputs["x"], np.float32)
    anchor_idx = np.asarray(inputs["anchor_idx"])
    pos_idx = np.asarray(inputs["pos_idx"])
    neg_idx = np.asarray(inputs["neg_idx"])
    P = anchor_idx.shape[0]

    plan = build_plan(anchor_idx, pos_idx, neg_idx)
    nc = build_nc(plan["SPt"], plan["NEt"], plan["WNt"], plan["SPOFF"],
                  plan["NEOFF"], plan["WNOFF"],
                  plan["pass_main"] is not None,
                  plan["pass_p3"] is not None,
                  plan["pass_p4"] is not None)
    in_maps = make_in_maps(x, plan)
    res = run_bass_kernel_spmd(nc, in_maps, list(range(NCORES)))
    total = 0.0
    for k in range(NCORES):
        den = np.asarray(res.results[k]["den"], np.float64)   # [128, SPSUM]
        l0 = np.asarray(res.results[k]["l0"], np.float64)
        pm = np.asarray(plan["pairmask"][k], np.float64)
        total += float((pm * (np.log(den) - KSC * l0)).sum())
    total += host_fixup(x, anchor_idx, pos_idx, neg_idx, plan["bad_pairs"])
    return np.float32(total / P)


# revision 28
# speedup vs baseline: 1.2479x; 1.0012x over previous
"""Trainium2 Bass kernel for ContrastiveNet loss (v11).

Per core k of 8 (SPMD):
  - host: xn = x/||x||*S (S=32) in fp8e4 -> sim = G/(S^2*TEMP); no on-device
    normalization. Rows are PERMUTED: sorted by pair count and banded into
    tiles (tile 0 heaviest), dealt round-robin across cores, so the per-tile
    slot space NE_t shrinks (~694/520/350/180 instead of 4x694) and the last
    tile's tail is the lightest. Core k's 512 rows sit at rotated columns
    0..511 of its gram; y is COLUMN-CHUNK-major ([8][128][16][512]) so each
    chunk's gram (4 row-tiles x 8 kp fp8 DoubleRow matmuls into [128,512]
    PSUM) starts as the chunk lands; chunk 7 is dead last in the DMA stream.
  - gather: per (tile, piece 1024/1024/1024/512/512) gpsimd local_scatter
    with HBM col->slot planes (int16, -1 unused), accumulated per tile.
    2-member duplicate (row,col) groups are fixed by scatter passes:
    main pass (rep col in pieces 0-2, width NE_t) and narrow piece-3/4
    passes (their pairs ranked into the lowest slots). Pairs touching
    >=3-member groups (~2%) are masked out and computed exactly on host.
  - device ships den (softmax denominators) + l0 per pair; the final
    ln/mask/sum runs on host along with the masked pairs.
  - PE p-state: tiny warm matmuls bridge DMA-paced idle gaps.
"""
import os
import sys
import numpy as np
import ml_dtypes

try:
    import concourse  # noqa: F401
except ImportError:
    sys.path.insert(0, "/opt/trn_rl_repo")

from contextlib import ExitStack

import concourse.bass as bass
import concourse.tile as tile
from concourse import bacc, mybir
from concourse._compat import with_exitstack
from concourse.bass_utils import run_bass_kernel_spmd

F16 = np.float16
FP8 = ml_dtypes.float8_e4m3
F32 = mybir.dt.float32
DF16 = mybir.dt.float16
F8 = mybir.dt.float8e4
I16 = mybir.dt.int16

B, D, J = 4096, 2048, 11
NCORES, RPC, NT, NKP = 8, 512, 4, 8
NCH, CW = 8, 512                    # gram column chunks (per core)
POFF = [0, 1024, 2048, 3072, 3584]  # scatter piece offsets
PW = [1024, 1024, 1024, 512, 512]   # scatter piece widths
NP = 5
TEMP = 0.1
S = 32.0
KSC = 1.0 / (S * S * TEMP)
WARMS = [0, 51, 0, 85, 0, 0, 35, 0]
WARM0 = 130
AF = mybir.ActivationFunctionType
ALU = mybir.AluOpType
AX = mybir.AxisListType
DR = mybir.MatmulPerfMode.DoubleRow


def _even(n):
    return int(n) + (int(n) % 2)


# ---------------------------------------------------------------- host prep
def build_plan(anchor_idx, pos_idx, neg_idx):
    r0 = anchor_idx.astype(np.int64)
    cols0 = np.concatenate([pos_idx[:, None], neg_idx], axis=1).astype(np.int64)
    P = r0.shape[0]

    # ---- row permutation: band rows by pair count (heaviest -> tile 0),
    # deal each band round-robin across cores
    n0 = np.bincount(r0, minlength=B)
    order_rows = np.argsort(-n0, kind="stable")
    virt = np.empty(B, np.int64)      # original row -> virtual row
    ranks = np.arange(B)
    t_of = ranks // (128 * NCORES)
    i_in = ranks % (128 * NCORES)
    k_of = i_in % NCORES
    p_of = i_in // NCORES
    virt[order_rows] = k_of * RPC + t_of * 128 + p_of

    r = virt[r0]
    cols = virt[cols0]

    er = np.repeat(r, J)
    ec = cols.ravel()
    pair_of = np.repeat(np.arange(P), J)
    core = er // RPC
    t = (er % RPC) // 128
    pp = er % 128
    ec_rot = (ec - core * RPC) % B
    piece = np.searchsorted(POFF, ec_rot, side="right") - 1

    key = er * B + ec
    o2 = np.argsort(key, kind="stable")
    k_sorted = key[o2]
    first2 = np.r_[True, k_sorted[1:] != k_sorted[:-1]]
    gid_sorted = np.cumsum(first2) - 1
    NG = int(gid_sorted[-1]) + 1
    gid = np.empty(P * J, np.int64)
    gid[o2] = gid_sorted
    occ_sorted = np.arange(P * J) - np.flatnonzero(first2)[gid_sorted]
    gsz_g = np.bincount(gid_sorted, minlength=NG)
    gsz = gsz_g[gid]

    bad_pairs = np.unique(pair_of[gsz >= 3])
    badp = np.zeros(P, bool)
    badp[bad_pairs] = True

    # 2-member groups: rep = member in the earliest piece
    two = gsz == 2
    order = np.lexsort((np.arange(P * J), piece, gid))
    go = order[two[order]]
    g_of_go = gid[go]
    firstg = np.r_[True, g_of_go[1:] != g_of_go[:-1]]
    rep = go[firstg]
    oth = go[~firstg]
    rep_bad = badp[pair_of[rep]]
    oth_bad = badp[pair_of[oth]]
    swap = rep_bad & ~oth_bad
    pk = ~rep_bad & ~oth_bad
    rep_p, oth_p = rep[pk], oth[pk]
    grp_piece = piece[rep_p]
    narrow_g = grp_piece >= 3

    # pairs touching narrow-pass groups ranked first
    narrow_pairs = np.unique(np.r_[pair_of[rep_p[narrow_g]],
                                   pair_of[oth_p[narrow_g]]])
    sev = np.zeros(P, np.int64)
    sev[narrow_pairs] = 1
    order_p = np.lexsort((np.arange(P), -sev, r))
    r_sp = r[order_p]
    firstp = np.r_[True, r_sp[1:] != r_sp[:-1]]
    gidp = np.cumsum(firstp) - 1
    rank_sorted = np.arange(P) - np.flatnonzero(firstp)[gidp]
    srank = np.empty(P, np.int64)
    srank[order_p] = rank_sorted

    n_per_row = np.bincount(r, minlength=B)
    # per-band (tile) widths
    nmat = n_per_row.reshape(NCORES, NT, 128)
    SPt = [max(int(nmat[:, tt, :].max()), 1) for tt in range(NT)]
    NEt = [_even(SPt[tt] * J) for tt in range(NT)]
    assert max(NEt) * 32 < 2**16
    SPOFF = np.r_[0, np.cumsum(SPt)].astype(int)
    NEOFF = np.r_[0, np.cumsum(NEt)].astype(int)

    eslot = srank[pair_of] * J + np.tile(np.arange(J), P)

    if len(narrow_pairs):
        cnt_n = np.bincount(r[narrow_pairs], minlength=B)
    else:
        cnt_n = np.zeros(B, np.int64)
    cn = cnt_n.reshape(NCORES, NT, 128)
    WNt = [_even(min(int(cn[:, tt, :].max()) * J + 2, NEt[tt]))
           if cn[:, tt, :].max() > 0 else 0 for tt in range(NT)]
    WNOFF = np.r_[0, np.cumsum(WNt)].astype(int)

    # ---- main scatter plane: col -> slot of occ0 entries
    is_rep = np.ones(P * J, bool)
    is_rep[oth] = False
    is_rep[oth[swap]] = True
    m0 = is_rep & ~badp[pair_of]
    plane = np.full((NCORES, NT, 128, B), -1, np.int16)
    plane[core[m0], t[m0], pp[m0], ec_rot[m0]] = eslot[m0].astype(np.int16)
    plane_lv = [np.ascontiguousarray(
        plane[:, :, :, POFF[pc]:POFF[pc] + PW[pc]].transpose(0, 2, 1, 3))
        for pc in range(NP)]

    # ---- dup passes, packed per-tile widths
    e_rep, e_oth = eslot[rep_p], eslot[oth_p]

    def mk_pass(mask, widths, woff, check=True):
        tot = int(woff[-1])
        if tot == 0 or not mask.any():
            return None
        pl = np.full((NCORES, 128, tot), -1, np.int16)
        src = e_rep[mask]
        tgt = e_oth[mask]
        tts = t[rep_p[mask]]
        offs = np.asarray(woff)[tts]
        ww = np.asarray(widths)[tts]
        ok = (src < ww) & (tgt < ww)
        if check:
            assert ok.all(), "pass slot overflow"
        pl[core[rep_p[mask]][ok], pp[rep_p[mask]][ok],
           (offs + src)[ok]] = tgt[ok].astype(np.int16)
        return pl

    main_g = grp_piece <= 2
    pass_main = mk_pass(main_g, NEt, NEOFF)
    pass_p3 = mk_pass(grp_piece == 3, WNt, WNOFF)
    pass_p4 = mk_pass(grp_piece == 4, WNt, WNOFF)

    pairmask = np.zeros((NCORES, 128, int(SPOFF[-1])), F16)
    for tt in range(NT):
        pm_t = (np.arange(SPt[tt])[None, None, :] <
                nmat[:, tt, :][:, :, None]).astype(F16)
        pairmask[:, :, SPOFF[tt]:SPOFF[tt + 1]] = pm_t
    bp = bad_pairs
    tb = (r[bp] % RPC) // 128
    pairmask[r[bp] // RPC, r[bp] % 128,
             np.asarray(SPOFF)[tb] + srank[bp]] = 0

    return dict(plane_lv=plane_lv, pass_main=pass_main, pass_p3=pass_p3,
                pass_p4=pass_p4, pairmask=pairmask, order_rows=order_rows,
                SPt=SPt, NEt=NEt, WNt=WNt, SPOFF=SPOFF, NEOFF=NEOFF,
                WNOFF=WNOFF, bad_pairs=bad_pairs)


# ------------------------------------------------------------- device kernel
@with_exitstack
def _build(ctx: ExitStack, tc: "tile.TileContext", io: dict, SPt, NEt, WNt,
           SPOFF, NEOFF, WNOFF, have_main, have_p3, have_p4):
    nc = tc.nc
    y_d = io["y8"]
    SPSUM, NESUM, WNSUM = int(SPOFF[-1]), int(NEOFF[-1]), int(WNOFF[-1])

    consts = ctx.enter_context(tc.tile_pool(name="consts", bufs=1))
    wz = consts.tile([128, 2, 128], F8, tag="wz")
    nc.vector.memset(wz[:], 0.0)

    ypool = ctx.enter_context(tc.tile_pool(name="y", bufs=1))
    y = ypool.tile([128, NCH, 2 * NKP, CW], F8, tag="y", name="y")

    gpool = ctx.enter_context(tc.tile_pool(name="gbf", bufs=1))
    gbf = {tt: gpool.tile([128, B], DF16, tag=f"gbf{tt}", name=f"gbf{tt}")
           for tt in range(NT)}
    plpool = ctx.enter_context(tc.tile_pool(name="plane", bufs=1))
    pl = {pc: plpool.tile([128, NT, PW[pc]], I16, tag=f"plv{pc}",
                          name=f"plv{pc}") for pc in range(NP)}
    papool = ctx.enter_context(tc.tile_pool(name="passes", bufs=1))
    pam = papool.tile([128, NESUM], I16, tag="pam", name="pam") \
        if have_main else None
    pa3 = papool.tile([128, WNSUM], I16, tag="pa3", name="pa3") \
        if have_p3 else None
    pa4 = papool.tile([128, WNSUM], I16, tag="pa4", name="pa4") \
        if have_p4 else None

    lpool = ctx.enter_context(tc.tile_pool(name="loss", bufs=1))
    denall = lpool.tile([128, SPSUM], F32, tag="denall")
    l0all = lpool.tile([128, SPSUM], DF16, tag="l0all")

    # ---- DMA: chunk 7 dead last, planes just-in-time
    nc.sync.dma_start(y[:, 0], y_d[0])
    nc.sync.dma_start(y[:, 1], y_d[1])
    nc.sync.dma_start(pl[0][:], io["plane0"][:])
    nc.sync.dma_start(y[:, 2], y_d[2])
    nc.sync.dma_start(y[:, 3], y_d[3])
    nc.sync.dma_start(pl[1][:], io["plane1"][:])
    nc.sync.dma_start(pl[2][:, :, 0:CW], io["plane2"][:, :, 0:CW])
    nc.sync.dma_start(y[:, 4], y_d[4])
    nc.sync.dma_start(y[:, 5], y_d[5])
    nc.sync.dma_start(y[:, 6], y_d[6])
    nc.sync.dma_start(pl[2][:, :, CW:2 * CW], io["plane2"][:, :, CW:2 * CW])
    nc.sync.dma_start(pl[3][:], io["plane3"][:])
    nc.sync.dma_start(y[:, 7], y_d[7])
    nc.sync.dma_start(pl[4][:], io["plane4"][:])
    if have_main:
        nc.sync.dma_start(pam[:], io["passm"][:])
    if have_p3:
        nc.sync.dma_start(pa3[:], io["pass3"][:])
    if have_p4:
        nc.sync.dma_start(pa4[:], io["pass4"][:])

    dpool = ctx.enter_context(tc.tile_pool(name="dq", bufs=2))
    hpool = ctx.enter_context(tc.tile_pool(name="hacc", bufs=1))
    expool = ctx.enter_context(tc.tile_pool(name="extra", bufs=2))
    elpool = ctx.enter_context(tc.tile_pool(name="elb", bufs=2))
    hacc = {tt: hpool.tile([128, NEt[tt]], DF16, tag=f"hacc{tt}",
                           name=f"hacc{tt}") for tt in range(NT)}

    # preload the Exp activation table during the initial DMA idle
    pre = elpool.tile([128, 1], F32, tag="pre")
    nc.scalar.activation(pre[:], wz[:, 0, 0:1], AF.Exp)

    dq = {}

    def scatter_piece(pc, tiles=range(NT), off=0, w=None, key=None):
        w = PW[pc] if w is None else w
        key = pc if key is None else key
        for tt in tiles:
            d = dpool.tile([128, NEt[tt]], DF16, tag=f"d{tt}",
                           name=f"d{tt}_{key}")
            dq[(tt, key)] = d
            nc.gpsimd.local_scatter(
                d[:], gbf[tt][:, POFF[pc] + off:POFF[pc] + off + w],
                pl[pc][:, tt, off:off + w], 128, NEt[tt], w)

    with tc.tile_pool(name="gpsum", bufs=1, space="PSUM") as gpsum:
        wps0 = gpsum.tile([128, CW], F32, tag="ps0_1", name="warm_init")
        for i in range(WARM0):
            nc.tensor.matmul(wps0[:, 0:64], lhsT=wz[:], rhs=wz[:, :, 0:64],
                             start=True, stop=True, perf_mode=DR)
        for c in range(NCH):
            for tt in range(NT):
                ps = gpsum.tile([128, CW], F32, tag=f"ps{tt}_{c % 2}",
                                name=f"ps{tt}_{c}")
                for kp in range(NKP):
                    nc.tensor.matmul(
                        ps[:],
                        lhsT=y[:, 0, 2 * kp:2 * kp + 2, tt * 128:(tt + 1) * 128],
                        rhs=y[:, c, 2 * kp:2 * kp + 2, :],
                        start=(kp == 0), stop=(kp == NKP - 1),
                        perf_mode=DR,
                    )
                dst = gbf[tt][:, c * CW:(c + 1) * CW]
                if (c * NT + tt) % 2 == 0:
                    nc.vector.tensor_copy(dst, ps[:])
                else:
                    nc.scalar.copy(dst, ps[:])
            if WARMS[c]:
                wps = gpsum.tile([128, CW], F32, tag=f"ps0_{(c + 1) % 2}",
                                 name=f"warm{c}")
                for i in range(WARMS[c]):
                    nc.tensor.matmul(wps[:, 0:64], lhsT=wz[:],
                                     rhs=wz[:, :, 0:64],
                                     start=True, stop=True, perf_mode=DR)
            if c == 1:
                scatter_piece(0)
            elif c == 3:
                scatter_piece(1)
            elif c == 4:
                scatter_piece(2, off=0, w=CW, key="2a")
            elif c == 5:
                scatter_piece(2, off=CW, w=CW, key="2b")
                for tt in range(NT):
                    nc.vector.tensor_tensor(hacc[tt][:], dq[(tt, 0)][:],
                                            dq[(tt, 1)][:], ALU.add)
                for tt in range(NT):
                    nc.vector.tensor_tensor(hacc[tt][:], hacc[tt][:],
                                            dq[(tt, "2a")][:], ALU.add)
            elif c == 6:
                scatter_piece(3)
                for tt in range(NT):
                    nc.vector.tensor_tensor(hacc[tt][:], hacc[tt][:],
                                            dq[(tt, "2b")][:], ALU.add)

        # ---- tail: main dup pass (pre-chunk-7), piece-4 scatters, narrows
        if have_main:
            for tt in range(NT):
                NEc = NEt[tt]
                e = expool.tile([128, NEc], DF16, tag=f"eAm{tt % 2}",
                                name=f"eAm{tt}")
                nc.gpsimd.local_scatter(
                    e[:], hacc[tt][:], pam[:, NEOFF[tt]:NEOFF[tt + 1]],
                    128, NEc, NEc)
                nc.vector.tensor_tensor(hacc[tt][:], hacc[tt][:],
                                        e[:], ALU.add)
        scatter_piece(4)
        for tt in range(NT):
            NEc, WNc = NEt[tt], WNt[tt]
            if have_p3 and WNc:
                e = expool.tile([128, WNc], DF16, tag=f"eA3{tt % 2}",
                                name=f"eA3{tt}")
                nc.gpsimd.local_scatter(
                    e[:], dq[(tt, 3)][:, 0:WNc],
                    pa3[:, WNOFF[tt]:WNOFF[tt + 1]], 128, WNc, WNc)
                nc.vector.tensor_tensor(hacc[tt][:, 0:WNc],
                                        hacc[tt][:, 0:WNc], e[:], ALU.add)
            if have_p4 and WNc:
                e = expool.tile([128, WNc], DF16, tag=f"eA4{tt % 2}",
                                name=f"eA4{tt}")
                nc.gpsimd.local_scatter(
                    e[:], dq[(tt, 4)][:, 0:WNc],
                    pa4[:, WNOFF[tt]:WNOFF[tt + 1]], 128, WNc, WNc)
                nc.vector.tensor_tensor(hacc[tt][:, 0:WNc],
                                        hacc[tt][:, 0:WNc], e[:], ALU.add)
            u = expool.tile([128, NEc], DF16, tag=f"u{tt % 2}", name=f"u{tt}")
            nc.vector.tensor_tensor(u[:], dq[(tt, 3)][:], dq[(tt, 4)][:],
                                    ALU.add)
            nc.vector.tensor_tensor(hacc[tt][:], hacc[tt][:], u[:], ALU.add)
            ebuf = elpool.tile([128, NEt[0]], F32, tag="ebuf")
            nc.scalar.activation(ebuf[:, 0:NEc], hacc[tt][:], AF.Exp,
                                 scale=KSC)
            e3 = ebuf[:, 0:SPt[tt] * J].rearrange("p (s j) -> p s j", j=J)
            nc.vector.tensor_reduce(denall[:, SPOFF[tt]:SPOFF[tt + 1]], e3,
                                    AX.X, ALU.add)
            l0 = hacc[tt][:, 0:SPt[tt] * J].rearrange(
                "p (s j) -> p s j", j=J)[:, :, 0]
            nc.scalar.copy(l0all[:, SPOFF[tt]:SPOFF[tt + 1]], l0)

    # ---- ship den + l0; ln/mask/sum finish on host (separate queues so
    # descriptor generation overlaps; l0 completes before the last den)
    nc.scalar.dma_start(io["l0"][:], l0all[:])
    nc.sync.dma_start(io["den"][:], denall[:])


def build_nc(SPt, NEt, WNt, SPOFF, NEOFF, WNOFF, have_main, have_p3, have_p4,
             enable_asserts=False):
    nc = bacc.Bacc("TRN2", target_bir_lowering=False, debug=False,
                   enable_asserts=enable_asserts, num_devices=NCORES)
    SPSUM, NESUM, WNSUM = int(SPOFF[-1]), int(NEOFF[-1]), int(WNOFF[-1])
    io = {
        "y8": nc.dram_tensor("y8", [NCH, 128, 2 * NKP, CW], F8,
                             kind="ExternalInput").ap(),
        "den": nc.dram_tensor("den", [128, SPSUM], F32,
                              kind="ExternalOutput").ap(),
        "l0": nc.dram_tensor("l0", [128, SPSUM], DF16,
                             kind="ExternalOutput").ap(),
    }
    for pc in range(NP):
        io[f"plane{pc}"] = nc.dram_tensor(
            f"plane{pc}", [128, NT, PW[pc]], I16, kind="ExternalInput").ap()
    if have_main:
        io["passm"] = nc.dram_tensor("passm", [128, NESUM], I16,
                                     kind="ExternalInput").ap()
    if have_p3:
        io["pass3"] = nc.dram_tensor("pass3", [128, WNSUM], I16,
                                     kind="ExternalInput").ap()
    if have_p4:
        io["pass4"] = nc.dram_tensor("pass4", [128, WNSUM], I16,
                                     kind="ExternalInput").ap()
    with tile.TileContext(nc) as tc:
        _build(tc, io, SPt, NEt, WNt, SPOFF, NEOFF, WNOFF,
               have_main, have_p3, have_p4)
    nc.compile()
    return nc


def _normalize(x):
    x = np.asarray(x, np.float32)
    w = np.sqrt((x.astype(np.float64) ** 2).sum(axis=1, keepdims=True))
    w = np.maximum(w, 1e-8)
    return (x / w).astype(np.float32)


def make_in_maps(x, plan):
    xn = _normalize(x)
    x8 = np.clip(xn * S, -240.0, 240.0).astype(FP8)
    # virtual-row layout: band-rank i -> virtual row v (see build_plan)
    ranks = np.arange(B)
    t_of = ranks // (128 * NCORES)
    i_in = ranks % (128 * NCORES)
    k_of = i_in % NCORES
    p_of = i_in // NCORES
    v_of = k_of * RPC + t_of * 128 + p_of
    xvirt = np.empty_like(x8)
    xvirt[v_of] = x8[plan["order_rows"]]
    in_maps = []
    for k in range(NCORES):
        xr = np.roll(xvirt, -RPC * k, axis=0)                  # [B, D]
        y8 = xr.T.reshape(2 * NKP, 128, B).transpose(1, 0, 2)  # [128, 16, B]
        y8c = np.ascontiguousarray(
            y8.reshape(128, 2 * NKP, NCH, CW).transpose(2, 0, 1, 3))
        m = {"y8": y8c}
        for pc in range(NP):
            m[f"plane{pc}"] = plan["plane_lv"][pc][k]
        if plan["pass_main"] is not None:
            m["passm"] = plan["pass_main"][k]
        if plan["pass_p3"] is not None:
            m["pass3"] = plan["pass_p3"][k]
        if plan["pass_p4"] is not None:
            m["pass4"] = plan["pass_p4"][k]
        in_maps.append(m)
    return in_maps


def host_fixup(x, anchor_idx, pos_idx, neg_idx, bad_pairs):
    if len(bad_pairs) == 0:
        return 0.0
    xn = _normalize(x).astype(np.float64)
    a = anchor_idx[bad_pairs]
    cols = np.concatenate([pos_idx[bad_pairs][:, None], neg_idx[bad_pairs]],
                          axis=1)
    logits = np.einsum("pd,pjd->pj", xn[a], xn[cols]) / TEMP
    mx = logits.max(axis=1, keepdims=True)
    lse = np.log(np.exp(logits - mx).sum(axis=1)) + mx[:, 0]
    return float((lse - logits[:, 0]).sum())


def kernel(**inputs):
    x = np.asarray(inputs["x"], np.float32)
    anchor_idx = np.asarray(inputs["anchor_idx"])
    pos_idx = np.asarray(inputs["pos_idx"])
    neg_idx = np.asarray(inputs["neg_idx"])
    P = anchor_idx.shape[0]

    plan = build_plan(anchor_idx, pos_idx, neg_idx)
    nc = build_nc(plan["SPt"], plan["NEt"], plan["WNt"], plan["SPOFF"],
                  plan["NEOFF"], plan["WNOFF"],
                  plan["pass_main"] is not None,
                  plan["pass_p3"] is not None,
                  plan["pass_p4"] is not None)
    in_maps = make_in_maps(x, plan)
    res = run_bass_kernel_spmd(nc, in_maps, list(range(NCORES)))
    total = 0.0
    for k in range(NCORES):
        den = np.asarray(res.results[k]["den"], np.float64)   # [128, SPSUM]
        l0 = np.asarray(res.results[k]["l0"], np.float64)
        pm = np.asarray(plan["pairmask"][k], np.float64)
        total += float((pm * (np.log(den) - KSC * l0)).sum())
    total += host_fixup(x, anchor_idx, pos_idx, neg_idx, plan["bad_pairs"])
    return np.float32(total / P)


# revision 29
# speedup vs baseline: 1.2577x; 1.0079x over previous
"""Trainium2 Bass kernel for ContrastiveNet loss (v11).

Per core k of 8 (SPMD):
  - host: xn = x/||x||*S (S=32) in fp8e4 -> sim = G/(S^2*TEMP); no on-device
    normalization. Rows are PERMUTED: sorted by pair count and banded into
    tiles (tile 0 heaviest), dealt round-robin across cores, so the per-tile
    slot space NE_t shrinks (~694/520/350/180 instead of 4x694) and the last
    tile's tail is the lightest. Core k's 512 rows sit at rotated columns
    0..511 of its gram; y is COLUMN-CHUNK-major ([8][128][16][512]) so each
    chunk's gram (4 row-tiles x 8 kp fp8 DoubleRow matmuls into [128,512]
    PSUM) starts as the chunk lands; chunk 7 is dead last in the DMA stream.
  - gather: per (tile, piece 1024/1024/1024/512/512) gpsimd local_scatter
    with HBM col->slot planes (int16, -1 unused), accumulated per tile.
    2-member duplicate (row,col) groups are fixed by scatter passes:
    main pass (rep col in pieces 0-2, width NE_t) and narrow piece-3/4
    passes (their pairs ranked into the lowest slots). Pairs touching
    >=3-member groups (~2%) are masked out and computed exactly on host.
  - device ships den (softmax denominators) + l0 per pair; the final
    ln/mask/sum runs on host along with the masked pairs.
  - PE p-state: tiny warm matmuls bridge DMA-paced idle gaps.
"""
import os
import sys
import numpy as np
import ml_dtypes

try:
    import concourse  # noqa: F401
except ImportError:
    sys.path.insert(0, "/opt/trn_rl_repo")

from contextlib import ExitStack

import concourse.bass as bass
import concourse.tile as tile
from concourse import bacc, mybir
from concourse._compat import with_exitstack
from concourse.bass_utils import run_bass_kernel_spmd

F16 = np.float16
FP8 = ml_dtypes.float8_e4m3
F32 = mybir.dt.float32
DF16 = mybir.dt.float16
F8 = mybir.dt.float8e4
I16 = mybir.dt.int16

B, D, J = 4096, 2048, 11
NCORES, RPC, NT, NKP = 8, 512, 4, 8
NCH, CW = 8, 512                    # gram column chunks (per core)
POFF = [0, 1024, 2048, 3072, 3584]  # scatter piece offsets
PW = [1024, 1024, 1024, 512, 512]   # scatter piece widths
NP = 5
TEMP = 0.1
S = 32.0
KSC = 1.0 / (S * S * TEMP)
WARMS = [0, 51, 0, 85, 0, 0, 35, 0]
WARM0 = 130
AF = mybir.ActivationFunctionType
ALU = mybir.AluOpType
AX = mybir.AxisListType
DR = mybir.MatmulPerfMode.DoubleRow


def _even(n):
    return int(n) + (int(n) % 2)


# ---------------------------------------------------------------- host prep
def build_plan(anchor_idx, pos_idx, neg_idx):
    r0 = anchor_idx.astype(np.int64)
    cols0 = np.concatenate([pos_idx[:, None], neg_idx], axis=1).astype(np.int64)
    P = r0.shape[0]

    # ---- row permutation: band rows by pair count (heaviest -> tile 0),
    # deal each band round-robin across cores
    n0 = np.bincount(r0, minlength=B)
    order_rows = np.argsort(-n0, kind="stable")
    virt = np.empty(B, np.int64)      # original row -> virtual row
    ranks = np.arange(B)
    t_of = ranks // (128 * NCORES)
    i_in = ranks % (128 * NCORES)
    k_of = i_in % NCORES
    p_of = i_in // NCORES
    virt[order_rows] = k_of * RPC + t_of * 128 + p_of

    r = virt[r0]
    cols = virt[cols0]

    er = np.repeat(r, J)
    ec = cols.ravel()
    pair_of = np.repeat(np.arange(P), J)
    core = er // RPC
    t = (er % RPC) // 128
    pp = er % 128
    ec_rot = (ec - core * RPC) % B
    piece = np.searchsorted(POFF, ec_rot, side="right") - 1

    key = er * B + ec
    o2 = np.argsort(key, kind="stable")
    k_sorted = key[o2]
    first2 = np.r_[True, k_sorted[1:] != k_sorted[:-1]]
    gid_sorted = np.cumsum(first2) - 1
    NG = int(gid_sorted[-1]) + 1
    gid = np.empty(P * J, np.int64)
    gid[o2] = gid_sorted
    occ_sorted = np.arange(P * J) - np.flatnonzero(first2)[gid_sorted]
    gsz_g = np.bincount(gid_sorted, minlength=NG)
    gsz = gsz_g[gid]

    bad_pairs = np.unique(pair_of[gsz >= 3])
    badp = np.zeros(P, bool)
    badp[bad_pairs] = True

    # 2-member groups: rep = member in the earliest piece
    two = gsz == 2
    order = np.lexsort((np.arange(P * J), piece, gid))
    go = order[two[order]]
    g_of_go = gid[go]
    firstg = np.r_[True, g_of_go[1:] != g_of_go[:-1]]
    rep = go[firstg]
    oth = go[~firstg]
    rep_bad = badp[pair_of[rep]]
    oth_bad = badp[pair_of[oth]]
    swap = rep_bad & ~oth_bad
    pk = ~rep_bad & ~oth_bad
    rep_p, oth_p = rep[pk], oth[pk]
    grp_piece = piece[rep_p]
    narrow_g = grp_piece >= 3

    # pairs touching narrow-pass groups ranked first
    narrow_pairs = np.unique(np.r_[pair_of[rep_p[narrow_g]],
                                   pair_of[oth_p[narrow_g]]])
    sev = np.zeros(P, np.int64)
    sev[narrow_pairs] = 1
    order_p = np.lexsort((np.arange(P), -sev, r))
    r_sp = r[order_p]
    firstp = np.r_[True, r_sp[1:] != r_sp[:-1]]
    gidp = np.cumsum(firstp) - 1
    rank_sorted = np.arange(P) - np.flatnonzero(firstp)[gidp]
    srank = np.empty(P, np.int64)
    srank[order_p] = rank_sorted

    n_per_row = np.bincount(r, minlength=B)
    # per-band (tile) widths
    nmat = n_per_row.reshape(NCORES, NT, 128)
    SPt = [max(int(nmat[:, tt, :].max()), 1) for tt in range(NT)]
    NEt = [_even(SPt[tt] * J) for tt in range(NT)]
    assert max(NEt) * 32 < 2**16
    SPOFF = np.r_[0, np.cumsum(SPt)].astype(int)
    NEOFF = np.r_[0, np.cumsum(NEt)].astype(int)

    eslot = srank[pair_of] * J + np.tile(np.arange(J), P)

    if len(narrow_pairs):
        cnt_n = np.bincount(r[narrow_pairs], minlength=B)
    else:
        cnt_n = np.zeros(B, np.int64)
    cn = cnt_n.reshape(NCORES, NT, 128)
    WNt = [_even(min(int(cn[:, tt, :].max()) * J + 2, NEt[tt]))
           if cn[:, tt, :].max() > 0 else 0 for tt in range(NT)]
    WNOFF = np.r_[0, np.cumsum(WNt)].astype(int)

    # ---- main scatter plane: col -> slot of occ0 entries
    is_rep = np.ones(P * J, bool)
    is_rep[oth] = False
    is_rep[oth[swap]] = True
    m0 = is_rep & ~badp[pair_of]
    plane = np.full((NCORES, NT, 128, B), -1, np.int16)
    plane[core[m0], t[m0], pp[m0], ec_rot[m0]] = eslot[m0].astype(np.int16)
    plane_lv = [np.ascontiguousarray(
        plane[:, :, :, POFF[pc]:POFF[pc] + PW[pc]].transpose(0, 2, 1, 3))
        for pc in range(NP)]

    # ---- dup passes, packed per-tile widths
    e_rep, e_oth = eslot[rep_p], eslot[oth_p]

    def mk_pass(mask, widths, woff, check=True):
        tot = int(woff[-1])
        if tot == 0 or not mask.any():
            return None
        pl = np.full((NCORES, 128, tot), -1, np.int16)
        src = e_rep[mask]
        tgt = e_oth[mask]
        tts = t[rep_p[mask]]
        offs = np.asarray(woff)[tts]
        ww = np.asarray(widths)[tts]
        ok = (src < ww) & (tgt < ww)
        if check:
            assert ok.all(), "pass slot overflow"
        pl[core[rep_p[mask]][ok], pp[rep_p[mask]][ok],
           (offs + src)[ok]] = tgt[ok].astype(np.int16)
        return pl

    main_g = grp_piece <= 2
    pass_main = mk_pass(main_g, NEt, NEOFF)
    pass_p3 = mk_pass(grp_piece == 3, WNt, WNOFF)
    pass_p4 = mk_pass(grp_piece == 4, WNt, WNOFF)

    pairmask = np.zeros((NCORES, 128, int(SPOFF[-1])), F16)
    for tt in range(NT):
        pm_t = (np.arange(SPt[tt])[None, None, :] <
                nmat[:, tt, :][:, :, None]).astype(F16)
        pairmask[:, :, SPOFF[tt]:SPOFF[tt + 1]] = pm_t
    bp = bad_pairs
    tb = (r[bp] % RPC) // 128
    pairmask[r[bp] // RPC, r[bp] % 128,
             np.asarray(SPOFF)[tb] + srank[bp]] = 0

    return dict(plane_lv=plane_lv, pass_main=pass_main, pass_p3=pass_p3,
                pass_p4=pass_p4, pairmask=pairmask, order_rows=order_rows,
                SPt=SPt, NEt=NEt, WNt=WNt, SPOFF=SPOFF, NEOFF=NEOFF,
                WNOFF=WNOFF, bad_pairs=bad_pairs)


# ------------------------------------------------------------- device kernel
@with_exitstack
def _build(ctx: ExitStack, tc: "tile.TileContext", io: dict, SPt, NEt, WNt,
           SPOFF, NEOFF, WNOFF, have_main, have_p3, have_p4):
    nc = tc.nc
    y_d = io["y8"]
    SPSUM, NESUM, WNSUM = int(SPOFF[-1]), int(NEOFF[-1]), int(WNOFF[-1])

    consts = ctx.enter_context(tc.tile_pool(name="consts", bufs=1))
    wz = consts.tile([128, 2, 128], F8, tag="wz")
    nc.vector.memset(wz[:], 0.0)

    ypool = ctx.enter_context(tc.tile_pool(name="y", bufs=1))
    y = ypool.tile([128, NCH, 2 * NKP, CW], F8, tag="y", name="y")

    gpool = ctx.enter_context(tc.tile_pool(name="gbf", bufs=1))
    gbf = {tt: gpool.tile([128, B], DF16, tag=f"gbf{tt}", name=f"gbf{tt}")
           for tt in range(NT)}
    plpool = ctx.enter_context(tc.tile_pool(name="plane", bufs=1))
    pl = {pc: plpool.tile([128, NT, PW[pc]], I16, tag=f"plv{pc}",
                          name=f"plv{pc}") for pc in range(NP)}
    papool = ctx.enter_context(tc.tile_pool(name="passes", bufs=1))
    pam = papool.tile([128, NESUM], I16, tag="pam", name="pam") \
        if have_main else None
    pa3 = papool.tile([128, WNSUM], I16, tag="pa3", name="pa3") \
        if have_p3 else None
    pa4 = papool.tile([128, WNSUM], I16, tag="pa4", name="pa4") \
        if have_p4 else None

    lpool = ctx.enter_context(tc.tile_pool(name="loss", bufs=1))
    denall = lpool.tile([128, SPSUM], F32, tag="denall")
    l0all = lpool.tile([128, SPSUM], DF16, tag="l0all")

    # ---- DMA: chunk 7 dead last, planes just-in-time
    nc.sync.dma_start(y[:, 0], y_d[0])
    nc.sync.dma_start(y[:, 1], y_d[1])
    nc.sync.dma_start(pl[0][:], io["plane0"][:])
    nc.sync.dma_start(y[:, 2], y_d[2])
    nc.sync.dma_start(y[:, 3], y_d[3])
    nc.sync.dma_start(pl[1][:], io["plane1"][:])
    nc.sync.dma_start(pl[2][:, :, 0:CW], io["plane2"][:, :, 0:CW])
    nc.sync.dma_start(y[:, 4], y_d[4])
    nc.sync.dma_start(y[:, 5], y_d[5])
    nc.sync.dma_start(y[:, 6], y_d[6])
    nc.sync.dma_start(pl[2][:, :, CW:2 * CW], io["plane2"][:, :, CW:2 * CW])
    nc.sync.dma_start(pl[3][:], io["plane3"][:])
    nc.sync.dma_start(y[:, 7], y_d[7])
    nc.sync.dma_start(pl[4][:], io["plane4"][:])
    if have_main:
        nc.sync.dma_start(pam[:], io["passm"][:])
    if have_p3:
        nc.sync.dma_start(pa3[:], io["pass3"][:])
    if have_p4:
        nc.sync.dma_start(pa4[:], io["pass4"][:])

    dpool = ctx.enter_context(tc.tile_pool(name="dq", bufs=2))
    hpool = ctx.enter_context(tc.tile_pool(name="hacc", bufs=1))
    expool = ctx.enter_context(tc.tile_pool(name="extra", bufs=2))
    elpool = ctx.enter_context(tc.tile_pool(name="elb", bufs=2))
    hacc = {tt: hpool.tile([128, NEt[tt]], DF16, tag=f"hacc{tt}",
                           name=f"hacc{tt}") for tt in range(NT)}

    # preload the Exp activation table during the initial DMA idle
    pre = elpool.tile([128, 1], F32, tag="pre")
    nc.scalar.activation(pre[:], wz[:, 0, 0:1], AF.Exp)

    dq = {}

    def scatter_piece(pc, tiles=range(NT), off=0, w=None, key=None):
        w = PW[pc] if w is None else w
        key = pc if key is None else key
        for tt in tiles:
            d = dpool.tile([128, NEt[tt]], DF16, tag=f"d{tt}",
                           name=f"d{tt}_{key}")
            dq[(tt, key)] = d
            nc.gpsimd.local_scatter(
                d[:], gbf[tt][:, POFF[pc] + off:POFF[pc] + off + w],
                pl[pc][:, tt, off:off + w], 128, NEt[tt], w)

    with tc.tile_pool(name="gpsum", bufs=1, space="PSUM") as gpsum:
        wps0 = gpsum.tile([128, CW], F32, tag="ps0_1", name="warm_init")
        for i in range(WARM0):
            nc.tensor.matmul(wps0[:, 0:64], lhsT=wz[:], rhs=wz[:, :, 0:64],
                             start=True, stop=True, perf_mode=DR)
        for c in range(NCH):
            for tt in range(NT):
                ps = gpsum.tile([128, CW], F32, tag=f"ps{tt}_{c % 2}",
                                name=f"ps{tt}_{c}")
                for kp in range(NKP):
                    nc.tensor.matmul(
                        ps[:],
                        lhsT=y[:, 0, 2 * kp:2 * kp + 2, tt * 128:(tt + 1) * 128],
                        rhs=y[:, c, 2 * kp:2 * kp + 2, :],
                        start=(kp == 0), stop=(kp == NKP - 1),
                        perf_mode=DR,
                    )
                dst = gbf[tt][:, c * CW:(c + 1) * CW]
                if (c * NT + tt) % 2 == 0:
                    nc.vector.tensor_copy(dst, ps[:])
                else:
                    nc.scalar.copy(dst, ps[:])
            if WARMS[c]:
                wps = gpsum.tile([128, CW], F32, tag=f"ps0_{(c + 1) % 2}",
                                 name=f"warm{c}")
                for i in range(WARMS[c]):
                    nc.tensor.matmul(wps[:, 0:64], lhsT=wz[:],
                                     rhs=wz[:, :, 0:64],
                                     start=True, stop=True, perf_mode=DR)
            if c == 1:
                scatter_piece(0)
            elif c == 3:
                scatter_piece(1)
            elif c == 4:
                scatter_piece(2, off=0, w=CW, key="2a")
            elif c == 5:
                scatter_piece(2, off=CW, w=CW, key="2b")
                for tt in range(NT):
                    nc.vector.tensor_tensor(hacc[tt][:], dq[(tt, 0)][:],
                                            dq[(tt, 1)][:], ALU.add)
                for tt in range(NT):
                    nc.vector.tensor_tensor(hacc[tt][:], hacc[tt][:],
                                            dq[(tt, "2a")][:], ALU.add)
            elif c == 6:
                scatter_piece(3)
                for tt in range(NT):
                    nc.vector.tensor_tensor(hacc[tt][:], hacc[tt][:],
                                            dq[(tt, "2b")][:], ALU.add)

        # ---- tail: main dup pass (pre-chunk-7), piece-4 scatters, narrows
        if have_main:
            for tt in range(NT):
                NEc = NEt[tt]
                e = expool.tile([128, NEc], DF16, tag=f"eAm{tt % 2}",
                                name=f"eAm{tt}")
                nc.gpsimd.local_scatter(
                    e[:], hacc[tt][:], pam[:, NEOFF[tt]:NEOFF[tt + 1]],
                    128, NEc, NEc)
                nc.vector.tensor_tensor(hacc[tt][:], hacc[tt][:],
                                        e[:], ALU.add)
        scatter_piece(4)
        for tt in range(NT):
            u = expool.tile([128, NEt[tt]], DF16, tag=f"u{tt % 2}",
                            name=f"u{tt}")
            nc.vector.tensor_tensor(u[:], dq[(tt, 3)][:], dq[(tt, 4)][:],
                                    ALU.add)
            nc.vector.tensor_tensor(hacc[tt][:], hacc[tt][:], u[:], ALU.add)
        for tt in range(NT):
            NEc, WNc = NEt[tt], WNt[tt]
            if have_p3 and WNc:
                e = expool.tile([128, WNc], DF16, tag=f"eA3{tt % 2}",
                                name=f"eA3{tt}")
                nc.gpsimd.local_scatter(
                    e[:], dq[(tt, 3)][:, 0:WNc],
                    pa3[:, WNOFF[tt]:WNOFF[tt + 1]], 128, WNc, WNc)
                nc.vector.tensor_tensor(hacc[tt][:, 0:WNc],
                                        hacc[tt][:, 0:WNc], e[:], ALU.add)
            if have_p4 and WNc:
                e = expool.tile([128, WNc], DF16, tag=f"eA4{tt % 2}",
                                name=f"eA4{tt}")
                nc.gpsimd.local_scatter(
                    e[:], dq[(tt, 4)][:, 0:WNc],
                    pa4[:, WNOFF[tt]:WNOFF[tt + 1]], 128, WNc, WNc)
                nc.vector.tensor_tensor(hacc[tt][:, 0:WNc],
                                        hacc[tt][:, 0:WNc], e[:], ALU.add)
            ebuf = elpool.tile([128, NEt[0]], F32, tag="ebuf")
            nc.scalar.activation(ebuf[:, 0:NEc], hacc[tt][:], AF.Exp,
                                 scale=KSC)
            e3 = ebuf[:, 0:SPt[tt] * J].rearrange("p (s j) -> p s j", j=J)
            nc.vector.tensor_reduce(denall[:, SPOFF[tt]:SPOFF[tt + 1]], e3,
                                    AX.X, ALU.add)
            l0 = hacc[tt][:, 0:SPt[tt] * J].rearrange(
                "p (s j) -> p s j", j=J)[:, :, 0]
            nc.scalar.copy(l0all[:, SPOFF[tt]:SPOFF[tt + 1]], l0)

    # ---- ship den + l0; ln/mask/sum finish on host (separate queues so
    # descriptor generation overlaps; l0 completes before the last den)
    nc.scalar.dma_start(io["l0"][:], l0all[:])
    nc.sync.dma_start(io["den"][:], denall[:])


def build_nc(SPt, NEt, WNt, SPOFF, NEOFF, WNOFF, have_main, have_p3, have_p4,
             enable_asserts=False):
    nc = bacc.Bacc("TRN2", target_bir_lowering=False, debug=False,
                   enable_asserts=enable_asserts, num_devices=NCORES)
    SPSUM, NESUM, WNSUM = int(SPOFF[-1]), int(NEOFF[-1]), int(WNOFF[-1])
    io = {
        "y8": nc.dram_tensor("y8", [NCH, 128, 2 * NKP, CW], F8,
                             kind="ExternalInput").ap(),
        "den": nc.dram_tensor("den", [128, SPSUM], F32,
                              kind="ExternalOutput").ap(),
        "l0": nc.dram_tensor("l0", [128, SPSUM], DF16,
                             kind="ExternalOutput").ap(),
    }
    for pc in range(NP):
        io[f"plane{pc}"] = nc.dram_tensor(
            f"plane{pc}", [128, NT, PW[pc]], I16, kind="ExternalInput").ap()
    if have_main:
        io["passm"] = nc.dram_tensor("passm", [128, NESUM], I16,
                                     kind="ExternalInput").ap()
    if have_p3:
        io["pass3"] = nc.dram_tensor("pass3", [128, WNSUM], I16,
                                     kind="ExternalInput").ap()
    if have_p4:
        io["pass4"] = nc.dram_tensor("pass4", [128, WNSUM], I16,
                                     kind="ExternalInput").ap()
    with tile.TileContext(nc) as tc:
        _build(tc, io, SPt, NEt, WNt, SPOFF, NEOFF, WNOFF,
               have_main, have_p3, have_p4)
    nc.compile()
    return nc


def _normalize(x):
    x = np.asarray(x, np.float32)
    w = np.sqrt((x.astype(np.float64) ** 2).sum(axis=1, keepdims=True))
    w = np.maximum(w, 1e-8)
    return (x / w).astype(np.float32)


def make_in_maps(x, plan):
    xn = _normalize(x)
    x8 = np.clip(xn * S, -240.0, 240.0).astype(FP8)
    # virtual-row layout: band-rank i -> virtual row v (see build_plan)
    ranks = np.arange(B)
    t_of = ranks // (128 * NCORES)
    i_in = ranks % (128 * NCORES)
    k_of = i_in % NCORES
    p_of = i_in // NCORES
    v_of = k_of * RPC + t_of * 128 + p_of
    xvirt = np.empty_like(x8)
    xvirt[v_of] = x8[plan["order_rows"]]
    in_maps = []
    for k in range(NCORES):
        xr = np.roll(xvirt, -RPC * k, axis=0)                  # [B, D]
        y8 = xr.T.reshape(2 * NKP, 128, B).transpose(1, 0, 2)  # [128, 16, B]
        y8c = np.ascontiguousarray(
            y8.reshape(128, 2 * NKP, NCH, CW).transpose(2, 0, 1, 3))
        m = {"y8": y8c}
        for pc in range(NP):
            m[f"plane{pc}"] = plan["plane_lv"][pc][k]
        if plan["pass_main"] is not None:
            m["passm"] = plan["pass_main"][k]
        if plan["pass_p3"] is not None:
            m["pass3"] = plan["pass_p3"][k]
        if plan["pass_p4"] is not None:
            m["pass4"] = plan["pass_p4"][k]
        in_maps.append(m)
    return in_maps


def host_fixup(x, anchor_idx, pos_idx, neg_idx, bad_pairs):
    if len(bad_pairs) == 0:
        return 0.0
    xn = _normalize(x).astype(np.float64)
    a = anchor_idx[bad_pairs]
    cols = np.concatenate([pos_idx[bad_pairs][:, None], neg_idx[bad_pairs]],
                          axis=1)
    logits = np.einsum("pd,pjd->pj", xn[a], xn[cols]) / TEMP
    mx = logits.max(axis=1, keepdims=True)
    lse = np.log(np.exp(logits - mx).sum(axis=1)) + mx[:, 0]
    return float((lse - logits[:, 0]).sum())


def kernel(**inputs):
    x = np.asarray(inputs["x"], np.float32)
    anchor_idx = np.asarray(inputs["anchor_idx"])
    pos_idx = np.asarray(inputs["pos_idx"])
    neg_idx = np.asarray(inputs["neg_idx"])
    P = anchor_idx.shape[0]

    plan = build_plan(anchor_idx, pos_idx, neg_idx)
    nc = build_nc(plan["SPt"], plan["NEt"], plan["WNt"], plan["SPOFF"],
                  plan["NEOFF"], plan["WNOFF"],
                  plan["pass_main"] is not None,
                  plan["pass_p3"] is not None,
                  plan["pass_p4"] is not None)
    in_maps = make_in_maps(x, plan)
    res = run_bass_kernel_spmd(nc, in_maps, list(range(NCORES)))
    total = 0.0
    for k in range(NCORES):
        den = np.asarray(res.results[k]["den"], np.float64)   # [128, SPSUM]
        l0 = np.asarray(res.results[k]["l0"], np.float64)
        pm = np.asarray(plan["pairmask"][k], np.float64)
        total += float((pm * (np.log(den) - KSC * l0)).sum())
    total += host_fixup(x, anchor_idx, pos_idx, neg_idx, plan["bad_pairs"])
    return np.float32(total / P)
